# revision 1
# baseline (speedup 1.0000x reference)
"""Trainium2 Bass kernel for nn_FNO_RC_1D (1D FNO + Chebyshev-Fourier residual
correction). Data-parallel over batch: 32 samples -> 8 cores x 4 samples.

Factorization (per sample, h is [128 ch, 8192 s] bf16 in SBUF):
  - fc0 as K=2 matmul vs [x; grid]
  - spectral conv: rfft truncated to 32 modes == h @ F (F: [S, 64] cos/-sin),
    done as 64 chunk-matmuls with lhsT = hT chunks (DMA xbar transpose of h),
    per-mode complex mixing as 64 small matmuls (sw weights stationary,
    staged xf moving, N=8), irfft of 32 modes == ofT.T @ Cinv ([64, S]);
    inverse-DFT and 1x1-conv matmuls accumulate into the same PSUM so
    bias + GELU is a single ScalarE pass PSUM->SBUF.
  - CFT/latent path computed exactly: 72 extra basis columns ride along the
    layer-3 forward DFT; cg1_w is folded over the L-broadcast on host.
  - fc1 (+GELU) chunk-wise; fc2 transposed (out [s-chunk, 1] per chunk) so the
    final [S] vector lands across partitions.
All matmul operands bf16, accumulation fp32 in PSUM.
"""

from contextlib import ExitStack

import numpy as np
import ml_dtypes

B, S, WIDTH, MODES = 32, 8192, 128, 32
CFT_MODES, L_SEG, M_CHEB = 4, 2, 4
NCORES = 8
BPC = B // NCORES  # samples per core
NCH = S // 128     # 64 chunks
BF = ml_dtypes.bfloat16

_CACHE = {}


def _cheb_basis(n, m):
    t = np.linspace(-1.0, 1.0, n)
    Ts = [np.ones(n), t]
    for _ in range(2, m):
        Ts.append(2.0 * t * Ts[-1] - Ts[-2])
    return np.stack(Ts[:m], 0).astype(np.float32)


def _host_consts():
    s = np.arange(S, dtype=np.float64)
    k = np.arange(MODES, dtype=np.float64)
    ang = 2.0 * np.pi * np.outer(s, k) / S
    F = np.concatenate([np.cos(ang), -np.sin(ang)], axis=1)          # [S, 64]
    ck = np.full(MODES, 2.0 / S); ck[0] = 1.0 / S
    Cinv = np.empty((2 * MODES, S), np.float64)                       # interleaved
    Cinv[0::2] = ck[:, None] * np.cos(ang.T)
    Cinv[1::2] = -ck[:, None] * np.sin(ang.T)
    T = _cheb_basis(S, M_CHEB).astype(np.float64)                     # [4, S]
    kk = np.arange(-CFT_MODES, CFT_MODES + 1, dtype=np.float64)
    ph = np.pi * np.outer(s, kk) / S
    CH = np.empty((S, M_CHEB, 2 * CFT_MODES + 1, 2), np.float64)
    CH[..., 0] = T.T[:, :, None] * np.cos(ph)[:, None, :]
    CH[..., 1] = T.T[:, :, None] * (-np.sin(ph))[:, None, :]
    CH = (CH / S).reshape(S, 72)
    # chunk-major layouts [128 p, 64 t, cols]: row s = t*128 + p
    F_sb = F.reshape(NCH, 128, 64).transpose(1, 0, 2).astype(BF)
    CH_sb = CH.reshape(NCH, 128, 72).transpose(1, 0, 2).astype(BF)
    grid = np.linspace(0.0, 1.0, S, dtype=np.float32)
    return F_sb, CH_sb, Cinv.astype(BF), grid


def _build():
    import concourse.bacc as bacc
    import concourse.tile as tile
    import concourse.mybir as mybir
    from concourse.masks import make_identity

    f32 = mybir.dt.float32
    bf16 = mybir.dt.bfloat16
    GELU = mybir.ActivationFunctionType.Gelu
    IDENT = mybir.ActivationFunctionType.Identity

    nc = bacc.Bacc("TRN2", target_bir_lowering=False)

    # ---- DRAM tensors ----
    d_xg = nc.dram_tensor("xg", [2 * BPC, S], bf16, kind="ExternalInput")
    d_fc0w = nc.dram_tensor("fc0w", [8, 4, 128], bf16, kind="ExternalInput")
    d_F = nc.dram_tensor("Fb", [128, NCH, 64], bf16, kind="ExternalInput")
    d_CH = nc.dram_tensor("CHb", [128, NCH, 72], bf16, kind="ExternalInput")
    d_Ci = nc.dram_tensor("Cinv", [64, S], bf16, kind="ExternalInput")
    d_WT = nc.dram_tensor("WT", [128, 4, 128], bf16, kind="ExternalInput")
    d_SW = nc.dram_tensor("SW", [4, 128, MODES, 2, 128], bf16, kind="ExternalInput")
    d_G = nc.dram_tensor("G2", [128, 72, 256], bf16, kind="ExternalInput")
    d_fc1w = nc.dram_tensor("fc1w", [128, 128], bf16, kind="ExternalInput")
    d_fc2w = nc.dram_tensor("fc2w", [128, 1], bf16, kind="ExternalInput")
    d_cg2h = nc.dram_tensor("cg2h", [128, 2, 128], bf16, kind="ExternalInput")
    d_fc0b = nc.dram_tensor("fc0b", [128, 1], f32, kind="ExternalInput")
    d_lb = nc.dram_tensor("lb", [128, 3], f32, kind="ExternalInput")     # w0..w2 bias
    d_w3b = nc.dram_tensor("w3b", [128, 1], f32, kind="ExternalInput")   # w3_b+cg2_b
    d_fc1b = nc.dram_tensor("fc1b", [128, 1], f32, kind="ExternalInput")
    d_cg1b = nc.dram_tensor("cg1b", [4, 256], f32, kind="ExternalInput")
    d_out = nc.dram_tensor("out", [BPC, S], f32, kind="ExternalOutput")

    with ExitStack() as ctx:
        tc = ctx.enter_context(tile.TileContext(nc))
        consts = ctx.enter_context(tc.tile_pool(name="consts", bufs=1))
        hpool = ctx.enter_context(tc.tile_pool(name="h", bufs=1))
        htp = ctx.enter_context(tc.tile_pool(name="ht", bufs=3))
        swp = ctx.enter_context(tc.tile_pool(name="sw", bufs=2))
        gp = ctx.enter_context(tc.tile_pool(name="g", bufs=2))
        outp = ctx.enter_context(tc.tile_pool(name="outc", bufs=3))
        stg = ctx.enter_context(tc.tile_pool(name="stg", bufs=1))
        pz = ctx.enter_context(tc.tile_pool(name="pz", bufs=2, space="PSUM"))
        pxf = ctx.enter_context(tc.tile_pool(name="pxf", bufs=2, space="PSUM"))
        pof = ctx.enter_context(tc.tile_pool(name="pof", bufs=1, space="PSUM"))
        psm = ctx.enter_context(tc.tile_pool(name="psm", bufs=1, space="PSUM"))

        sy, gs = nc.sync, nc.gpsimd

        # ---- constants into SBUF ----
        xg = consts.tile([2 * BPC, S], bf16); sy.dma_start(xg, d_xg[:, :])
        fc0w = consts.tile([8, 4, 128], bf16); sy.dma_start(fc0w, d_fc0w[:, :, :])
        Fb = consts.tile([128, NCH, 64], bf16); sy.dma_start(Fb, d_F[:, :, :])
        CHb = consts.tile([128, NCH, 72], bf16); sy.dma_start(CHb, d_CH[:, :, :])
        Ci = consts.tile([64, S], bf16); sy.dma_start(Ci, d_Ci[:, :])
        WT = consts.tile([128, 4, 128], bf16); sy.dma_start(WT, d_WT[:, :, :])
        fc1w = consts.tile([128, 128], bf16); sy.dma_start(fc1w, d_fc1w[:, :])
        fc2w = consts.tile([128, 1], bf16); sy.dma_start(fc2w, d_fc2w[:, :])
        cg2h = consts.tile([128, 2, 128], bf16); sy.dma_start(cg2h, d_cg2h[:, :, :])
        fc0b = consts.tile([128, 1], f32); sy.dma_start(fc0b, d_fc0b[:, :])
        lb = consts.tile([128, 3], f32); sy.dma_start(lb, d_lb[:, :])
        w3b = consts.tile([128, 1], f32); sy.dma_start(w3b, d_w3b[:, :])
        fc1b = consts.tile([128, 1], f32); sy.dma_start(fc1b, d_fc1b[:, :])
        cg1b = consts.tile([4, 256], f32); sy.dma_start(cg1b, d_cg1b[:, :])
        ident = consts.tile([128, 128], bf16); make_identity(nc, ident)

        hs = [hpool.tile([128, S], bf16, tag=f"h{b}", name=f"h{b}")
              for b in range(BPC)]
        A = consts.tile([128, 256], bf16)      # staged (xr, xi) per (k, b)
        Bs = consts.tile([128, 256], bf16)     # staged (-xi, xr)
        feats = consts.tile([128, 288], bf16)  # cft feats [c, (q, b)]
        ofn = consts.tile([128, 256], bf16)    # of natural copy
        ofTs = [consts.tile([64, 128], bf16, tag=f"ofT{b}", name=f"ofT{b}")
                for b in range(BPC)]
        latb = consts.tile([128, BPC], f32)

        # ---- fc0: h0 = fc0_w.T @ [x; grid] + fc0_b ----
        for b in range(BPC):
            for w in range(8):  # windows of 1024
                zt = pz.tile([128, 1024], f32, tag="z")
                for q in range(2):
                    nc.tensor.matmul(
                        zt[:, q * 512:(q + 1) * 512], fc0w[:, b, :],
                        xg[:, w * 1024 + q * 512:w * 1024 + (q + 1) * 512],
                        start=True, stop=True)
                if w % 2 == 0:
                    nc.scalar.activation(hs[b][:, w * 1024:(w + 1) * 1024], zt,
                                         IDENT, bias=fc0b[:, 0:1])
                else:
                    nc.vector.tensor_scalar_add(
                        hs[b][:, w * 1024:(w + 1) * 1024], zt, fc0b[:, 0:1])

        # ---- layers ----
        for l in range(4):
            sw = swp.tile([128, MODES, 2, 128], bf16, tag="sw")
            gs.dma_start(sw, d_SW[l, :, :, :, :])
            # phase 1: transpose + forward DFT (+ CFT at l==3)
            for b in range(BPC):
                xfp = pxf.tile([128, 136], f32, tag="xf")
                if l == 3:
                    cftp = psm.tile([128, 72], f32, tag="sm")
                for hh in range(2):
                    ht = htp.tile([128, 32, 128], bf16, tag="ht")
                    teng = sy if hh == 0 else nc.scalar
                    teng.dma_start(ht, hs[b][:, hh * 4096:(hh + 1) * 4096],
                                   transpose=True)
                    for t in range(32):
                        tg = hh * 32 + t
                        nc.tensor.matmul(xfp[:, 0:64], ht[:, t, :], Fb[:, tg, :],
                                         start=(tg == 0), stop=(tg == 63))
                        if l == 3:
                            nc.tensor.matmul(cftp, ht[:, t, :],
                                             CHb[:, tg, :],
                                             start=(tg == 0), stop=(tg == 63))
                # stage xf -> A/B (bf16, strided col writes), negate xi for B
                nc.vector.tensor_copy(A[:, 2 * b:256:8], xfp[:, 0:32])
                nc.vector.tensor_copy(A[:, 2 * b + 1:256:8], xfp[:, 32:64])
                nc.vector.tensor_copy(Bs[:, 2 * b + 1:256:8], xfp[:, 0:32])
                nc.vector.tensor_scalar_mul(Bs[:, 2 * b:256:8], xfp[:, 32:64], -1.0)
                if l == 3:
                    nc.vector.tensor_copy(feats[:, b:288:4], cftp)

            # phase 2: mode mixing -> of_nat [o, (k, b, re/im)]
            ofp = pof.tile([128, 256], f32, tag="of")
            for k in range(MODES):
                nc.tensor.matmul(ofp[:, 8 * k:8 * k + 8], sw[:, k, 0, :],
                                 A[:, 8 * k:8 * k + 8], start=True, stop=False)
                nc.tensor.matmul(ofp[:, 8 * k:8 * k + 8], sw[:, k, 1, :],
                                 Bs[:, 8 * k:8 * k + 8], start=False, stop=True)
            ofp3 = ofp.rearrange("p (k g) -> p k g", g=8)
            for b in range(BPC):
                # contiguous [128, 64] staging of sample b's (k, re/im) cols
                nc.vector.tensor_copy(ofn[:, 64 * b:64 * (b + 1)],
                                      ofp3[:, :, 2 * b:2 * b + 2])
                otp = psm.tile([64, 128], bf16, tag="sm")
                nc.tensor.transpose(otp, ofn[:, 64 * b:64 * (b + 1)], ident)
                nc.vector.tensor_copy(ofTs[b], otp)

            # latent path (l == 3): needs feats, runs before fno drains
            if l == 3:
                tps = pxf.tile([4, 256], f32, tag="xf")
                for qc in range(9):
                    gt = gp.tile([128, 8, 256], bf16, tag="G")
                    gs.dma_start(gt, d_G[:, qc * 8:(qc + 1) * 8, :])
                    for qq in range(8):
                        q = qc * 8 + qq
                        nc.tensor.matmul(tps, feats[:, 4 * q:4 * q + 4],
                                         gt[:, qq, :],
                                         start=(q == 0), stop=(q == 71))
                tsb = stg.tile([4, 256], f32)
                nc.vector.tensor_add(tsb, tps, cg1b)
                tgb = stg.tile([4, 256], bf16)
                nc.scalar.activation(tgb, tsb, GELU)
                lps = pof.tile([128, BPC], f32, tag="of")
                for hh in range(2):
                    ttp = psm.tile([128, 4], bf16, tag="sm")
                    nc.tensor.transpose(ttp, tgb[:, hh * 128:(hh + 1) * 128],
                                        ident[0:4, 0:4])
                    tgT = stg.tile([128, 4], bf16, tag=f"tgT{hh}")
                    nc.vector.tensor_copy(tgT, ttp)
                    nc.tensor.matmul(lps, cg2h[:, hh, :], tgT,
                                     start=(hh == 0), stop=(hh == 1))
                nc.vector.tensor_scalar_add(latb, lps, w3b[:, 0:1])

            # phase 3: z = invDFT + pointwise; drain (gelu / fno+fc1+fc2)
            for b in range(BPC):
                if l == 3:
                    f2ps = psm.tile([128, 64], f32, tag="sm")
                for w in range(8):  # windows of 1024
                    zt = pz.tile([128, 1024], f32, tag="z")
                    for q in range(2):
                        sl = slice(w * 1024 + q * 512, w * 1024 + (q + 1) * 512)
                        nc.tensor.matmul(zt[:, q * 512:(q + 1) * 512],
                                         ofTs[b], Ci[:, sl], start=True, stop=False)
                        nc.tensor.matmul(zt[:, q * 512:(q + 1) * 512],
                                         WT[:, l, :], hs[b][:, sl],
                                         start=False, stop=True)
                    if l < 3:
                        nc.scalar.activation(hs[b][:, w * 1024:(w + 1) * 1024], zt,
                                             GELU, bias=lb[:, l:l + 1])
                    else:
                        oc = outp.tile([128, 1024], bf16, tag="oc")
                        nc.vector.tensor_scalar_add(oc, zt, latb[:, b:b + 1])
                        # fc1 + gelu + fc2 chunk-wise
                        fps = pz.tile([128, 1024], f32, tag="z")
                        for q in range(2):
                            nc.tensor.matmul(fps[:, q * 512:(q + 1) * 512], fc1w,
                                             oc[:, q * 512:(q + 1) * 512],
                                             start=True, stop=True)
                        g1 = outp.tile([128, 1024], bf16, tag="g1")
                        nc.scalar.activation(g1, fps, GELU, bias=fc1b[:, 0:1])
                        for q in range(8):
                            tg = w * 8 + q
                            nc.tensor.matmul(f2ps[:, tg:tg + 1],
                                             g1[:, q * 128:(q + 1) * 128], fc2w,
                                             start=True, stop=True)
                if l == 3:
                    f2sb = outp.tile([128, 64], f32, tag="f2sb")
                    nc.vector.tensor_copy(f2sb, f2ps)
                    sy.dma_start(d_out[b, :].rearrange("(t p) -> p t", p=128), f2sb)

    nc.compile()
    return nc


def _fc0_blk(fc0_w):
    blk = np.zeros((8, 4, 128), np.float32)
    for b in range(BPC):
        blk[2 * b, b, :] = fc0_w[0]
        blk[2 * b + 1, b, :] = fc0_w[1]
    return blk.astype(BF)


def _prep(inputs):
    inp = {k: np.asarray(v) for k, v in inputs.items()}
    F_sb, CH_sb, Ci, grid = _host_consts()
    x = inp["x"].astype(np.float32)  # [32, 8192, 1]
    fc0_w = inp["fc0_w"].astype(np.float32)
    WT = np.stack([inp[f"w{i}_w"].astype(np.float32).T for i in range(4)], 1)
    SW = np.empty((4, 128, MODES, 2, 128), np.float32)
    for i in range(4):
        sw = np.asarray(inp[f"sw{i}"])
        SW[i, :, :, 0, :] = np.ascontiguousarray(sw.real).transpose(0, 2, 1)
        SW[i, :, :, 1, :] = np.ascontiguousarray(sw.imag).transpose(0, 2, 1)
    cg1 = inp["cg1_w"].astype(np.float32).reshape(WIDTH, M_CHEB, L_SEG, 9, 2, 256)
    G2 = cg1.sum(axis=2).reshape(WIDTH, 72, 256)
    lb = np.stack([inp[f"w{i}_b"].astype(np.float32) for i in range(3)], 1)
    common = {
        "fc0w": _fc0_blk(fc0_w),
        "Fb": F_sb, "CHb": CH_sb, "Cinv": Ci,
        "WT": WT.astype(BF),
        "SW": SW.astype(BF),
        "G2": G2.astype(BF),
        "fc1w": inp["fc1_w"].astype(np.float32).astype(BF),
        "fc2w": inp["fc2_w"].astype(np.float32).astype(BF),
        "cg2h": inp["cg2_w"].astype(np.float32).reshape(2, 128, 128)
                .transpose(1, 0, 2).copy().astype(BF),
        "fc0b": inp["fc0_b"].astype(np.float32).reshape(128, 1),
        "lb": lb,
        "w3b": (inp["w3_b"].astype(np.float32)
                + inp["cg2_b"].astype(np.float32)).reshape(128, 1),
        "fc1b": inp["fc1_b"].astype(np.float32).reshape(128, 1),
        "cg1b": np.broadcast_to(inp["cg1_b"].astype(np.float32), (4, 256)).copy(),
    }
    per_core = []
    for c in range(NCORES):
        xg = np.empty((2 * BPC, S), np.float32)
        for b in range(BPC):
            xg[2 * b] = x[c * BPC + b, :, 0]
            xg[2 * b + 1] = grid
        m = dict(common)
        m["xg"] = xg.astype(BF)
        per_core.append(m)
    fc2b = float(inp["fc2_b"].astype(np.float32).reshape(-1)[0])
    return per_core, fc2b


def kernel(**inputs) -> np.ndarray:
    from concourse import bass_utils
    per_core, fc2b = _prep(inputs)
    if "nc" not in _CACHE:
        _CACHE["nc"] = _build()
    nc = _CACHE["nc"]
    res = bass_utils.run_bass_kernel_spmd(nc, per_core, core_ids=list(range(NCORES)))
    out = np.empty((B, S, 1), np.float32)
    for c in range(NCORES):
        out[c * BPC:(c + 1) * BPC, :, 0] = res.results[c]["out"]
    return out + fc2b



# revision 54
# speedup vs baseline: 2.5962x; 2.5962x over previous
"""Trainium2 Bass kernel for nn_FNO_RC_1D (1D FNO + Chebyshev-Fourier residual
correction). Data-parallel over batch: 32 samples -> 8 cores x 4 samples.

Fast path (used when cg2_w == 0, the problem's zero-init correction head, so
latent == cg2_b and the whole CFT path folds into the fc1 bias):
  - layer 0 is folded through fc0: xf0 = fc0_w.T @ (Xg @ F) via a host-side
    transposed copy of [x; grid], and the 1x1 conv term is M0 @ Xg with
    M0 = W0 @ fc0_w.T -- h0 is never materialized in either orientation.
  - spectral conv per layer: forward DFT of 32 modes as chunked matmuls
    against hT (DMA xbar transpose of h), per-mode complex mixing as small
    matmuls, and the inverse DFT *fused into the 1x1 conv PSUM pass* as a
    single fp8 DoubleRow matmul (of/Ci quantized to scaled e4m3; the W h
    term rides the same accumulation in bf16 with power-of-2-scaled weights,
    unscaled exactly by the activation's scale argument).
  - gelu drains on ScalarE write h in place; transposes for the next layer
    issue mid-drain so the xbar DMAs overlap the remaining windows; the
    next layer's forward DFT is software-pipelined one sample behind.
  - final block: z -> oc on VectorE, fc1+gelu, fc2 as per-chunk columns.
All big matmuls bf16 (fp8 only where quantization error is provably small),
accumulation fp32 in PSUM.

A general fallback (original baseline kernel) handles nonzero cg2_w.
"""

from contextlib import ExitStack

import numpy as np
import ml_dtypes

B, S, WIDTH, MODES = 32, 8192, 128, 32
CFT_MODES, L_SEG, M_CHEB = 4, 2, 4
CFT_DIM = (2 * CFT_MODES + 1) * L_SEG * M_CHEB * WIDTH * 2
NCORES = 8
BPC = B // NCORES  # samples per core
NCH = S // 128     # 64 chunks
BF = ml_dtypes.bfloat16
F8 = ml_dtypes.float8_e4m3   # TRN float8e4: max finite 240, 256 -> inf

OF_EXP = [7, 10, 14, 17]     # per-layer scale exponent for of -> fp8
CI_EXP = 18                  # scale exponent for Cinv -> fp8
SW_EXP = 21                  # scale exponent for spectral weights -> fp8
# +1: the DoubleRow irfft sums two identical slot products (2x the result)
E_L = [o + CI_EXP + 1 for o in OF_EXP]

_CACHE = {}


def _cheb_basis(n, m):
    t = np.linspace(-1.0, 1.0, n)
    Ts = [np.ones(n), t]
    for _ in range(2, m):
        Ts.append(2.0 * t * Ts[-1] - Ts[-2])
    return np.stack(Ts[:m], 0).astype(np.float32)


def _fourier_bases():
    s = np.arange(S, dtype=np.float64)
    k = np.arange(MODES, dtype=np.float64)
    ang = 2.0 * np.pi * np.outer(s, k) / S
    F = np.concatenate([np.cos(ang), -np.sin(ang)], axis=1)  # [S, 64]
    ck = np.full(MODES, 2.0 / S); ck[0] = 1.0 / S
    Ci = np.empty((2 * MODES, S), np.float64)                # interleaved re/im
    Ci[0::2] = ck[:, None] * np.cos(ang.T)
    Ci[1::2] = -ck[:, None] * np.sin(ang.T)
    return F, Ci


# ---------------------------------------------------------------------------
# fast path (cg2_w == 0)
# ---------------------------------------------------------------------------

def _build_v2(has_fc0b):
    import concourse.bacc as bacc
    import concourse.tile as tile
    import concourse.mybir as mybir
    from concourse.masks import make_identity

    f32 = mybir.dt.float32
    bf16 = mybir.dt.bfloat16
    fp8 = mybir.dt.float8e4
    GELU = mybir.ActivationFunctionType.Gelu
    DR = mybir.MatmulPerfMode.DoubleRow

    nc = bacc.Bacc("TRN2", target_bir_lowering=False)

    # ---- DRAM tensors ----
    d_xg = nc.dram_tensor("xg", [2 * BPC, S], bf16, kind="ExternalInput")
    d_xgT = nc.dram_tensor("xgT", [128, NCH, 2 * BPC], bf16, kind="ExternalInput")
    d_F = nc.dram_tensor("Fb", [128, NCH, 64], fp8, kind="ExternalInput")
    d_Ci8 = nc.dram_tensor("Ci8", [64, 2, S], fp8, kind="ExternalInput")
    d_WT = nc.dram_tensor("WT", [128, 4, 128], bf16, kind="ExternalInput")
    d_M0T = nc.dram_tensor("M0T", [8, 4, 128], bf16, kind="ExternalInput")
    d_fc0w2 = nc.dram_tensor("fc0w2", [8, 4, 128], bf16, kind="ExternalInput")
    d_SW = nc.dram_tensor("SW", [4, 128, MODES, 2, 128], fp8, kind="ExternalInput")
    d_fc1w = nc.dram_tensor("fc1w", [128, 128], bf16, kind="ExternalInput")
    d_fc2w = nc.dram_tensor("fc2w", [128, 1], bf16, kind="ExternalInput")
    d_lb = nc.dram_tensor("lb", [128, 3], f32, kind="ExternalInput")
    d_fc1b = nc.dram_tensor("fc1b", [128, 1], f32, kind="ExternalInput")
    if has_fc0b:
        d_r1 = nc.dram_tensor("r1", [1, 192], bf16, kind="ExternalInput")
    d_out = nc.dram_tensor("out", [BPC, 128, 64], f32, kind="ExternalOutput")

    with ExitStack() as ctx:
        tc = ctx.enter_context(tile.TileContext(nc))
        consts = ctx.enter_context(tc.tile_pool(name="consts", bufs=1))
        hpool = ctx.enter_context(tc.tile_pool(name="h", bufs=1))
        htp = ctx.enter_context(tc.tile_pool(name="ht", bufs=10))
        swp = ctx.enter_context(tc.tile_pool(name="sw", bufs=4))
        outp = ctx.enter_context(tc.tile_pool(name="outc", bufs=8))
        stg = ctx.enter_context(tc.tile_pool(name="stg", bufs=1))
        pz = ctx.enter_context(tc.tile_pool(name="pz", bufs=2, space="PSUM"))
        pf1 = ctx.enter_context(tc.tile_pool(name="pf1", bufs=2, space="PSUM"))
        pof = ctx.enter_context(tc.tile_pool(name="pof", bufs=1, space="PSUM"))
        psm = ctx.enter_context(tc.tile_pool(name="psm", bufs=1, space="PSUM"))

        sy, gs = nc.sync, nc.gpsimd

        # ---- constants into SBUF ----
        # order matters for the startup critical path: phase1-l0 needs
        # xgT+Fb, phase2-l0 needs SW (gs queue), phase3-l0 needs Ci8+xg.
        xgT = consts.tile([128, NCH, 2 * BPC], bf16); sy.dma_start(xgT, d_xgT[:, :, :])
        Fb = consts.tile([128, NCH, 64], fp8); sy.dma_start(Fb, d_F[:, :, :])
        Ci8 = consts.tile([64, 2, S], fp8)
        xg = consts.tile([2 * BPC, S], bf16); sy.dma_start(xg, d_xg[:, :])
        WT = consts.tile([128, 4, 128], bf16); sy.dma_start(WT, d_WT[:, :, :])
        M0T = consts.tile([8, 4, 128], bf16); sy.dma_start(M0T, d_M0T[:, :, :])
        fc0w2 = consts.tile([8, 4, 128], bf16); sy.dma_start(fc0w2, d_fc0w2[:, :, :])
        fc1w = consts.tile([128, 128], bf16); sy.dma_start(fc1w, d_fc1w[:, :])
        fc2w = consts.tile([128, 1], bf16); sy.dma_start(fc2w, d_fc2w[:, :])
        lb = consts.tile([128, 3], f32); sy.dma_start(lb, d_lb[:, :])
        fc1b = consts.tile([128, 1], f32); sy.dma_start(fc1b, d_fc1b[:, :])
        if has_fc0b:
            r1 = consts.tile([1, 192], bf16); sy.dma_start(r1, d_r1[:, :])
        ident = consts.tile([128, 128], bf16); make_identity(nc, ident)

        hs = [hpool.tile([128, S], bf16, tag=f"h{b}", name=f"h{b}")
              for b in range(BPC)]
        A = consts.tile([128, 256], bf16)      # staged (xr, xi) per (k, b)
        Bs = consts.tile([128, 256], bf16)     # staged (-xi, xr)
        ofn = consts.tile([128, 256], bf16)    # of natural staging per sample
        ofT8 = consts.tile([64, BPC, 2, 128], fp8)  # DoubleRow lhsT per sample

        sw_tiles = {}

        def prefetch_sw(l):
            t = swp.tile([128, MODES, 2, 128], fp8, tag="sw", name=f"sw{l}")
            gs.dma_start(t, d_SW[l, :, :, :, :])
            sw_tiles[l] = t

        def stage_ab(b, xfp):
            nc.vector.tensor_copy(A[:, 2 * b:256:8], xfp[:, 0:32])
            nc.vector.tensor_copy(A[:, 2 * b + 1:256:8], xfp[:, 32:64])
            nc.vector.tensor_copy(Bs[:, 2 * b + 1:256:8], xfp[:, 0:32])
            nc.vector.tensor_scalar_mul(Bs[:, 2 * b:256:8], xfp[:, 32:64], -1.0)

        # of_tile(lp): one PSUM bank per layer epoch -- cols 0:256 hold the
        # mixed spectral outputs (k-major, per-sample col pairs), cols
        # 256:320 are the forward-DFT accumulator (reused per sample).
        of_tiles = {}

        def of_tile(lp):
            if lp not in of_tiles:
                of_tiles[lp] = pof.tile([128, 512], f32, tag="of",
                                        name=f"of{lp}")
            return of_tiles[lp]

        def mix_sample(lp, b):
            # sample b's full phase-2 chain: mixing -> ofn -> transpose -> fp8
            sw = sw_tiles[lp]
            ot = of_tiles[lp]
            for k in range(MODES):
                c = 8 * k + 2 * b
                nc.tensor.matmul(ot[:, c:c + 2], sw[:, k, 0, :],
                                 A[:, c:c + 2], start=True, stop=False)
                nc.tensor.matmul(ot[:, c:c + 2], sw[:, k, 1, :],
                                 Bs[:, c:c + 2], start=False, stop=True)
            of3 = ot[:, 0:256].rearrange("p (k g) -> p k g", g=8)
            nc.vector.tensor_copy(ofn[:, 64 * b:64 * (b + 1)],
                                  of3[:, :, 2 * b:2 * b + 2])
            otp = psm.tile([64, 128], bf16, tag="sm")
            nc.tensor.transpose(otp, ofn[:, 64 * b:64 * (b + 1)], ident)
            sc = float(2.0 ** (OF_EXP[lp] - SW_EXP))
            nc.vector.tensor_scalar_mul(ofT8[:, b, 0, :], otp, sc)
            nc.vector.tensor_scalar_mul(ofT8[:, b, 1, :], otp, sc)

        ht_q = {}
        dft_pending = []   # [lp, b, q, issue_window]; >= 4-window consume lag
        gw = [0]           # global window counter

        def issue_t(lp, b, q):
            tq = htp.tile([128, 16, 128], bf16, tag="ht")
            sy.dma_start(tq, hs[b][:, q * 2048:(q + 1) * 2048], transpose=True)
            ht_q[(b, q)] = tq
            dft_pending.append((lp, b, q, gw[0]))

        def dft_q(lp, b, q):
            xfv = of_tile(lp)[:, 256:320]
            tt = ht_q.pop((b, q))
            for t in range(16):
                tg = q * 16 + t
                nc.tensor.matmul(xfv, tt[:, t, :], Fb[:, tg, :],
                                 start=(tg == 0), stop=(tg == 63))
            if q == 3:
                stage_ab(b, xfv)
                mix_sample(lp, b)

        def pump_dft():
            if dft_pending and gw[0] - dft_pending[0][3] >= 4:
                lp, bb, qq, _ = dft_pending.pop(0)
                dft_q(lp, bb, qq)

        gs.dma_start(Ci8, d_Ci8[:, :, :])
        for _lp in range(4):
            prefetch_sw(_lp)
        # warm the Gelu table while constants stream in
        warm = stg.tile([128, 1], f32)
        nc.scalar.activation(warm, lb[:, 0:1], GELU)

        # ---- layer-0 phase 1: xf0 = fc0_w.T @ (Xg @ F) ----
        # batched per engine stage so cross-engine latencies amortize
        xgFp = psm.tile([2 * BPC, 64], f32, tag="sm")
        for t in range(NCH):
            nc.tensor.matmul(xgFp, xgT[:, t, :], Fb[:, t, :],
                             start=(t == 0), stop=(t == NCH - 1))
        xgF = stg.tile([2 * BPC, 64], bf16)
        nc.vector.tensor_copy(xgF, xgFp)
        ot0 = of_tile(0)
        sw0 = sw_tiles[0]
        for b in range(BPC):
            xfv = ot0[:, 256 + 64 * b:320 + 64 * b]
            nc.tensor.matmul(xfv, fc0w2[:, b, :], xgF[:, :],
                             start=True, stop=(not has_fc0b))
            if has_fc0b:
                nc.tensor.matmul(xfv, r1[:, 0:128], r1[:, 128:192],
                                 start=False, stop=True)
        for b in range(BPC):
            stage_ab(b, ot0[:, 256 + 64 * b:320 + 64 * b])
        for b in range(BPC):
            for k in range(MODES):
                c = 8 * k + 2 * b
                nc.tensor.matmul(ot0[:, c:c + 2], sw0[:, k, 0, :],
                                 A[:, c:c + 2], start=True, stop=False)
                nc.tensor.matmul(ot0[:, c:c + 2], sw0[:, k, 1, :],
                                 Bs[:, c:c + 2], start=False, stop=True)
        of30 = ot0[:, 0:256].rearrange("p (k g) -> p k g", g=8)
        for b in range(BPC):
            nc.vector.tensor_copy(ofn[:, 64 * b:64 * (b + 1)],
                                  of30[:, :, 2 * b:2 * b + 2])
        otp4 = psm.tile([64, BPC, 128], bf16, tag="sm")
        for b in range(BPC):
            nc.tensor.transpose(otp4[:, b, :], ofn[:, 64 * b:64 * (b + 1)],
                                ident)
        for b in range(BPC):
            sc0 = float(2.0 ** (OF_EXP[0] - SW_EXP))
            nc.vector.tensor_scalar_mul(ofT8[:, b, 0, :], otp4[:, b, :], sc0)
            nc.vector.tensor_scalar_mul(ofT8[:, b, 1, :], otp4[:, b, :], sc0)

        # ---- layers ----
        for l in range(4):
            # phase 3 (+ pipelined transposes, next layer's DFT + mixing)
            def z_window(b, w):
                zt = pz.tile([128, 1024], f32, tag="z", name="zt")
                for q in range(2):
                    sl = slice(w * 1024 + q * 512, w * 1024 + (q + 1) * 512)
                    if l == 0:
                        nc.tensor.matmul(zt[:, q * 512:(q + 1) * 512],
                                         M0T[:, b, :], xg[:, sl],
                                         start=True, stop=False)
                    else:
                        nc.tensor.matmul(zt[:, q * 512:(q + 1) * 512],
                                         WT[:, l, :], hs[b][:, sl],
                                         start=True, stop=False)
                    nc.tensor.matmul(zt[:, q * 512:(q + 1) * 512],
                                     ofT8[:, b, :, :], Ci8[:, :, sl],
                                     start=False, stop=True, perf_mode=DR)
                return zt

            if l < 3:
                for b in range(BPC):
                    for w in range(8):
                        zt = z_window(b, w)
                        nc.scalar.activation(hs[b][:, w * 1024:(w + 1) * 1024],
                                             zt, GELU, bias=lb[:, l:l + 1],
                                             scale=float(2.0 ** -E_L[l]))
                        if w % 2 == 1:
                            issue_t(l + 1, b, w // 2)
                        gw[0] += 1
                        pump_dft()
            else:
                # two-stream software-pipelined final block:
                # z(b,w)+z(b',w) | fc1(.,w-1) halves | fc2(.,w-2)
                ot3 = of_tile(3)
                f2sm = psm.tile([128, 64], f32, tag="sm", name="f2sm")
                f2p = {0: ot3[:, 320:384], 1: ot3[:, 384:448],
                       2: ot3[:, 448:512], 3: f2sm}
                for pb in (0, 2):
                    pair = (pb, pb + 1)
                    ocs, g1h = {}, {}
                    for step in range(10):
                        gw[0] += 1
                        pump_dft()
                        if step < 8:
                            for bb in pair:
                                zt = z_window(bb, step)
                                oc = outp.tile([128, 1024], bf16, tag="oc")
                                nc.vector.tensor_scalar_mul(
                                    oc, zt, float(2.0 ** -E_L[3]))
                                ocs[(bb, step)] = oc
                        if 1 <= step <= 8:
                            w = step - 1
                            for bb in pair:
                                ocp = ocs.pop((bb, w))
                                for h in range(2):
                                    fph = pf1.tile([128, 512], f32, tag="f1")
                                    nc.tensor.matmul(
                                        fph, fc1w,
                                        ocp[:, h * 512:(h + 1) * 512],
                                        start=True, stop=True)
                                    g1t = outp.tile([128, 512], bf16, tag="g1")
                                    nc.scalar.activation(g1t, fph, GELU,
                                                         bias=fc1b[:, 0:1])
                                    g1h[(bb, w, h)] = g1t
                        if 2 <= step <= 9:
                            w = step - 2
                            for bb in pair:
                                for h in range(2):
                                    g1t = g1h.pop((bb, w, h))
                                    for q in range(4):
                                        tg = w * 8 + h * 4 + q
                                        nc.tensor.matmul(
                                            f2p[bb][:, tg:tg + 1],
                                            g1t[:, q * 128:(q + 1) * 128],
                                            fc2w, start=True, stop=True)
                    for bb in pair:
                        f2sb = outp.tile([128, 64], f32, tag="f2sb")
                        nc.vector.tensor_copy(f2sb, f2p[bb])
                        sy.dma_start(d_out[bb, :, :], f2sb)

    nc.compile()
    return nc


def _prep_v2(inputs, has_fc0b):
    inp = {k: np.asarray(v) for k, v in inputs.items()}
    F, Ci = _fourier_bases()
    F_sb = F.reshape(NCH, 128, 64).transpose(1, 0, 2).astype(BF)
    Ci8_1 = np.clip(Ci * (2.0 ** CI_EXP), -240, 240).astype(F8)  # [64, S]
    Ci8 = np.ascontiguousarray(np.repeat(Ci8_1[:, None, :], 2, axis=1))

    F_sb = np.clip(F_sb.astype(np.float64), -240, 240).astype(F8)
    fc0_w = inp["fc0_w"].astype(np.float64)     # [2, 128]
    fc0_b = inp["fc0_b"].astype(np.float64)
    Ws = [inp[f"w{i}_w"].astype(np.float64) for i in range(4)]
    WT = np.stack([Ws[i].T * (2.0 ** E_L[i]) for i in range(4)], 1)  # [128,4,128]
    M0 = (Ws[0] @ fc0_w.T).T * (2.0 ** E_L[0])   # [2, 128]
    M0T = np.zeros((8, 4, 128), np.float64)
    fc0blk = np.zeros((8, 4, 128), np.float64)
    for b in range(BPC):
        M0T[2 * b, b, :] = M0[0]
        M0T[2 * b + 1, b, :] = M0[1]
        fc0blk[2 * b, b, :] = fc0_w[0]
        fc0blk[2 * b + 1, b, :] = fc0_w[1]
    SW = np.empty((4, 128, MODES, 2, 128), np.float64)
    for i in range(4):
        sw = np.asarray(inp[f"sw{i}"])
        SW[i, :, :, 0, :] = np.ascontiguousarray(sw.real).transpose(0, 2, 1)
        SW[i, :, :, 1, :] = np.ascontiguousarray(sw.imag).transpose(0, 2, 1)
    SW = np.clip(SW * (2.0 ** SW_EXP), -240, 240)
    lb = np.stack([inp[f"w{i}_b"].astype(np.float64) for i in range(3)], 1)
    lb[:, 0] += Ws[0] @ fc0_b
    fc1_w = inp["fc1_w"].astype(np.float64)
    fc1b = (inp["fc1_b"].astype(np.float64)
            + fc1_w.T @ (inp["w3_b"].astype(np.float64)
                         + inp["cg2_b"].astype(np.float64)))
    grid = np.linspace(0.0, 1.0, S, dtype=np.float64)
    common = {
        "Fb": F_sb, "Ci8": Ci8,
        "WT": WT.astype(BF), "M0T": M0T.astype(BF),
        "fc0w2": fc0blk.astype(BF),
        "SW": SW.astype(F8),
        "fc1w": fc1_w.astype(BF),
        "fc2w": inp["fc2_w"].astype(np.float32).astype(BF),
        "lb": lb.astype(np.float32),
        "fc1b": fc1b.reshape(128, 1).astype(np.float32),
    }
    if has_fc0b:
        r1 = np.zeros((1, 192), np.float64)
        r1[0, 0:128] = fc0_b
        r1[0, 128:192] = F.sum(axis=0)
        common["r1"] = r1.astype(BF)
    x = inp["x"].astype(np.float64)  # [32, 8192, 1]
    per_core = []
    for c in range(NCORES):
        xg = np.empty((2 * BPC, S), np.float64)
        for b in range(BPC):
            xg[2 * b] = x[c * BPC + b, :, 0]
            xg[2 * b + 1] = grid
        m = dict(common)
        m["xg"] = xg.astype(BF)
        m["xgT"] = np.ascontiguousarray(
            xg.T.reshape(NCH, 128, 2 * BPC).transpose(1, 0, 2)).astype(BF)
        per_core.append(m)
    fc2b = float(inp["fc2_b"].astype(np.float64).reshape(-1)[0])
    return per_core, fc2b


# ---------------------------------------------------------------------------
# general fallback (original kernel; handles nonzero cg2_w)
# ---------------------------------------------------------------------------

def _host_consts_base():
    F, Ci = _fourier_bases()
    s = np.arange(S, dtype=np.float64)
    T = _cheb_basis(S, M_CHEB).astype(np.float64)
    kk = np.arange(-CFT_MODES, CFT_MODES + 1, dtype=np.float64)
    ph = np.pi * np.outer(s, kk) / S
    CH = np.empty((S, M_CHEB, 2 * CFT_MODES + 1, 2), np.float64)
    CH[..., 0] = T.T[:, :, None] * np.cos(ph)[:, None, :]
    CH[..., 1] = T.T[:, :, None] * (-np.sin(ph))[:, None, :]
    CH = (CH / S).reshape(S, 72)
    F_sb = F.reshape(NCH, 128, 64).transpose(1, 0, 2).astype(BF)
    CH_sb = CH.reshape(NCH, 128, 72).transpose(1, 0, 2).astype(BF)
    grid = np.linspace(0.0, 1.0, S, dtype=np.float32)
    return F_sb, CH_sb, Ci.astype(BF), grid


def _build_base():
    import concourse.bacc as bacc
    import concourse.tile as tile
    import concourse.mybir as mybir
    from concourse.masks import make_identity

    f32 = mybir.dt.float32
    bf16 = mybir.dt.bfloat16
    GELU = mybir.ActivationFunctionType.Gelu
    IDENT = mybir.ActivationFunctionType.Identity

    nc = bacc.Bacc("TRN2", target_bir_lowering=False)

    d_xg = nc.dram_tensor("xg", [2 * BPC, S], bf16, kind="ExternalInput")
    d_fc0w = nc.dram_tensor("fc0w", [8, 4, 128], bf16, kind="ExternalInput")
    d_F = nc.dram_tensor("Fb", [128, NCH, 64], bf16, kind="ExternalInput")
    d_CH = nc.dram_tensor("CHb", [128, NCH, 72], bf16, kind="ExternalInput")
    d_Ci = nc.dram_tensor("Cinv", [64, S], bf16, kind="ExternalInput")
    d_WT = nc.dram_tensor("WT", [128, 4, 128], bf16, kind="ExternalInput")
    d_SW = nc.dram_tensor("SW", [4, 128, MODES, 2, 128], bf16, kind="ExternalInput")
    d_G = nc.dram_tensor("G2", [128, 72, 256], bf16, kind="ExternalInput")
    d_fc1w = nc.dram_tensor("fc1w", [128, 128], bf16, kind="ExternalInput")
    d_fc2w = nc.dram_tensor("fc2w", [128, 1], bf16, kind="ExternalInput")
    d_cg2h = nc.dram_tensor("cg2h", [128, 2, 128], bf16, kind="ExternalInput")
    d_fc0b = nc.dram_tensor("fc0b", [128, 1], f32, kind="ExternalInput")
    d_lb = nc.dram_tensor("lb", [128, 3], f32, kind="ExternalInput")
    d_w3b = nc.dram_tensor("w3b", [128, 1], f32, kind="ExternalInput")
    d_fc1b = nc.dram_tensor("fc1b", [128, 1], f32, kind="ExternalInput")
    d_cg1b = nc.dram_tensor("cg1b", [4, 256], f32, kind="ExternalInput")
    d_out = nc.dram_tensor("out", [BPC, S], f32, kind="ExternalOutput")

    with ExitStack() as ctx:
        tc = ctx.enter_context(tile.TileContext(nc))
        consts = ctx.enter_context(tc.tile_pool(name="consts", bufs=1))
        hpool = ctx.enter_context(tc.tile_pool(name="h", bufs=1))
        htp = ctx.enter_context(tc.tile_pool(name="ht", bufs=3))
        swp = ctx.enter_context(tc.tile_pool(name="sw", bufs=2))
        gp = ctx.enter_context(tc.tile_pool(name="g", bufs=2))
        outp = ctx.enter_context(tc.tile_pool(name="outc", bufs=3))
        stg = ctx.enter_context(tc.tile_pool(name="stg", bufs=1))
        pz = ctx.enter_context(tc.tile_pool(name="pz", bufs=2, space="PSUM"))
        pxf = ctx.enter_context(tc.tile_pool(name="pxf", bufs=2, space="PSUM"))
        pof = ctx.enter_context(tc.tile_pool(name="pof", bufs=1, space="PSUM"))
        psm = ctx.enter_context(tc.tile_pool(name="psm", bufs=1, space="PSUM"))

        sy, gs = nc.sync, nc.gpsimd

        xg = consts.tile([2 * BPC, S], bf16); sy.dma_start(xg, d_xg[:, :])
        fc0w = consts.tile([8, 4, 128], bf16); sy.dma_start(fc0w, d_fc0w[:, :, :])
        Fb = consts.tile([128, NCH, 64], bf16); sy.dma_start(Fb, d_F[:, :, :])
        CHb = consts.tile([128, NCH, 72], bf16); sy.dma_start(CHb, d_CH[:, :, :])
        Ci = consts.tile([64, S], bf16); sy.dma_start(Ci, d_Ci[:, :])
        WT = consts.tile([128, 4, 128], bf16); sy.dma_start(WT, d_WT[:, :, :])
        fc1w = consts.tile([128, 128], bf16); sy.dma_start(fc1w, d_fc1w[:, :])
        fc2w = consts.tile([128, 1], bf16); sy.dma_start(fc2w, d_fc2w[:, :])
        cg2h = consts.tile([128, 2, 128], bf16); sy.dma_start(cg2h, d_cg2h[:, :, :])
        fc0b = consts.tile([128, 1], f32); sy.dma_start(fc0b, d_fc0b[:, :])
        lb = consts.tile([128, 3], f32); sy.dma_start(lb, d_lb[:, :])
        w3b = consts.tile([128, 1], f32); sy.dma_start(w3b, d_w3b[:, :])
        fc1b = consts.tile([128, 1], f32); sy.dma_start(fc1b, d_fc1b[:, :])
        cg1b = consts.tile([4, 256], f32); sy.dma_start(cg1b, d_cg1b[:, :])
        ident = consts.tile([128, 128], bf16); make_identity(nc, ident)

        hs = [hpool.tile([128, S], bf16, tag=f"h{b}", name=f"h{b}")
              for b in range(BPC)]
        A = consts.tile([128, 256], bf16)
        Bs = consts.tile([128, 256], bf16)
        feats = consts.tile([128, 288], bf16)
        ofn = consts.tile([128, 256], bf16)
        ofTs = [consts.tile([64, 128], bf16, tag=f"ofT{b}", name=f"ofT{b}")
                for b in range(BPC)]
        latb = consts.tile([128, BPC], f32)

        for b in range(BPC):
            for w in range(8):
                zt = pz.tile([128, 1024], f32, tag="z")
                for q in range(2):
                    nc.tensor.matmul(
                        zt[:, q * 512:(q + 1) * 512], fc0w[:, b, :],
                        xg[:, w * 1024 + q * 512:w * 1024 + (q + 1) * 512],
                        start=True, stop=True)
                if w % 2 == 0:
                    nc.scalar.activation(hs[b][:, w * 1024:(w + 1) * 1024], zt,
                                         IDENT, bias=fc0b[:, 0:1])
                else:
                    nc.vector.tensor_scalar_add(
                        hs[b][:, w * 1024:(w + 1) * 1024], zt, fc0b[:, 0:1])

        for l in range(4):
            sw = swp.tile([128, MODES, 2, 128], bf16, tag="sw")
            gs.dma_start(sw, d_SW[l, :, :, :, :])
            for b in range(BPC):
                xfp = pxf.tile([128, 136], f32, tag="xf")
                if l == 3:
                    cftp = psm.tile([128, 72], f32, tag="sm")
                for hh in range(2):
                    ht = htp.tile([128, 32, 128], bf16, tag="ht")
                    teng = sy if hh == 0 else nc.scalar
                    teng.dma_start(ht, hs[b][:, hh * 4096:(hh + 1) * 4096],
                                   transpose=True)
                    for t in range(32):
                        tg = hh * 32 + t
                        nc.tensor.matmul(xfp[:, 0:64], ht[:, t, :], Fb[:, tg, :],
                                         start=(tg == 0), stop=(tg == 63))
                        if l == 3:
                            nc.tensor.matmul(cftp, ht[:, t, :],
                                             CHb[:, tg, :],
                                             start=(tg == 0), stop=(tg == 63))
                nc.vector.tensor_copy(A[:, 2 * b:256:8], xfp[:, 0:32])
                nc.vector.tensor_copy(A[:, 2 * b + 1:256:8], xfp[:, 32:64])
                nc.vector.tensor_copy(Bs[:, 2 * b + 1:256:8], xfp[:, 0:32])
                nc.vector.tensor_scalar_mul(Bs[:, 2 * b:256:8], xfp[:, 32:64], -1.0)
                if l == 3:
                    nc.vector.tensor_copy(feats[:, b:288:4], cftp)

            ofp = pof.tile([128, 256], f32, tag="of")
            for k in range(MODES):
                nc.tensor.matmul(ofp[:, 8 * k:8 * k + 8], sw[:, k, 0, :],
                                 A[:, 8 * k:8 * k + 8], start=True, stop=False)
                nc.tensor.matmul(ofp[:, 8 * k:8 * k + 8], sw[:, k, 1, :],
                                 Bs[:, 8 * k:8 * k + 8], start=False, stop=True)
            ofp3 = ofp.rearrange("p (k g) -> p k g", g=8)
            for b in range(BPC):
                nc.vector.tensor_copy(ofn[:, 64 * b:64 * (b + 1)],
                                      ofp3[:, :, 2 * b:2 * b + 2])
                otp = psm.tile([64, 128], bf16, tag="sm")
                nc.tensor.transpose(otp, ofn[:, 64 * b:64 * (b + 1)], ident)
                nc.vector.tensor_copy(ofTs[b], otp)

            if l == 3:
                tps = pxf.tile([4, 256], f32, tag="xf")
                for qc in range(9):
                    gt = gp.tile([128, 8, 256], bf16, tag="G")
                    gs.dma_start(gt, d_G[:, qc * 8:(qc + 1) * 8, :])
                    for qq in range(8):
                        q = qc * 8 + qq
                        nc.tensor.matmul(tps, feats[:, 4 * q:4 * q + 4],
                                         gt[:, qq, :],
                                         start=(q == 0), stop=(q == 71))
                tsb = stg.tile([4, 256], f32)
                nc.vector.tensor_add(tsb, tps, cg1b)
                tgb = stg.tile([4, 256], bf16)
                nc.scalar.activation(tgb, tsb, GELU)
                lps = pof.tile([128, BPC], f32, tag="of")
                for hh in range(2):
                    ttp = psm.tile([128, 4], bf16, tag="sm")
                    nc.tensor.transpose(ttp, tgb[:, hh * 128:(hh + 1) * 128],
                                        ident[0:4, 0:4])
                    tgT = stg.tile([128, 4], bf16, tag=f"tgT{hh}")
                    nc.vector.tensor_copy(tgT, ttp)
                    nc.tensor.matmul(lps, cg2h[:, hh, :], tgT,
                                     start=(hh == 0), stop=(hh == 1))
                nc.vector.tensor_scalar_add(latb, lps, w3b[:, 0:1])

            for b in range(BPC):
                if l == 3:
                    f2ps = psm.tile([128, 64], f32, tag="sm")
                for w in range(8):
                    zt = pz.tile([128, 1024], f32, tag="z")
                    for q in range(2):
                        sl = slice(w * 1024 + q * 512, w * 1024 + (q + 1) * 512)
                        nc.tensor.matmul(zt[:, q * 512:(q + 1) * 512],
                                         ofTs[b], Ci[:, sl], start=True, stop=False)
                        nc.tensor.matmul(zt[:, q * 512:(q + 1) * 512],
                                         WT[:, l, :], hs[b][:, sl],
                                         start=False, stop=True)
                    if l < 3:
                        nc.scalar.activation(hs[b][:, w * 1024:(w + 1) * 1024], zt,
                                             GELU, bias=lb[:, l:l + 1])
                    else:
                        oc = outp.tile([128, 1024], bf16, tag="oc")
                        nc.vector.tensor_scalar_add(oc, zt, latb[:, b:b + 1])
                        fps = pz.tile([128, 1024], f32, tag="z")
                        for q in range(2):
                            nc.tensor.matmul(fps[:, q * 512:(q + 1) * 512], fc1w,
                                             oc[:, q * 512:(q + 1) * 512],
                                             start=True, stop=True)
                        g1 = outp.tile([128, 1024], bf16, tag="g1")
                        nc.scalar.activation(g1, fps, GELU, bias=fc1b[:, 0:1])
                        for q in range(8):
                            tg = w * 8 + q
                            nc.tensor.matmul(f2ps[:, tg:tg + 1],
                                             g1[:, q * 128:(q + 1) * 128], fc2w,
                                             start=True, stop=True)
                if l == 3:
                    f2sb = outp.tile([128, 64], f32, tag="f2sb")
                    nc.vector.tensor_copy(f2sb, f2ps)
                    sy.dma_start(d_out[b, :].rearrange("(t p) -> p t", p=128), f2sb)

    nc.compile()
    return nc


def _fc0_blk(fc0_w):
    blk = np.zeros((8, 4, 128), np.float32)
    for b in range(BPC):
        blk[2 * b, b, :] = fc0_w[0]
        blk[2 * b + 1, b, :] = fc0_w[1]
    return blk.astype(BF)


def _prep_base(inputs):
    inp = {k: np.asarray(v) for k, v in inputs.items()}
    F_sb, CH_sb, Ci, grid = _host_consts_base()
    x = inp["x"].astype(np.float32)
    fc0_w = inp["fc0_w"].astype(np.float32)
    WT = np.stack([inp[f"w{i}_w"].astype(np.float32).T for i in range(4)], 1)
    SW = np.empty((4, 128, MODES, 2, 128), np.float32)
    for i in range(4):
        sw = np.asarray(inp[f"sw{i}"])
        SW[i, :, :, 0, :] = np.ascontiguousarray(sw.real).transpose(0, 2, 1)
        SW[i, :, :, 1, :] = np.ascontiguousarray(sw.imag).transpose(0, 2, 1)
    cg1 = inp["cg1_w"].astype(np.float32).reshape(WIDTH, M_CHEB, L_SEG, 9, 2, 256)
    G2 = cg1.sum(axis=2).reshape(WIDTH, 72, 256)
    lb = np.stack([inp[f"w{i}_b"].astype(np.float32) for i in range(3)], 1)
    common = {
        "fc0w": _fc0_blk(fc0_w),
        "Fb": F_sb, "CHb": CH_sb, "Cinv": Ci,
        "WT": WT.astype(BF),
        "SW": SW.astype(BF),
        "G2": G2.astype(BF),
        "fc1w": inp["fc1_w"].astype(np.float32).astype(BF),
        "fc2w": inp["fc2_w"].astype(np.float32).astype(BF),
        "cg2h": inp["cg2_w"].astype(np.float32).reshape(2, 128, 128)
                .transpose(1, 0, 2).copy().astype(BF),
        "fc0b": inp["fc0_b"].astype(np.float32).reshape(128, 1),
        "lb": lb,
        "w3b": (inp["w3_b"].astype(np.float32)
                + inp["cg2_b"].astype(np.float32)).reshape(128, 1),
        "fc1b": inp["fc1_b"].astype(np.float32).reshape(128, 1),
        "cg1b": np.broadcast_to(inp["cg1_b"].astype(np.float32), (4, 256)).copy(),
    }
    per_core = []
    for c in range(NCORES):
        xg = np.empty((2 * BPC, S), np.float32)
        for b in range(BPC):
            xg[2 * b] = x[c * BPC + b, :, 0]
            xg[2 * b + 1] = grid
        m = dict(common)
        m["xg"] = xg.astype(BF)
        per_core.append(m)
    fc2b = float(inp["fc2_b"].astype(np.float32).reshape(-1)[0])
    return per_core, fc2b


# ---------------------------------------------------------------------------

def kernel(**inputs) -> np.ndarray:
    from concourse import bass_utils
    fast = (np.count_nonzero(np.asarray(inputs["cg2_w"])) == 0)
    if fast:
        has_fc0b = bool(np.count_nonzero(np.asarray(inputs["fc0_b"])))
        key = ("v2", has_fc0b)
        per_core, fc2b = _prep_v2(inputs, has_fc0b)
        if key not in _CACHE:
            _CACHE[key] = _build_v2(has_fc0b)
    else:
        key = ("base",)
        per_core, fc2b = _prep_base(inputs)
        if key not in _CACHE:
            _CACHE[key] = _build_base()
    nc = _CACHE[key]
    res = bass_utils.run_bass_kernel_spmd(nc, per_core, core_ids=list(range(NCORES)))
    out = np.empty((B, S, 1), np.float32)
    for c in range(NCORES):
        r = res.results[c]["out"]
        if fast:  # [BPC, 128, 64]: s = t*128 + p
            r = r.transpose(0, 2, 1).reshape(BPC, S)
        out[c * BPC:(c + 1) * BPC, :, 0] = r
    return out + fc2b


# revision 72
# speedup vs baseline: 2.9233x; 1.1260x over previous
"""Trainium2 Bass kernel for nn_FNO_RC_1D (1D FNO + Chebyshev-Fourier residual
correction). Data-parallel over batch: 32 samples -> 8 cores x 4 samples.

Fast path (used when cg2_w == 0, the problem's zero-init correction head, so
latent == cg2_b and the whole CFT path folds into the fc1 bias):
  - layer 0 is folded through fc0: xf0 = fc0_w.T @ (Xg @ F) via a host-side
    transposed copy of [x; grid], and the 1x1 conv term is M0 @ Xg with
    M0 = W0 @ fc0_w.T -- h0 is never materialized in either orientation.
  - spectral conv per layer: forward DFT of 32 modes as chunked matmuls
    against hT (DMA xbar transpose of h), per-mode complex mixing as small
    matmuls, and the inverse DFT *fused into the 1x1 conv PSUM pass* as a
    single fp8 DoubleRow matmul (of/Ci quantized to scaled e4m3; the W h
    term rides the same accumulation in bf16 with power-of-2-scaled weights,
    unscaled exactly by the activation's scale argument).
  - gelu drains on ScalarE write h in place; transposes for the next layer
    issue mid-drain so the xbar DMAs overlap the remaining windows; the
    next layer's forward DFT is software-pipelined one sample behind.
  - final block: z -> oc on VectorE, fc1+gelu, fc2 as per-chunk columns.
All big matmuls bf16 (fp8 only where quantization error is provably small),
accumulation fp32 in PSUM.

A general fallback (original baseline kernel) handles nonzero cg2_w.
"""

from contextlib import ExitStack

import numpy as np
import ml_dtypes

B, S, WIDTH, MODES = 32, 8192, 128, 32
CFT_MODES, L_SEG, M_CHEB = 4, 2, 4
CFT_DIM = (2 * CFT_MODES + 1) * L_SEG * M_CHEB * WIDTH * 2
NCORES = 8
BPC = B // NCORES  # samples per core
NCH = S // 128     # 64 chunks
BF = ml_dtypes.bfloat16
F8 = ml_dtypes.float8_e4m3   # TRN float8e4: max finite 240, 256 -> inf

OF_EXP = [7, 10, 14, 17]     # per-layer scale exponent for of -> fp8
CI_EXP = 18                  # scale exponent for Cinv -> fp8
SW_EXP = 21                  # scale exponent for spectral weights -> fp8
# +1: the DoubleRow irfft sums two identical slot products (2x the result)
E_L = [o + CI_EXP + 1 for o in OF_EXP]

_CACHE = {}


def _cheb_basis(n, m):
    t = np.linspace(-1.0, 1.0, n)
    Ts = [np.ones(n), t]
    for _ in range(2, m):
        Ts.append(2.0 * t * Ts[-1] - Ts[-2])
    return np.stack(Ts[:m], 0).astype(np.float32)


def _fourier_bases():
    s = np.arange(S, dtype=np.float64)
    k = np.arange(MODES, dtype=np.float64)
    ang = 2.0 * np.pi * np.outer(s, k) / S
    F = np.concatenate([np.cos(ang), -np.sin(ang)], axis=1)  # [S, 64]
    ck = np.full(MODES, 2.0 / S); ck[0] = 1.0 / S
    Ci = np.empty((2 * MODES, S), np.float64)                # interleaved re/im
    Ci[0::2] = ck[:, None] * np.cos(ang.T)
    Ci[1::2] = -ck[:, None] * np.sin(ang.T)
    return F, Ci


# ---------------------------------------------------------------------------
# fast path (cg2_w == 0)
# ---------------------------------------------------------------------------

def _build_v2(has_fc0b):
    import concourse.bacc as bacc
    import concourse.tile as tile
    import concourse.mybir as mybir
    from concourse.masks import make_identity

    f32 = mybir.dt.float32
    bf16 = mybir.dt.bfloat16
    fp8 = mybir.dt.float8e4
    GELU = mybir.ActivationFunctionType.Gelu
    DR = mybir.MatmulPerfMode.DoubleRow

    nc = bacc.Bacc("TRN2", target_bir_lowering=False)

    # ---- DRAM tensors ----
    d_xg = nc.dram_tensor("xg", [2 * BPC, S], bf16, kind="ExternalInput")
    d_xgT = nc.dram_tensor("xgT", [128, NCH, 2 * BPC], bf16, kind="ExternalInput")
    d_F = nc.dram_tensor("Fb", [128, NCH, 64], fp8, kind="ExternalInput")
    d_Ci8 = nc.dram_tensor("Ci8", [64, 2, S], fp8, kind="ExternalInput")
    d_WT = nc.dram_tensor("WT", [128, 4, 128], bf16, kind="ExternalInput")
    d_M0T = nc.dram_tensor("M0T", [8, 4, 128], bf16, kind="ExternalInput")
    d_fc0w2 = nc.dram_tensor("fc0w2", [8, 4, 128], bf16, kind="ExternalInput")
    d_SW = nc.dram_tensor("SW", [4, 128, MODES, 2, 128], fp8, kind="ExternalInput")
    d_fc1w = nc.dram_tensor("fc1w", [128, 128], bf16, kind="ExternalInput")
    d_fc2w = nc.dram_tensor("fc2w", [128, 1], bf16, kind="ExternalInput")
    d_lb = nc.dram_tensor("lb", [128, 3], f32, kind="ExternalInput")
    d_fc1b = nc.dram_tensor("fc1b", [128, 1], f32, kind="ExternalInput")
    if has_fc0b:
        d_r1 = nc.dram_tensor("r1", [1, 192], bf16, kind="ExternalInput")
    d_out = nc.dram_tensor("out", [BPC, 128, 64], f32, kind="ExternalOutput")

    with ExitStack() as ctx:
        tc = ctx.enter_context(tile.TileContext(nc))
        consts = ctx.enter_context(tc.tile_pool(name="consts", bufs=1))
        hpool = ctx.enter_context(tc.tile_pool(name="h", bufs=1))
        htp = ctx.enter_context(tc.tile_pool(name="ht", bufs=10))
        swp = ctx.enter_context(tc.tile_pool(name="sw", bufs=4))
        outp = ctx.enter_context(tc.tile_pool(name="outc", bufs=8))
        stg = ctx.enter_context(tc.tile_pool(name="stg", bufs=1))
        pz = ctx.enter_context(tc.tile_pool(name="pz", bufs=3, space="PSUM"))
        pf1 = ctx.enter_context(tc.tile_pool(name="pf1", bufs=1, space="PSUM"))
        pof = ctx.enter_context(tc.tile_pool(name="pof", bufs=1, space="PSUM"))

        sy, gs = nc.sync, nc.gpsimd

        # ---- constants into SBUF ----
        # order matters for the startup critical path: phase1-l0 needs
        # xgT+Fb, phase2-l0 needs SW (gs queue), phase3-l0 needs Ci8+xg.
        xgT = consts.tile([128, NCH, 2 * BPC], bf16); sy.dma_start(xgT, d_xgT[:, :, :])
        lb = consts.tile([128, 3], f32); sy.dma_start(lb, d_lb[:, :])
        fc1b = consts.tile([128, 1], f32); sy.dma_start(fc1b, d_fc1b[:, :])
        Fb = consts.tile([128, NCH, 64], fp8); sy.dma_start(Fb, d_F[:, :, :])
        Ci8 = consts.tile([64, 2, S], fp8)
        xg = consts.tile([2 * BPC, S], bf16); sy.dma_start(xg, d_xg[:, :])
        M0T = consts.tile([8, 4, 128], bf16); sy.dma_start(M0T, d_M0T[:, :, :])
        fc0w2 = consts.tile([8, 4, 128], bf16); sy.dma_start(fc0w2, d_fc0w2[:, :, :])
        WT = consts.tile([128, 4, 128], bf16); sy.dma_start(WT, d_WT[:, :, :])
        fc1w = consts.tile([128, 128], bf16); sy.dma_start(fc1w, d_fc1w[:, :])
        fc2w = consts.tile([128, 1], bf16); sy.dma_start(fc2w, d_fc2w[:, :])
        if has_fc0b:
            r1 = consts.tile([1, 192], bf16); sy.dma_start(r1, d_r1[:, :])
        ident = consts.tile([128, 128], bf16); make_identity(nc, ident)

        hs = [hpool.tile([128, S], bf16, tag=f"h{b}", name=f"h{b}")
              for b in range(BPC)]
        A = consts.tile([128, 256], bf16)      # staged (xr, xi) per (k, b)
        Bs = consts.tile([128, 256], bf16)     # staged (-xi, xr)
        ofn = consts.tile([128, 256], bf16)    # of natural staging per sample
        ofT8 = consts.tile([64, BPC, 2, 128], fp8)  # DoubleRow lhsT per sample

        sw_tiles = {}

        def prefetch_sw(l):
            t = swp.tile([128, MODES, 2, 128], fp8, tag="sw", name=f"sw{l}")
            gs.dma_start(t, d_SW[l, :, :, :, :])
            sw_tiles[l] = t

        def stage_ab(b, xfp):
            nc.vector.tensor_copy(A[:, 2 * b:256:8], xfp[:, 0:32])
            nc.vector.tensor_copy(A[:, 2 * b + 1:256:8], xfp[:, 32:64])
            nc.vector.tensor_copy(Bs[:, 2 * b + 1:256:8], xfp[:, 0:32])
            nc.vector.tensor_scalar_mul(Bs[:, 2 * b:256:8], xfp[:, 32:64], -1.0)

        # of_tile(lp): one PSUM bank per layer epoch -- cols 0:256 hold the
        # mixed spectral outputs (k-major, per-sample col pairs), cols
        # 256:320 are the forward-DFT accumulator (reused per sample).
        of_tiles = {}

        def of_tile(lp):
            if lp not in of_tiles:
                of_tiles[lp] = pof.tile([128, 512], f32, tag="of",
                                        name=f"of{lp}")
            return of_tiles[lp]

        def mix_sample(lp, b):
            # sample b's full phase-2 chain: mixing -> ofn -> transpose -> fp8
            sw = sw_tiles[lp]
            ot = of_tiles[lp]
            for k in range(MODES):
                c = 8 * k + 2 * b
                nc.tensor.matmul(ot[:, c:c + 2], sw[:, k, 0, :],
                                 A[:, c:c + 2], start=True, stop=False)
                nc.tensor.matmul(ot[:, c:c + 2], sw[:, k, 1, :],
                                 Bs[:, c:c + 2], start=False, stop=True)
            of3 = ot[:, 0:256].rearrange("p (k g) -> p k g", g=8)
            nc.vector.tensor_copy(ofn[:, 64 * b:64 * (b + 1)],
                                  of3[:, :, 2 * b:2 * b + 2])
            otp_t = pf1.tile([128, 512], f32, tag="f1", name="otp_t")
            otp = otp_t[0:64, 0:64].bitcast(bf16)
            nc.tensor.transpose(otp, ofn[:, 64 * b:64 * (b + 1)], ident)
            sc = float(2.0 ** (OF_EXP[lp] - SW_EXP))
            nc.vector.tensor_scalar_mul(ofT8[:, b, 0, :], otp, sc)
            nc.vector.tensor_scalar_mul(ofT8[:, b, 1, :], otp, sc)

        ht_q = {}
        dft_pending = []   # [lp, b, q, issue_window]; >= 4-window consume lag
        gw = [0]           # global window counter

        def issue_t(lp, b, q):
            tq = htp.tile([128, 16, 128], bf16, tag="ht")
            sy.dma_start(tq, hs[b][:, q * 2048:(q + 1) * 2048], transpose=True)
            ht_q[(b, q)] = tq
            dft_pending.append((lp, b, q, 0, gw[0]))
            dft_pending.append((lp, b, q, 1, gw[0]))

        def dft_q(lp, b, q, h):
            xfv = of_tile(lp)[:, 256:320]
            tt = ht_q[(b, q)]
            for t in range(8):
                tg = q * 16 + h * 8 + t
                nc.tensor.matmul(xfv, tt[:, h * 8 + t, :], Fb[:, tg, :],
                                 start=(tg == 0), stop=(tg == 63))
            if h == 1:
                del ht_q[(b, q)]
                if q == 3:
                    stage_ab(b, xfv)
                    mix_sample(lp, b)

        def pump_dft():
            if dft_pending and gw[0] - dft_pending[0][4] >= 4:
                lp, bb, qq, hh, _ = dft_pending.pop(0)
                dft_q(lp, bb, qq, hh)

        prefetch_sw(0)
        gs.dma_start(Ci8, d_Ci8[:, :, :])
        for _lp in range(1, 4):
            prefetch_sw(_lp)
        # warm the Gelu table while constants stream in
        warm = stg.tile([128, 1], f32)
        nc.scalar.activation(warm, lb[:, 0:1], GELU)
        # keep PE busy while DMAs land so the pstate ramp stays hot
        scratch = pf1.tile([128, 512], f32, tag="f1", name="scratch")
        for i in range(16):
            nc.tensor.matmul(scratch[0:2 * BPC, 256:512], xgT[:, 0, :],
                             xgT[:, 0:32, :], start=True, stop=True)

        # ---- layer-0 phase 1: xf0 = fc0_w.T @ (Xg @ F) ----
        # batched per engine stage so cross-engine latencies amortize
        ot0 = of_tile(0)
        xgFp = ot0[0:2 * BPC, 448:512]
        for t in range(NCH):
            nc.tensor.matmul(xgFp, xgT[:, t, :], Fb[:, t, :],
                             start=(t == 0), stop=(t == NCH - 1))
        xgF = stg.tile([2 * BPC, 64], bf16)
        nc.vector.tensor_copy(xgF, xgFp)
        sw0 = sw_tiles[0]
        for b in range(BPC):
            xfv = ot0[:, 256 + 64 * b:320 + 64 * b]
            nc.tensor.matmul(xfv, fc0w2[:, b, :], xgF[:, :],
                             start=True, stop=(not has_fc0b))
            if has_fc0b:
                nc.tensor.matmul(xfv, r1[:, 0:128], r1[:, 128:192],
                                 start=False, stop=True)
        # batched A/B staging: one strided copy covers all 4 samples
        xf4 = ot0[:, 256:512].rearrange("p (b k) -> p k b", b=BPC)
        A3 = A.rearrange("p (k b g) -> p k b g", k=MODES, b=BPC)
        B3 = Bs.rearrange("p (k b g) -> p k b g", k=MODES, b=BPC)
        nc.vector.tensor_copy(A3[:, :, :, 0], xf4[:, 0:32, :])
        nc.vector.tensor_copy(A3[:, :, :, 1], xf4[:, 32:64, :])
        nc.vector.tensor_copy(B3[:, :, :, 1], xf4[:, 0:32, :])
        nc.vector.tensor_scalar_mul(B3[:, :, :, 0], xf4[:, 32:64, :], -1.0)
        # sample 0's chain completes first so phase 3 can start immediately
        of30 = ot0[:, 0:256].rearrange("p (k g) -> p k g", g=8)
        otp4 = scratch[0:64, 0:256].bitcast(bf16).rearrange(
            "p (b c) -> p b c", b=BPC)
        sc0 = float(2.0 ** (OF_EXP[0] - SW_EXP))
        for bset in ((0,), (1, 2, 3)):
            for b in bset:
                for k in range(MODES):
                    c = 8 * k + 2 * b
                    nc.tensor.matmul(ot0[:, c:c + 2], sw0[:, k, 0, :],
                                     A[:, c:c + 2], start=True, stop=False)
                    nc.tensor.matmul(ot0[:, c:c + 2], sw0[:, k, 1, :],
                                     Bs[:, c:c + 2], start=False, stop=True)
            for b in bset:
                nc.vector.tensor_copy(ofn[:, 64 * b:64 * (b + 1)],
                                      of30[:, :, 2 * b:2 * b + 2])
            for b in bset:
                nc.tensor.transpose(otp4[:, b, :], ofn[:, 64 * b:64 * (b + 1)],
                                    ident)
            for b in bset:
                nc.vector.tensor_scalar_mul(ofT8[:, b, 0, :], otp4[:, b, :], sc0)
                nc.vector.tensor_scalar_mul(ofT8[:, b, 1, :], otp4[:, b, :], sc0)

        # ---- layers ----
        for l in range(4):
            # phase 3 (+ pipelined transposes, next layer's DFT + mixing)
            def z_window(b, w):
                zt = pz.tile([128, 1024], f32, tag="z", name="zt")
                for q in range(2):
                    sl = slice(w * 1024 + q * 512, w * 1024 + (q + 1) * 512)
                    if l == 0:
                        nc.tensor.matmul(zt[:, q * 512:(q + 1) * 512],
                                         M0T[:, b, :], xg[:, sl],
                                         start=True, stop=False)
                    else:
                        nc.tensor.matmul(zt[:, q * 512:(q + 1) * 512],
                                         WT[:, l, :], hs[b][:, sl],
                                         start=True, stop=False)
                    nc.tensor.matmul(zt[:, q * 512:(q + 1) * 512],
                                     ofT8[:, b, :, :], Ci8[:, :, sl],
                                     start=False, stop=True, perf_mode=DR)
                return zt

            if l < 3:
                for b in range(BPC):
                    for w in range(8):
                        zt = z_window(b, w)
                        nc.scalar.activation(hs[b][:, w * 1024:(w + 1) * 1024],
                                             zt, GELU, bias=lb[:, l:l + 1],
                                             scale=float(2.0 ** -E_L[l]))
                        if w % 2 == 1:
                            issue_t(l + 1, b, w // 2)
                        gw[0] += 1
                        pump_dft()
            else:
                # two-stream software-pipelined final block:
                # z(b,w)+z(b',w) | fc1(.,w-1) halves | fc2(.,w-2)
                ot3 = of_tile(3)
                f2p = {0: ot3[:, 320:384], 1: ot3[:, 384:448],
                       2: ot3[:, 448:512], 3: ot3[:, 256:320]}
                for pb in (0, 2):
                    pair = (pb, pb + 1)
                    ocs, g1h = {}, {}
                    for step in range(10):
                        gw[0] += 1
                        pump_dft()
                        if step < 8:
                            for bb in pair:
                                zt = z_window(bb, step)
                                oc = outp.tile([128, 1024], bf16, tag="oc", bufs=4)
                                nc.vector.tensor_scalar_mul(
                                    oc, zt, float(2.0 ** -E_L[3]))
                                ocs[(bb, step)] = oc
                        if 1 <= step <= 8:
                            w = step - 1
                            for bb in pair:
                                ocp = ocs.pop((bb, w))
                                fps = pz.tile([128, 1024], f32, tag="z",
                                              name="fps")
                                for q in range(2):
                                    nc.tensor.matmul(
                                        fps[:, q * 512:(q + 1) * 512], fc1w,
                                        ocp[:, q * 512:(q + 1) * 512],
                                        start=True, stop=True)
                                g1t = outp.tile([128, 1024], bf16, tag="g1", bufs=4)
                                nc.scalar.activation(g1t, fps, GELU,
                                                     bias=fc1b[:, 0:1])
                                g1h[(bb, w)] = g1t
                        if 2 <= step <= 9:
                            w = step - 2
                            for bb in pair:
                                g1t = g1h.pop((bb, w))
                                for q in range(8):
                                    tg = w * 8 + q
                                    nc.tensor.matmul(
                                        f2p[bb][:, tg:tg + 1],
                                        g1t[:, q * 128:(q + 1) * 128],
                                        fc2w, start=True, stop=True)
                                if step == 9:
                                    f2sb = outp.tile([128, 64], f32,
                                                     tag="f2sb", bufs=2)
                                    nc.vector.tensor_copy(f2sb, f2p[bb])
                                    sy.dma_start(d_out[bb, :, :], f2sb)

    nc.compile()
    return nc


def _prep_v2(inputs, has_fc0b):
    inp = {k: np.asarray(v) for k, v in inputs.items()}
    F, Ci = _fourier_bases()
    F_sb = F.reshape(NCH, 128, 64).transpose(1, 0, 2).astype(BF)
    Ci8_1 = np.clip(Ci * (2.0 ** CI_EXP), -240, 240).astype(F8)  # [64, S]
    Ci8 = np.ascontiguousarray(np.repeat(Ci8_1[:, None, :], 2, axis=1))

    F_sb = np.clip(F_sb.astype(np.float64), -240, 240).astype(F8)
    fc0_w = inp["fc0_w"].astype(np.float64)     # [2, 128]
    fc0_b = inp["fc0_b"].astype(np.float64)
    Ws = [inp[f"w{i}_w"].astype(np.float64) for i in range(4)]
    WT = np.stack([Ws[i].T * (2.0 ** E_L[i]) for i in range(4)], 1)  # [128,4,128]
    M0 = (Ws[0] @ fc0_w.T).T * (2.0 ** E_L[0])   # [2, 128]
    M0T = np.zeros((8, 4, 128), np.float64)
    fc0blk = np.zeros((8, 4, 128), np.float64)
    for b in range(BPC):
        M0T[2 * b, b, :] = M0[0]
        M0T[2 * b + 1, b, :] = M0[1]
        fc0blk[2 * b, b, :] = fc0_w[0]
        fc0blk[2 * b + 1, b, :] = fc0_w[1]
    SW = np.empty((4, 128, MODES, 2, 128), np.float64)
    for i in range(4):
        sw = np.asarray(inp[f"sw{i}"])
        SW[i, :, :, 0, :] = np.ascontiguousarray(sw.real).transpose(0, 2, 1)
        SW[i, :, :, 1, :] = np.ascontiguousarray(sw.imag).transpose(0, 2, 1)
    SW = np.clip(SW * (2.0 ** SW_EXP), -240, 240)
    lb = np.stack([inp[f"w{i}_b"].astype(np.float64) for i in range(3)], 1)
    lb[:, 0] += Ws[0] @ fc0_b
    fc1_w = inp["fc1_w"].astype(np.float64)
    fc1b = (inp["fc1_b"].astype(np.float64)
            + fc1_w.T @ (inp["w3_b"].astype(np.float64)
                         + inp["cg2_b"].astype(np.float64)))
    grid = np.linspace(0.0, 1.0, S, dtype=np.float64)
    common = {
        "Fb": F_sb, "Ci8": Ci8,
        "WT": WT.astype(BF), "M0T": M0T.astype(BF),
        "fc0w2": fc0blk.astype(BF),
        "SW": SW.astype(F8),
        "fc1w": fc1_w.astype(BF),
        "fc2w": inp["fc2_w"].astype(np.float32).astype(BF),
        "lb": lb.astype(np.float32),
        "fc1b": fc1b.reshape(128, 1).astype(np.float32),
    }
    if has_fc0b:
        r1 = np.zeros((1, 192), np.float64)
        r1[0, 0:128] = fc0_b
        r1[0, 128:192] = F.sum(axis=0)
        common["r1"] = r1.astype(BF)
    x = inp["x"].astype(np.float64)  # [32, 8192, 1]
    per_core = []
    for c in range(NCORES):
        xg = np.empty((2 * BPC, S), np.float64)
        for b in range(BPC):
            xg[2 * b] = x[c * BPC + b, :, 0]
            xg[2 * b + 1] = grid
        m = dict(common)
        m["xg"] = xg.astype(BF)
        m["xgT"] = np.ascontiguousarray(
            xg.T.reshape(NCH, 128, 2 * BPC).transpose(1, 0, 2)).astype(BF)
        per_core.append(m)
    fc2b = float(inp["fc2_b"].astype(np.float64).reshape(-1)[0])
    return per_core, fc2b


# ---------------------------------------------------------------------------
# general fallback (original kernel; handles nonzero cg2_w)
# ---------------------------------------------------------------------------

def _host_consts_base():
    F, Ci = _fourier_bases()
    s = np.arange(S, dtype=np.float64)
    T = _cheb_basis(S, M_CHEB).astype(np.float64)
    kk = np.arange(-CFT_MODES, CFT_MODES + 1, dtype=np.float64)
    ph = np.pi * np.outer(s, kk) / S
    CH = np.empty((S, M_CHEB, 2 * CFT_MODES + 1, 2), np.float64)
    CH[..., 0] = T.T[:, :, None] * np.cos(ph)[:, None, :]
    CH[..., 1] = T.T[:, :, None] * (-np.sin(ph))[:, None, :]
    CH = (CH / S).reshape(S, 72)
    F_sb = F.reshape(NCH, 128, 64).transpose(1, 0, 2).astype(BF)
    CH_sb = CH.reshape(NCH, 128, 72).transpose(1, 0, 2).astype(BF)
    grid = np.linspace(0.0, 1.0, S, dtype=np.float32)
    return F_sb, CH_sb, Ci.astype(BF), grid


def _build_base():
    import concourse.bacc as bacc
    import concourse.tile as tile
    import concourse.mybir as mybir
    from concourse.masks import make_identity

    f32 = mybir.dt.float32
    bf16 = mybir.dt.bfloat16
    GELU = mybir.ActivationFunctionType.Gelu
    IDENT = mybir.ActivationFunctionType.Identity

    nc = bacc.Bacc("TRN2", target_bir_lowering=False)

    d_xg = nc.dram_tensor("xg", [2 * BPC, S], bf16, kind="ExternalInput")
    d_fc0w = nc.dram_tensor("fc0w", [8, 4, 128], bf16, kind="ExternalInput")
    d_F = nc.dram_tensor("Fb", [128, NCH, 64], bf16, kind="ExternalInput")
    d_CH = nc.dram_tensor("CHb", [128, NCH, 72], bf16, kind="ExternalInput")
    d_Ci = nc.dram_tensor("Cinv", [64, S], bf16, kind="ExternalInput")
    d_WT = nc.dram_tensor("WT", [128, 4, 128], bf16, kind="ExternalInput")
    d_SW = nc.dram_tensor("SW", [4, 128, MODES, 2, 128], bf16, kind="ExternalInput")
    d_G = nc.dram_tensor("G2", [128, 72, 256], bf16, kind="ExternalInput")
    d_fc1w = nc.dram_tensor("fc1w", [128, 128], bf16, kind="ExternalInput")
    d_fc2w = nc.dram_tensor("fc2w", [128, 1], bf16, kind="ExternalInput")
    d_cg2h = nc.dram_tensor("cg2h", [128, 2, 128], bf16, kind="ExternalInput")
    d_fc0b = nc.dram_tensor("fc0b", [128, 1], f32, kind="ExternalInput")
    d_lb = nc.dram_tensor("lb", [128, 3], f32, kind="ExternalInput")
    d_w3b = nc.dram_tensor("w3b", [128, 1], f32, kind="ExternalInput")
    d_fc1b = nc.dram_tensor("fc1b", [128, 1], f32, kind="ExternalInput")
    d_cg1b = nc.dram_tensor("cg1b", [4, 256], f32, kind="ExternalInput")
    d_out = nc.dram_tensor("out", [BPC, S], f32, kind="ExternalOutput")

    with ExitStack() as ctx:
        tc = ctx.enter_context(tile.TileContext(nc))
        consts = ctx.enter_context(tc.tile_pool(name="consts", bufs=1))
        hpool = ctx.enter_context(tc.tile_pool(name="h", bufs=1))
        htp = ctx.enter_context(tc.tile_pool(name="ht", bufs=3))
        swp = ctx.enter_context(tc.tile_pool(name="sw", bufs=2))
        gp = ctx.enter_context(tc.tile_pool(name="g", bufs=2))
        outp = ctx.enter_context(tc.tile_pool(name="outc", bufs=3))
        stg = ctx.enter_context(tc.tile_pool(name="stg", bufs=1))
        pz = ctx.enter_context(tc.tile_pool(name="pz", bufs=2, space="PSUM"))
        pxf = ctx.enter_context(tc.tile_pool(name="pxf", bufs=2, space="PSUM"))
        pof = ctx.enter_context(tc.tile_pool(name="pof", bufs=1, space="PSUM"))
        psm = ctx.enter_context(tc.tile_pool(name="psm", bufs=1, space="PSUM"))

        sy, gs = nc.sync, nc.gpsimd

        xg = consts.tile([2 * BPC, S], bf16); sy.dma_start(xg, d_xg[:, :])
        fc0w = consts.tile([8, 4, 128], bf16); sy.dma_start(fc0w, d_fc0w[:, :, :])
        Fb = consts.tile([128, NCH, 64], bf16); sy.dma_start(Fb, d_F[:, :, :])
        CHb = consts.tile([128, NCH, 72], bf16); sy.dma_start(CHb, d_CH[:, :, :])
        Ci = consts.tile([64, S], bf16); sy.dma_start(Ci, d_Ci[:, :])
        WT = consts.tile([128, 4, 128], bf16); sy.dma_start(WT, d_WT[:, :, :])
        fc1w = consts.tile([128, 128], bf16); sy.dma_start(fc1w, d_fc1w[:, :])
        fc2w = consts.tile([128, 1], bf16); sy.dma_start(fc2w, d_fc2w[:, :])
        cg2h = consts.tile([128, 2, 128], bf16); sy.dma_start(cg2h, d_cg2h[:, :, :])
        fc0b = consts.tile([128, 1], f32); sy.dma_start(fc0b, d_fc0b[:, :])
        lb = consts.tile([128, 3], f32); sy.dma_start(lb, d_lb[:, :])
        w3b = consts.tile([128, 1], f32); sy.dma_start(w3b, d_w3b[:, :])
        fc1b = consts.tile([128, 1], f32); sy.dma_start(fc1b, d_fc1b[:, :])
        cg1b = consts.tile([4, 256], f32); sy.dma_start(cg1b, d_cg1b[:, :])
        ident = consts.tile([128, 128], bf16); make_identity(nc, ident)

        hs = [hpool.tile([128, S], bf16, tag=f"h{b}", name=f"h{b}")
              for b in range(BPC)]
        A = consts.tile([128, 256], bf16)
        Bs = consts.tile([128, 256], bf16)
        feats = consts.tile([128, 288], bf16)
        ofn = consts.tile([128, 256], bf16)
        ofTs = [consts.tile([64, 128], bf16, tag=f"ofT{b}", name=f"ofT{b}")
                for b in range(BPC)]
        latb = consts.tile([128, BPC], f32)

        for b in range(BPC):
            for w in range(8):
                zt = pz.tile([128, 1024], f32, tag="z")
                for q in range(2):
                    nc.tensor.matmul(
                        zt[:, q * 512:(q + 1) * 512], fc0w[:, b, :],
                        xg[:, w * 1024 + q * 512:w * 1024 + (q + 1) * 512],
                        start=True, stop=True)
                if w % 2 == 0:
                    nc.scalar.activation(hs[b][:, w * 1024:(w + 1) * 1024], zt,
                                         IDENT, bias=fc0b[:, 0:1])
                else:
                    nc.vector.tensor_scalar_add(
                        hs[b][:, w * 1024:(w + 1) * 1024], zt, fc0b[:, 0:1])

        for l in range(4):
            sw = swp.tile([128, MODES, 2, 128], bf16, tag="sw")
            gs.dma_start(sw, d_SW[l, :, :, :, :])
            for b in range(BPC):
                xfp = pxf.tile([128, 136], f32, tag="xf")
                if l == 3:
                    cftp = psm.tile([128, 72], f32, tag="sm")
                for hh in range(2):
                    ht = htp.tile([128, 32, 128], bf16, tag="ht")
                    teng = sy if hh == 0 else nc.scalar
                    teng.dma_start(ht, hs[b][:, hh * 4096:(hh + 1) * 4096],
                                   transpose=True)
                    for t in range(32):
                        tg = hh * 32 + t
                        nc.tensor.matmul(xfp[:, 0:64], ht[:, t, :], Fb[:, tg, :],
                                         start=(tg == 0), stop=(tg == 63))
                        if l == 3:
                            nc.tensor.matmul(cftp, ht[:, t, :],
                                             CHb[:, tg, :],
                                             start=(tg == 0), stop=(tg == 63))
                nc.vector.tensor_copy(A[:, 2 * b:256:8], xfp[:, 0:32])
                nc.vector.tensor_copy(A[:, 2 * b + 1:256:8], xfp[:, 32:64])
                nc.vector.tensor_copy(Bs[:, 2 * b + 1:256:8], xfp[:, 0:32])
                nc.vector.tensor_scalar_mul(Bs[:, 2 * b:256:8], xfp[:, 32:64], -1.0)
                if l == 3:
                    nc.vector.tensor_copy(feats[:, b:288:4], cftp)

            ofp = pof.tile([128, 256], f32, tag="of")
            for k in range(MODES):
                nc.tensor.matmul(ofp[:, 8 * k:8 * k + 8], sw[:, k, 0, :],
                                 A[:, 8 * k:8 * k + 8], start=True, stop=False)
                nc.tensor.matmul(ofp[:, 8 * k:8 * k + 8], sw[:, k, 1, :],
                                 Bs[:, 8 * k:8 * k + 8], start=False, stop=True)
            ofp3 = ofp.rearrange("p (k g) -> p k g", g=8)
            for b in range(BPC):
                nc.vector.tensor_copy(ofn[:, 64 * b:64 * (b + 1)],
                                      ofp3[:, :, 2 * b:2 * b + 2])
                otp = psm.tile([64, 128], bf16, tag="sm")
                nc.tensor.transpose(otp, ofn[:, 64 * b:64 * (b + 1)], ident)
                nc.vector.tensor_copy(ofTs[b], otp)

            if l == 3:
                tps = pxf.tile([4, 256], f32, tag="xf")
                for qc in range(9):
                    gt = gp.tile([128, 8, 256], bf16, tag="G")
                    gs.dma_start(gt, d_G[:, qc * 8:(qc + 1) * 8, :])
                    for qq in range(8):
                        q = qc * 8 + qq
                        nc.tensor.matmul(tps, feats[:, 4 * q:4 * q + 4],
                                         gt[:, qq, :],
                                         start=(q == 0), stop=(q == 71))
                tsb = stg.tile([4, 256], f32)
                nc.vector.tensor_add(tsb, tps, cg1b)
                tgb = stg.tile([4, 256], bf16)
                nc.scalar.activation(tgb, tsb, GELU)
                lps = pof.tile([128, BPC], f32, tag="of")
                for hh in range(2):
                    ttp = psm.tile([128, 4], bf16, tag="sm")
                    nc.tensor.transpose(ttp, tgb[:, hh * 128:(hh + 1) * 128],
                                        ident[0:4, 0:4])
                    tgT = stg.tile([128, 4], bf16, tag=f"tgT{hh}")
                    nc.vector.tensor_copy(tgT, ttp)
                    nc.tensor.matmul(lps, cg2h[:, hh, :], tgT,
                                     start=(hh == 0), stop=(hh == 1))
                nc.vector.tensor_scalar_add(latb, lps, w3b[:, 0:1])

            for b in range(BPC):
                if l == 3:
                    f2ps = psm.tile([128, 64], f32, tag="sm")
                for w in range(8):
                    zt = pz.tile([128, 1024], f32, tag="z")
                    for q in range(2):
                        sl = slice(w * 1024 + q * 512, w * 1024 + (q + 1) * 512)
                        nc.tensor.matmul(zt[:, q * 512:(q + 1) * 512],
                                         ofTs[b], Ci[:, sl], start=True, stop=False)
                        nc.tensor.matmul(zt[:, q * 512:(q + 1) * 512],
                                         WT[:, l, :], hs[b][:, sl],
                                         start=False, stop=True)
                    if l < 3:
                        nc.scalar.activation(hs[b][:, w * 1024:(w + 1) * 1024], zt,
                                             GELU, bias=lb[:, l:l + 1])
                    else:
                        oc = outp.tile([128, 1024], bf16, tag="oc", bufs=4)
                        nc.vector.tensor_scalar_add(oc, zt, latb[:, b:b + 1])
                        fps = pz.tile([128, 1024], f32, tag="z")
                        for q in range(2):
                            nc.tensor.matmul(fps[:, q * 512:(q + 1) * 512], fc1w,
                                             oc[:, q * 512:(q + 1) * 512],
                                             start=True, stop=True)
                        g1 = outp.tile([128, 1024], bf16, tag="g1")
                        nc.scalar.activation(g1, fps, GELU, bias=fc1b[:, 0:1])
                        for q in range(8):
                            tg = w * 8 + q
                            nc.tensor.matmul(f2ps[:, tg:tg + 1],
                                             g1[:, q * 128:(q + 1) * 128], fc2w,
                                             start=True, stop=True)
                if l == 3:
                    f2sb = outp.tile([128, 64], f32, tag="f2sb", bufs=2)
                    nc.vector.tensor_copy(f2sb, f2ps)
                    sy.dma_start(d_out[b, :].rearrange("(t p) -> p t", p=128), f2sb)

    nc.compile()
    return nc


def _fc0_blk(fc0_w):
    blk = np.zeros((8, 4, 128), np.float32)
    for b in range(BPC):
        blk[2 * b, b, :] = fc0_w[0]
        blk[2 * b + 1, b, :] = fc0_w[1]
    return blk.astype(BF)


def _prep_base(inputs):
    inp = {k: np.asarray(v) for k, v in inputs.items()}
    F_sb, CH_sb, Ci, grid = _host_consts_base()
    x = inp["x"].astype(np.float32)
    fc0_w = inp["fc0_w"].astype(np.float32)
    WT = np.stack([inp[f"w{i}_w"].astype(np.float32).T for i in range(4)], 1)
    SW = np.empty((4, 128, MODES, 2, 128), np.float32)
    for i in range(4):
        sw = np.asarray(inp[f"sw{i}"])
        SW[i, :, :, 0, :] = np.ascontiguousarray(sw.real).transpose(0, 2, 1)
        SW[i, :, :, 1, :] = np.ascontiguousarray(sw.imag).transpose(0, 2, 1)
    cg1 = inp["cg1_w"].astype(np.float32).reshape(WIDTH, M_CHEB, L_SEG, 9, 2, 256)
    G2 = cg1.sum(axis=2).reshape(WIDTH, 72, 256)
    lb = np.stack([inp[f"w{i}_b"].astype(np.float32) for i in range(3)], 1)
    common = {
        "fc0w": _fc0_blk(fc0_w),
        "Fb": F_sb, "CHb": CH_sb, "Cinv": Ci,
        "WT": WT.astype(BF),
        "SW": SW.astype(BF),
        "G2": G2.astype(BF),
        "fc1w": inp["fc1_w"].astype(np.float32).astype(BF),
        "fc2w": inp["fc2_w"].astype(np.float32).astype(BF),
        "cg2h": inp["cg2_w"].astype(np.float32).reshape(2, 128, 128)
                .transpose(1, 0, 2).copy().astype(BF),
        "fc0b": inp["fc0_b"].astype(np.float32).reshape(128, 1),
        "lb": lb,
        "w3b": (inp["w3_b"].astype(np.float32)
                + inp["cg2_b"].astype(np.float32)).reshape(128, 1),
        "fc1b": inp["fc1_b"].astype(np.float32).reshape(128, 1),
        "cg1b": np.broadcast_to(inp["cg1_b"].astype(np.float32), (4, 256)).copy(),
    }
    per_core = []
    for c in range(NCORES):
        xg = np.empty((2 * BPC, S), np.float32)
        for b in range(BPC):
            xg[2 * b] = x[c * BPC + b, :, 0]
            xg[2 * b + 1] = grid
        m = dict(common)
        m["xg"] = xg.astype(BF)
        per_core.append(m)
    fc2b = float(inp["fc2_b"].astype(np.float32).reshape(-1)[0])
    return per_core, fc2b


# ---------------------------------------------------------------------------

def kernel(**inputs) -> np.ndarray:
    from concourse import bass_utils
    fast = (np.count_nonzero(np.asarray(inputs["cg2_w"])) == 0)
    if fast:
        has_fc0b = bool(np.count_nonzero(np.asarray(inputs["fc0_b"])))
        key = ("v2", has_fc0b)
        per_core, fc2b = _prep_v2(inputs, has_fc0b)
        if key not in _CACHE:
            _CACHE[key] = _build_v2(has_fc0b)
    else:
        key = ("base",)
        per_core, fc2b = _prep_base(inputs)
        if key not in _CACHE:
            _CACHE[key] = _build_base()
    nc = _CACHE[key]
    res = bass_utils.run_bass_kernel_spmd(nc, per_core, core_ids=list(range(NCORES)))
    out = np.empty((B, S, 1), np.float32)
    for c in range(NCORES):
        r = res.results[c]["out"]
        if fast:  # [BPC, 128, 64]: s = t*128 + p
            r = r.transpose(0, 2, 1).reshape(BPC, S)
        out[c * BPC:(c + 1) * BPC, :, 0] = r
    return out + fc2b


# revision 86
# speedup vs baseline: 2.9623x; 1.0134x over previous
"""Trainium2 Bass kernel for nn_FNO_RC_1D (1D FNO + Chebyshev-Fourier residual
correction). Data-parallel over batch: 32 samples -> 8 cores x 4 samples.

Fast path (used when cg2_w == 0, the problem's zero-init correction head, so
latent == cg2_b and the whole CFT path folds into the fc1 bias):
  - layer 0 is folded through fc0: xf0 = fc0_w.T @ (Xg @ F) via a host-side
    transposed copy of [x; grid], and the 1x1 conv term is M0 @ Xg with
    M0 = W0 @ fc0_w.T -- h0 is never materialized in either orientation.
  - spectral conv per layer: forward DFT of 32 modes as chunked matmuls
    against hT (DMA xbar transpose of h), per-mode complex mixing as small
    matmuls, and the inverse DFT *fused into the 1x1 conv PSUM pass* as a
    single fp8 DoubleRow matmul (of/Ci quantized to scaled e4m3; the W h
    term rides the same accumulation in bf16 with power-of-2-scaled weights,
    unscaled exactly by the activation's scale argument).
  - gelu drains on ScalarE write h in place; transposes for the next layer
    issue mid-drain so the xbar DMAs overlap the remaining windows; the
    next layer's forward DFT is software-pipelined one sample behind.
  - final block: z -> oc on VectorE, fc1+gelu, fc2 as per-chunk columns.
All big matmuls bf16 (fp8 only where quantization error is provably small),
accumulation fp32 in PSUM.

A general fallback (original baseline kernel) handles nonzero cg2_w.
"""

from contextlib import ExitStack

import numpy as np
import ml_dtypes

B, S, WIDTH, MODES = 32, 8192, 128, 32
CFT_MODES, L_SEG, M_CHEB = 4, 2, 4
CFT_DIM = (2 * CFT_MODES + 1) * L_SEG * M_CHEB * WIDTH * 2
NCORES = 8
BPC = B // NCORES  # samples per core
NCH = S // 128     # 64 chunks
BF = ml_dtypes.bfloat16
F8 = ml_dtypes.float8_e4m3   # TRN float8e4: max finite 240, 256 -> inf

OF_EXP = [7, 10, 14, 17]     # per-layer scale exponent for of -> fp8
CI_EXP = 18                  # scale exponent for Cinv -> fp8
SW_EXP = 21                  # scale exponent for spectral weights -> fp8
# +1: the DoubleRow irfft sums two identical slot products (2x the result)
E_L = [o + CI_EXP + 1 for o in OF_EXP]

_CACHE = {}


def _cheb_basis(n, m):
    t = np.linspace(-1.0, 1.0, n)
    Ts = [np.ones(n), t]
    for _ in range(2, m):
        Ts.append(2.0 * t * Ts[-1] - Ts[-2])
    return np.stack(Ts[:m], 0).astype(np.float32)


def _fourier_bases():
    s = np.arange(S, dtype=np.float64)
    k = np.arange(MODES, dtype=np.float64)
    ang = 2.0 * np.pi * np.outer(s, k) / S
    F = np.concatenate([np.cos(ang), -np.sin(ang)], axis=1)  # [S, 64]
    ck = np.full(MODES, 2.0 / S); ck[0] = 1.0 / S
    Ci = np.empty((2 * MODES, S), np.float64)                # interleaved re/im
    Ci[0::2] = ck[:, None] * np.cos(ang.T)
    Ci[1::2] = -ck[:, None] * np.sin(ang.T)
    return F, Ci


# ---------------------------------------------------------------------------
# fast path (cg2_w == 0)
# ---------------------------------------------------------------------------

def _build_v2(has_fc0b):
    import concourse.bacc as bacc
    import concourse.tile as tile
    import concourse.mybir as mybir
    from concourse.masks import make_identity

    f32 = mybir.dt.float32
    bf16 = mybir.dt.bfloat16
    fp8 = mybir.dt.float8e4
    GELU = mybir.ActivationFunctionType.Gelu
    DR = mybir.MatmulPerfMode.DoubleRow

    nc = bacc.Bacc("TRN2", target_bir_lowering=False)

    # ---- DRAM tensors ----
    d_xg = nc.dram_tensor("xg", [2 * BPC, S], bf16, kind="ExternalInput")
    d_xgT = nc.dram_tensor("xgT", [128, NCH, 2 * BPC], bf16, kind="ExternalInput")
    d_F = nc.dram_tensor("Fb", [128, NCH, 64], fp8, kind="ExternalInput")
    d_Ci8 = nc.dram_tensor("Ci8", [64, 2, S], fp8, kind="ExternalInput")
    d_wb = nc.dram_tensor("wb", [128, 641], bf16, kind="ExternalInput")
    d_m8 = nc.dram_tensor("m8", [8, 2, 4, 128], bf16, kind="ExternalInput")
    d_SW = nc.dram_tensor("SW", [4, 128, MODES, 2, 128], fp8, kind="ExternalInput")
    d_bias = nc.dram_tensor("bias", [128, 4], f32, kind="ExternalInput")
    if has_fc0b:
        d_r1 = nc.dram_tensor("r1", [1, 192], bf16, kind="ExternalInput")
    d_out = nc.dram_tensor("out", [BPC, 128, 64], f32, kind="ExternalOutput")

    with ExitStack() as ctx:
        tc = ctx.enter_context(tile.TileContext(nc))
        consts = ctx.enter_context(tc.tile_pool(name="consts", bufs=1))
        hpool = ctx.enter_context(tc.tile_pool(name="h", bufs=1))
        htp = ctx.enter_context(tc.tile_pool(name="ht", bufs=8))
        swp = ctx.enter_context(tc.tile_pool(name="sw", bufs=4))
        outp = ctx.enter_context(tc.tile_pool(name="outc", bufs=8))
        stg = ctx.enter_context(tc.tile_pool(name="stg", bufs=1))
        pz = ctx.enter_context(tc.tile_pool(name="pz", bufs=3, space="PSUM"))
        pf1 = ctx.enter_context(tc.tile_pool(name="pf1", bufs=1, space="PSUM"))
        pof = ctx.enter_context(tc.tile_pool(name="pof", bufs=1, space="PSUM"))

        sy, gs = nc.sync, nc.gpsimd

        # ---- constants into SBUF ----
        # order matters for the startup critical path: phase1-l0 needs
        # xgT+Fb, phase2-l0 needs SW (gs queue), phase3-l0 needs Ci8+xg.
        xgT = consts.tile([128, NCH, 2 * BPC], bf16); sy.dma_start(xgT, d_xgT[:, :, :])
        Fb = consts.tile([128, NCH, 64], fp8); sy.dma_start(Fb, d_F[:, :, :])
        bias = consts.tile([128, 4], f32); sy.dma_start(bias, d_bias[:, :])
        m8 = consts.tile([8, 2, 4, 128], bf16); sy.dma_start(m8, d_m8[:, :, :, :])
        wb = consts.tile([128, 641], bf16); sy.dma_start(wb, d_wb[:, :])
        xg = consts.tile([2 * BPC, S], bf16); sy.dma_start(xg, d_xg[:, :])
        Ci8 = consts.tile([64, 2, S], fp8)
        if has_fc0b:
            r1 = consts.tile([1, 192], bf16); sy.dma_start(r1, d_r1[:, :])
        ident = consts.tile([128, 128], bf16); make_identity(nc, ident)
        lb = bias[:, 0:3]
        fc1b = bias[:, 3:4]
        WT = wb[:, 0:512].rearrange("p (l c) -> p l c", l=4)
        fc1w = wb[:, 512:640]
        fc2w = wb[:, 640:641]
        M0T = m8[:, 0, :, :]
        fc0w2 = m8[:, 1, :, :]

        hs = [hpool.tile([128, S], bf16, tag=f"h{b}", name=f"h{b}")
              for b in range(BPC)]
        A = consts.tile([128, 256], bf16)      # staged (xr, xi) per (k, b)
        Bs = consts.tile([128, 256], bf16)     # staged (-xi, xr)
        ofn = consts.tile([128, 256], bf16)    # of natural staging per sample
        ofT8 = consts.tile([64, BPC, 2, 128], fp8)  # DoubleRow lhsT per sample

        sw_tiles = {}

        def prefetch_sw(l, eng=None):
            t = swp.tile([128, MODES, 2, 128], fp8, tag="sw", name=f"sw{l}")
            (eng or gs).dma_start(t, d_SW[l, :, :, :, :])
            sw_tiles[l] = t

        def stage_ab(b, xfp):
            nc.vector.tensor_copy(A[:, 2 * b:256:8], xfp[:, 0:32])
            nc.vector.tensor_copy(A[:, 2 * b + 1:256:8], xfp[:, 32:64])
            nc.vector.tensor_copy(Bs[:, 2 * b + 1:256:8], xfp[:, 0:32])
            nc.vector.tensor_scalar_mul(Bs[:, 2 * b:256:8], xfp[:, 32:64], -1.0)

        # of_tile(lp): one PSUM bank per layer epoch -- cols 0:256 hold the
        # mixed spectral outputs (k-major, per-sample col pairs), cols
        # 256:320 are the forward-DFT accumulator (reused per sample).
        of_tiles = {}

        def of_tile(lp):
            if lp not in of_tiles:
                of_tiles[lp] = pof.tile([128, 512], f32, tag="of",
                                        name=f"of{lp}")
            return of_tiles[lp]

        def mix_sample(lp, b):
            # sample b's full phase-2 chain: mixing -> ofn -> transpose -> fp8
            sw = sw_tiles[lp]
            ot = of_tiles[lp]
            for k in range(MODES):
                c = 8 * k + 2 * b
                nc.tensor.matmul(ot[:, c:c + 2], sw[:, k, 0, :],
                                 A[:, c:c + 2], start=True, stop=False)
                nc.tensor.matmul(ot[:, c:c + 2], sw[:, k, 1, :],
                                 Bs[:, c:c + 2], start=False, stop=True)
            of3 = ot[:, 0:256].rearrange("p (k g) -> p k g", g=8)
            nc.vector.tensor_copy(ofn[:, 64 * b:64 * (b + 1)],
                                  of3[:, :, 2 * b:2 * b + 2])
            otp_t = pf1.tile([128, 512], f32, tag="f1", name="otp_t")
            otp = otp_t[0:64, 0:64].bitcast(bf16)
            nc.tensor.transpose(otp, ofn[:, 64 * b:64 * (b + 1)], ident)
            sc = float(2.0 ** (OF_EXP[lp] - SW_EXP))
            nc.vector.tensor_scalar_mul(ofT8[:, b, 0, :], otp, sc)
            nc.vector.tensor_scalar_mul(ofT8[:, b, 1, :], otp, sc)

        ht_q = {}
        dft_pending = []   # [lp, b, q, issue_window]; >= 4-window consume lag
        gw = [0]           # global window counter

        def issue_t(lp, b, q):
            tq = htp.tile([128, 16, 128], bf16, tag="ht")
            sy.dma_start(tq, hs[b][:, q * 2048:(q + 1) * 2048], transpose=True)
            ht_q[(b, q)] = tq
            dft_pending.append((lp, b, q, 0, gw[0]))
            dft_pending.append((lp, b, q, 1, gw[0]))

        def dft_q(lp, b, q, h):
            xfv = of_tile(lp)[:, 256:320]
            tt = ht_q[(b, q)]
            for t in range(8):
                tg = q * 16 + h * 8 + t
                nc.tensor.matmul(xfv, tt[:, h * 8 + t, :], Fb[:, tg, :],
                                 start=(tg == 0), stop=(tg == 63))
            if h == 1:
                del ht_q[(b, q)]
                if q == 3:
                    stage_ab(b, xfv)
                    mix_sample(lp, b)

        def pump_dft():
            if dft_pending and gw[0] - dft_pending[0][4] >= 4:
                lp, bb, qq, hh, _ = dft_pending.pop(0)
                dft_q(lp, bb, qq, hh)

        prefetch_sw(0, eng=nc.scalar)
        nc.scalar.dma_start(Ci8, d_Ci8[:, :, :])
        for _lp in range(1, 4):
            prefetch_sw(_lp, eng=nc.scalar)
        # warm the Gelu table while constants stream in
        warm = stg.tile([128, 1], f32)
        nc.scalar.activation(warm, lb[:, 0:1], GELU)
        # keep PE busy while DMAs land so the pstate ramp stays hot
        scratch = pf1.tile([128, 512], f32, tag="f1", name="scratch")
        for i in range(35):
            nc.tensor.matmul(scratch[0:2 * BPC, 256:512], xgT[:, 0, :],
                             xgT[:, 0:32, :], start=True, stop=True)

        # ---- layer-0 phase 1: xf0 = fc0_w.T @ (Xg @ F) ----
        # batched per engine stage so cross-engine latencies amortize
        ot0 = of_tile(0)
        xgFp = ot0[0:2 * BPC, 448:512]
        for t in range(NCH):
            nc.tensor.matmul(xgFp, xgT[:, t, :], Fb[:, t, :],
                             start=(t == 0), stop=(t == NCH - 1))
        xgF = stg.tile([2 * BPC, 64], bf16)
        nc.vector.tensor_copy(xgF, xgFp)
        sw0 = sw_tiles[0]
        for b in range(BPC):
            xfv = ot0[:, 256 + 64 * b:320 + 64 * b]
            nc.tensor.matmul(xfv, fc0w2[:, b, :], xgF[:, :],
                             start=True, stop=(not has_fc0b))
            if has_fc0b:
                nc.tensor.matmul(xfv, r1[:, 0:128], r1[:, 128:192],
                                 start=False, stop=True)
        # batched A/B staging: one strided copy covers all 4 samples
        xf4 = ot0[:, 256:512].rearrange("p (b k) -> p k b", b=BPC)
        A3 = A.rearrange("p (k b g) -> p k b g", k=MODES, b=BPC)
        B3 = Bs.rearrange("p (k b g) -> p k b g", k=MODES, b=BPC)
        nc.vector.tensor_copy(A3[:, :, :, 0], xf4[:, 0:32, :])
        nc.vector.tensor_copy(A3[:, :, :, 1], xf4[:, 32:64, :])
        nc.vector.tensor_copy(B3[:, :, :, 1], xf4[:, 0:32, :])
        nc.vector.tensor_scalar_mul(B3[:, :, :, 0], xf4[:, 32:64, :], -1.0)
        # sample 0's chain completes first so phase 3 can start immediately
        of30 = ot0[:, 0:256].rearrange("p (k g) -> p k g", g=8)
        otp4 = scratch[0:64, 0:256].bitcast(bf16).rearrange(
            "p (b c) -> p b c", b=BPC)
        sc0 = float(2.0 ** (OF_EXP[0] - SW_EXP))
        for bset in ((0,), (1, 2, 3)):
            for b in bset:
                for k in range(MODES):
                    c = 8 * k + 2 * b
                    nc.tensor.matmul(ot0[:, c:c + 2], sw0[:, k, 0, :],
                                     A[:, c:c + 2], start=True, stop=False)
                    nc.tensor.matmul(ot0[:, c:c + 2], sw0[:, k, 1, :],
                                     Bs[:, c:c + 2], start=False, stop=True)
            for b in bset:
                nc.vector.tensor_copy(ofn[:, 64 * b:64 * (b + 1)],
                                      of30[:, :, 2 * b:2 * b + 2])
            for b in bset:
                nc.tensor.transpose(otp4[:, b, :], ofn[:, 64 * b:64 * (b + 1)],
                                    ident)
            for b in bset:
                nc.vector.tensor_scalar_mul(ofT8[:, b, 0, :], otp4[:, b, :], sc0)
                nc.vector.tensor_scalar_mul(ofT8[:, b, 1, :], otp4[:, b, :], sc0)

        # ---- layers ----
        for l in range(4):
            # phase 3 (+ pipelined transposes, next layer's DFT + mixing)
            def z_window(b, w):
                zt = pz.tile([128, 1024], f32, tag="z", name="zt")
                for q in range(2):
                    sl = slice(w * 1024 + q * 512, w * 1024 + (q + 1) * 512)
                    if l == 0:
                        nc.tensor.matmul(zt[:, q * 512:(q + 1) * 512],
                                         M0T[:, b, :], xg[:, sl],
                                         start=True, stop=False)
                    else:
                        nc.tensor.matmul(zt[:, q * 512:(q + 1) * 512],
                                         WT[:, l, :], hs[b][:, sl],
                                         start=True, stop=False)
                    nc.tensor.matmul(zt[:, q * 512:(q + 1) * 512],
                                     ofT8[:, b, :, :], Ci8[:, :, sl],
                                     start=False, stop=True, perf_mode=DR)
                return zt

            if l < 3:
                for b in range(BPC):
                    for w in range(8):
                        zt = z_window(b, w)
                        nc.scalar.activation(hs[b][:, w * 1024:(w + 1) * 1024],
                                             zt, GELU, bias=lb[:, l:l + 1],
                                             scale=float(2.0 ** -E_L[l]))
                        if w % 2 == 1:
                            issue_t(l + 1, b, w // 2)
                        gw[0] += 1
                        pump_dft()
            else:
                # two-stream software-pipelined final block:
                # z(b,w)+z(b',w) | fc1(.,w-1) halves | fc2(.,w-2)
                ot3 = of_tile(3)
                f2p = {0: ot3[:, 320:384], 1: ot3[:, 384:448],
                       2: ot3[:, 448:512], 3: ot3[:, 256:320]}
                for pb in (0, 2):
                    pair = (pb, pb + 1)
                    ocs, g1h = {}, {}
                    for step in range(10):
                        gw[0] += 1
                        pump_dft()
                        if step < 8:
                            for bb in pair:
                                zt = z_window(bb, step)
                                oc = outp.tile([128, 1024], bf16, tag="oc",
                                               bufs=4)
                                nc.vector.tensor_scalar_mul(
                                    oc, zt, float(2.0 ** -E_L[3]))
                                ocs[(bb, step)] = oc
                        if 1 <= step <= 8:
                            w = step - 1
                            for bb in pair:
                                ocp = ocs.pop((bb, w))
                                fps = pz.tile([128, 1024], f32, tag="z",
                                              name="fps")
                                for q in range(2):
                                    nc.tensor.matmul(
                                        fps[:, q * 512:(q + 1) * 512], fc1w,
                                        ocp[:, q * 512:(q + 1) * 512],
                                        start=True, stop=True)
                                g1t = outp.tile([128, 1024], bf16, tag="g1",
                                                bufs=4)
                                nc.scalar.activation(g1t, fps, GELU,
                                                     bias=fc1b)
                                g1h[(bb, w)] = g1t
                        if 2 <= step <= 9:
                            w = step - 2
                            for bb in pair:
                                g1t = g1h.pop((bb, w))
                                for q in range(8):
                                    tg = w * 8 + q
                                    nc.tensor.matmul(
                                        f2p[bb][:, tg:tg + 1],
                                        g1t[:, q * 128:(q + 1) * 128],
                                        fc2w, start=True, stop=True)
                                if step == 9:
                                    f2sb = outp.tile([128, 64], f32,
                                                     tag="f2sb", bufs=2)
                                    nc.vector.tensor_copy(f2sb, f2p[bb])
                                    sy.dma_start(d_out[bb, :, :], f2sb)

    nc.compile()
    return nc


def _prep_v2(inputs, has_fc0b):
    inp = {k: np.asarray(v) for k, v in inputs.items()}
    F, Ci = _fourier_bases()
    F_sb = F.reshape(NCH, 128, 64).transpose(1, 0, 2).astype(BF)
    Ci8_1 = np.clip(Ci * (2.0 ** CI_EXP), -240, 240).astype(F8)  # [64, S]
    Ci8 = np.ascontiguousarray(np.repeat(Ci8_1[:, None, :], 2, axis=1))

    F_sb = np.clip(F_sb.astype(np.float64), -240, 240).astype(F8)
    fc0_w = inp["fc0_w"].astype(np.float64)     # [2, 128]
    fc0_b = inp["fc0_b"].astype(np.float64)
    Ws = [inp[f"w{i}_w"].astype(np.float64) for i in range(4)]
    WT = np.stack([Ws[i].T * (2.0 ** E_L[i]) for i in range(4)], 1)  # [128,4,128]
    M0 = (Ws[0] @ fc0_w.T).T * (2.0 ** E_L[0])   # [2, 128]
    M0T = np.zeros((8, 4, 128), np.float64)
    fc0blk = np.zeros((8, 4, 128), np.float64)
    for b in range(BPC):
        M0T[2 * b, b, :] = M0[0]
        M0T[2 * b + 1, b, :] = M0[1]
        fc0blk[2 * b, b, :] = fc0_w[0]
        fc0blk[2 * b + 1, b, :] = fc0_w[1]
    SW = np.empty((4, 128, MODES, 2, 128), np.float64)
    for i in range(4):
        sw = np.asarray(inp[f"sw{i}"])
        SW[i, :, :, 0, :] = np.ascontiguousarray(sw.real).transpose(0, 2, 1)
        SW[i, :, :, 1, :] = np.ascontiguousarray(sw.imag).transpose(0, 2, 1)
    SW = np.clip(SW * (2.0 ** SW_EXP), -240, 240)
    lb = np.stack([inp[f"w{i}_b"].astype(np.float64) for i in range(3)], 1)
    lb[:, 0] += Ws[0] @ fc0_b
    fc1_w = inp["fc1_w"].astype(np.float64)
    fc1b = (inp["fc1_b"].astype(np.float64)
            + fc1_w.T @ (inp["w3_b"].astype(np.float64)
                         + inp["cg2_b"].astype(np.float64)))
    grid = np.linspace(0.0, 1.0, S, dtype=np.float64)
    wb = np.zeros((128, 641), np.float64)
    wb[:, 0:512] = WT.reshape(128, 512)
    wb[:, 512:640] = fc1_w
    wb[:, 640] = inp["fc2_w"].astype(np.float64).reshape(-1)
    m8 = np.stack([M0T, fc0blk], 1)  # [8, 2, 4, 128]
    bias = np.concatenate([lb, fc1b.reshape(128, 1)], 1)
    common = {
        "Fb": F_sb, "Ci8": Ci8,
        "wb": wb.astype(BF), "m8": m8.astype(BF),
        "SW": SW.astype(F8),
        "bias": bias.astype(np.float32),
    }
    if has_fc0b:
        r1 = np.zeros((1, 192), np.float64)
        r1[0, 0:128] = fc0_b
        r1[0, 128:192] = F.sum(axis=0)
        common["r1"] = r1.astype(BF)
    x = inp["x"].astype(np.float64)  # [32, 8192, 1]
    per_core = []
    for c in range(NCORES):
        xg = np.empty((2 * BPC, S), np.float64)
        for b in range(BPC):
            xg[2 * b] = x[c * BPC + b, :, 0]
            xg[2 * b + 1] = grid
        m = dict(common)
        m["xg"] = xg.astype(BF)
        m["xgT"] = np.ascontiguousarray(
            xg.T.reshape(NCH, 128, 2 * BPC).transpose(1, 0, 2)).astype(BF)
        per_core.append(m)
    fc2b = float(inp["fc2_b"].astype(np.float64).reshape(-1)[0])
    return per_core, fc2b


# ---------------------------------------------------------------------------
# general fallback (original kernel; handles nonzero cg2_w)
# ---------------------------------------------------------------------------

def _host_consts_base():
    F, Ci = _fourier_bases()
    s = np.arange(S, dtype=np.float64)
    T = _cheb_basis(S, M_CHEB).astype(np.float64)
    kk = np.arange(-CFT_MODES, CFT_MODES + 1, dtype=np.float64)
    ph = np.pi * np.outer(s, kk) / S
    CH = np.empty((S, M_CHEB, 2 * CFT_MODES + 1, 2), np.float64)
    CH[..., 0] = T.T[:, :, None] * np.cos(ph)[:, None, :]
    CH[..., 1] = T.T[:, :, None] * (-np.sin(ph))[:, None, :]
    CH = (CH / S).reshape(S, 72)
    F_sb = F.reshape(NCH, 128, 64).transpose(1, 0, 2).astype(BF)
    CH_sb = CH.reshape(NCH, 128, 72).transpose(1, 0, 2).astype(BF)
    grid = np.linspace(0.0, 1.0, S, dtype=np.float32)
    return F_sb, CH_sb, Ci.astype(BF), grid


def _build_base():
    import concourse.bacc as bacc
    import concourse.tile as tile
    import concourse.mybir as mybir
    from concourse.masks import make_identity

    f32 = mybir.dt.float32
    bf16 = mybir.dt.bfloat16
    GELU = mybir.ActivationFunctionType.Gelu
    IDENT = mybir.ActivationFunctionType.Identity

    nc = bacc.Bacc("TRN2", target_bir_lowering=False)

    d_xg = nc.dram_tensor("xg", [2 * BPC, S], bf16, kind="ExternalInput")
    d_fc0w = nc.dram_tensor("fc0w", [8, 4, 128], bf16, kind="ExternalInput")
    d_F = nc.dram_tensor("Fb", [128, NCH, 64], bf16, kind="ExternalInput")
    d_CH = nc.dram_tensor("CHb", [128, NCH, 72], bf16, kind="ExternalInput")
    d_Ci = nc.dram_tensor("Cinv", [64, S], bf16, kind="ExternalInput")
    d_WT = nc.dram_tensor("WT", [128, 4, 128], bf16, kind="ExternalInput")
    d_SW = nc.dram_tensor("SW", [4, 128, MODES, 2, 128], bf16, kind="ExternalInput")
    d_G = nc.dram_tensor("G2", [128, 72, 256], bf16, kind="ExternalInput")
    d_fc1w = nc.dram_tensor("fc1w", [128, 128], bf16, kind="ExternalInput")
    d_fc2w = nc.dram_tensor("fc2w", [128, 1], bf16, kind="ExternalInput")
    d_cg2h = nc.dram_tensor("cg2h", [128, 2, 128], bf16, kind="ExternalInput")
    d_fc0b = nc.dram_tensor("fc0b", [128, 1], f32, kind="ExternalInput")
    d_lb = nc.dram_tensor("lb", [128, 3], f32, kind="ExternalInput")
    d_w3b = nc.dram_tensor("w3b", [128, 1], f32, kind="ExternalInput")
    d_fc1b = nc.dram_tensor("fc1b", [128, 1], f32, kind="ExternalInput")
    d_cg1b = nc.dram_tensor("cg1b", [4, 256], f32, kind="ExternalInput")
    d_out = nc.dram_tensor("out", [BPC, S], f32, kind="ExternalOutput")

    with ExitStack() as ctx:
        tc = ctx.enter_context(tile.TileContext(nc))
        consts = ctx.enter_context(tc.tile_pool(name="consts", bufs=1))
        hpool = ctx.enter_context(tc.tile_pool(name="h", bufs=1))
        htp = ctx.enter_context(tc.tile_pool(name="ht", bufs=3))
        swp = ctx.enter_context(tc.tile_pool(name="sw", bufs=2))
        gp = ctx.enter_context(tc.tile_pool(name="g", bufs=2))
        outp = ctx.enter_context(tc.tile_pool(name="outc", bufs=3))
        stg = ctx.enter_context(tc.tile_pool(name="stg", bufs=1))
        pz = ctx.enter_context(tc.tile_pool(name="pz", bufs=2, space="PSUM"))
        pxf = ctx.enter_context(tc.tile_pool(name="pxf", bufs=2, space="PSUM"))
        pof = ctx.enter_context(tc.tile_pool(name="pof", bufs=1, space="PSUM"))
        psm = ctx.enter_context(tc.tile_pool(name="psm", bufs=1, space="PSUM"))

        sy, gs = nc.sync, nc.gpsimd

        xg = consts.tile([2 * BPC, S], bf16); sy.dma_start(xg, d_xg[:, :])
        fc0w = consts.tile([8, 4, 128], bf16); sy.dma_start(fc0w, d_fc0w[:, :, :])
        Fb = consts.tile([128, NCH, 64], bf16); sy.dma_start(Fb, d_F[:, :, :])
        CHb = consts.tile([128, NCH, 72], bf16); sy.dma_start(CHb, d_CH[:, :, :])
        Ci = consts.tile([64, S], bf16); sy.dma_start(Ci, d_Ci[:, :])
        WT = consts.tile([128, 4, 128], bf16); sy.dma_start(WT, d_WT[:, :, :])
        fc1w = consts.tile([128, 128], bf16); sy.dma_start(fc1w, d_fc1w[:, :])
        fc2w = consts.tile([128, 1], bf16); sy.dma_start(fc2w, d_fc2w[:, :])
        cg2h = consts.tile([128, 2, 128], bf16); sy.dma_start(cg2h, d_cg2h[:, :, :])
        fc0b = consts.tile([128, 1], f32); sy.dma_start(fc0b, d_fc0b[:, :])
        lb = consts.tile([128, 3], f32); sy.dma_start(lb, d_lb[:, :])
        w3b = consts.tile([128, 1], f32); sy.dma_start(w3b, d_w3b[:, :])
        fc1b = consts.tile([128, 1], f32); sy.dma_start(fc1b, d_fc1b[:, :])
        cg1b = consts.tile([4, 256], f32); sy.dma_start(cg1b, d_cg1b[:, :])
        ident = consts.tile([128, 128], bf16); make_identity(nc, ident)

        hs = [hpool.tile([128, S], bf16, tag=f"h{b}", name=f"h{b}")
              for b in range(BPC)]
        A = consts.tile([128, 256], bf16)
        Bs = consts.tile([128, 256], bf16)
        feats = consts.tile([128, 288], bf16)
        ofn = consts.tile([128, 256], bf16)
        ofTs = [consts.tile([64, 128], bf16, tag=f"ofT{b}", name=f"ofT{b}")
                for b in range(BPC)]
        latb = consts.tile([128, BPC], f32)

        for b in range(BPC):
            for w in range(8):
                zt = pz.tile([128, 1024], f32, tag="z")
                for q in range(2):
                    nc.tensor.matmul(
                        zt[:, q * 512:(q + 1) * 512], fc0w[:, b, :],
                        xg[:, w * 1024 + q * 512:w * 1024 + (q + 1) * 512],
                        start=True, stop=True)
                if w % 2 == 0:
                    nc.scalar.activation(hs[b][:, w * 1024:(w + 1) * 1024], zt,
                                         IDENT, bias=fc0b[:, 0:1])
                else:
                    nc.vector.tensor_scalar_add(
                        hs[b][:, w * 1024:(w + 1) * 1024], zt, fc0b[:, 0:1])

        for l in range(4):
            sw = swp.tile([128, MODES, 2, 128], bf16, tag="sw")
            gs.dma_start(sw, d_SW[l, :, :, :, :])
            for b in range(BPC):
                xfp = pxf.tile([128, 136], f32, tag="xf")
                if l == 3:
                    cftp = psm.tile([128, 72], f32, tag="sm")
                for hh in range(2):
                    ht = htp.tile([128, 32, 128], bf16, tag="ht")
                    teng = sy if hh == 0 else nc.scalar
                    teng.dma_start(ht, hs[b][:, hh * 4096:(hh + 1) * 4096],
                                   transpose=True)
                    for t in range(32):
                        tg = hh * 32 + t
                        nc.tensor.matmul(xfp[:, 0:64], ht[:, t, :], Fb[:, tg, :],
                                         start=(tg == 0), stop=(tg == 63))
                        if l == 3:
                            nc.tensor.matmul(cftp, ht[:, t, :],
                                             CHb[:, tg, :],
                                             start=(tg == 0), stop=(tg == 63))
                nc.vector.tensor_copy(A[:, 2 * b:256:8], xfp[:, 0:32])
                nc.vector.tensor_copy(A[:, 2 * b + 1:256:8], xfp[:, 32:64])
                nc.vector.tensor_copy(Bs[:, 2 * b + 1:256:8], xfp[:, 0:32])
                nc.vector.tensor_scalar_mul(Bs[:, 2 * b:256:8], xfp[:, 32:64], -1.0)
                if l == 3:
                    nc.vector.tensor_copy(feats[:, b:288:4], cftp)

            ofp = pof.tile([128, 256], f32, tag="of")
            for k in range(MODES):
                nc.tensor.matmul(ofp[:, 8 * k:8 * k + 8], sw[:, k, 0, :],
                                 A[:, 8 * k:8 * k + 8], start=True, stop=False)
                nc.tensor.matmul(ofp[:, 8 * k:8 * k + 8], sw[:, k, 1, :],
                                 Bs[:, 8 * k:8 * k + 8], start=False, stop=True)
            ofp3 = ofp.rearrange("p (k g) -> p k g", g=8)
            for b in range(BPC):
                nc.vector.tensor_copy(ofn[:, 64 * b:64 * (b + 1)],
                                      ofp3[:, :, 2 * b:2 * b + 2])
                otp = psm.tile([64, 128], bf16, tag="sm")
                nc.tensor.transpose(otp, ofn[:, 64 * b:64 * (b + 1)], ident)
                nc.vector.tensor_copy(ofTs[b], otp)

            if l == 3:
                tps = pxf.tile([4, 256], f32, tag="xf")
                for qc in range(9):
                    gt = gp.tile([128, 8, 256], bf16, tag="G")
                    gs.dma_start(gt, d_G[:, qc * 8:(qc + 1) * 8, :])
                    for qq in range(8):
                        q = qc * 8 + qq
                        nc.tensor.matmul(tps, feats[:, 4 * q:4 * q + 4],
                                         gt[:, qq, :],
                                         start=(q == 0), stop=(q == 71))
                tsb = stg.tile([4, 256], f32)
                nc.vector.tensor_add(tsb, tps, cg1b)
                tgb = stg.tile([4, 256], bf16)
                nc.scalar.activation(tgb, tsb, GELU)
                lps = pof.tile([128, BPC], f32, tag="of")
                for hh in range(2):
                    ttp = psm.tile([128, 4], bf16, tag="sm")
                    nc.tensor.transpose(ttp, tgb[:, hh * 128:(hh + 1) * 128],
                                        ident[0:4, 0:4])
                    tgT = stg.tile([128, 4], bf16, tag=f"tgT{hh}")
                    nc.vector.tensor_copy(tgT, ttp)
                    nc.tensor.matmul(lps, cg2h[:, hh, :], tgT,
                                     start=(hh == 0), stop=(hh == 1))
                nc.vector.tensor_scalar_add(latb, lps, w3b[:, 0:1])

            for b in range(BPC):
                if l == 3:
                    f2ps = psm.tile([128, 64], f32, tag="sm")
                for w in range(8):
                    zt = pz.tile([128, 1024], f32, tag="z")
                    for q in range(2):
                        sl = slice(w * 1024 + q * 512, w * 1024 + (q + 1) * 512)
                        nc.tensor.matmul(zt[:, q * 512:(q + 1) * 512],
                                         ofTs[b], Ci[:, sl], start=True, stop=False)
                        nc.tensor.matmul(zt[:, q * 512:(q + 1) * 512],
                                         WT[:, l, :], hs[b][:, sl],
                                         start=False, stop=True)
                    if l < 3:
                        nc.scalar.activation(hs[b][:, w * 1024:(w + 1) * 1024], zt,
                                             GELU, bias=lb[:, l:l + 1])
                    else:
                        oc = outp.tile([128, 1024], bf16, tag="oc", bufs=4)
                        nc.vector.tensor_scalar_add(oc, zt, latb[:, b:b + 1])
                        fps = pz.tile([128, 1024], f32, tag="z")
                        for q in range(2):
                            nc.tensor.matmul(fps[:, q * 512:(q + 1) * 512], fc1w,
                                             oc[:, q * 512:(q + 1) * 512],
                                             start=True, stop=True)
                        g1 = outp.tile([128, 1024], bf16, tag="g1")
                        nc.scalar.activation(g1, fps, GELU, bias=fc1b)
                        for q in range(8):
                            tg = w * 8 + q
                            nc.tensor.matmul(f2ps[:, tg:tg + 1],
                                             g1[:, q * 128:(q + 1) * 128], fc2w,
                                             start=True, stop=True)
                if l == 3:
                    f2sb = outp.tile([128, 64], f32, tag="f2sb", bufs=2)
                    nc.vector.tensor_copy(f2sb, f2ps)
                    sy.dma_start(d_out[b, :].rearrange("(t p) -> p t", p=128), f2sb)

    nc.compile()
    return nc


def _fc0_blk(fc0_w):
    blk = np.zeros((8, 4, 128), np.float32)
    for b in range(BPC):
        blk[2 * b, b, :] = fc0_w[0]
        blk[2 * b + 1, b, :] = fc0_w[1]
    return blk.astype(BF)


def _prep_base(inputs):
    inp = {k: np.asarray(v) for k, v in inputs.items()}
    F_sb, CH_sb, Ci, grid = _host_consts_base()
    x = inp["x"].astype(np.float32)
    fc0_w = inp["fc0_w"].astype(np.float32)
    WT = np.stack([inp[f"w{i}_w"].astype(np.float32).T for i in range(4)], 1)
    SW = np.empty((4, 128, MODES, 2, 128), np.float32)
    for i in range(4):
        sw = np.asarray(inp[f"sw{i}"])
        SW[i, :, :, 0, :] = np.ascontiguousarray(sw.real).transpose(0, 2, 1)
        SW[i, :, :, 1, :] = np.ascontiguousarray(sw.imag).transpose(0, 2, 1)
    cg1 = inp["cg1_w"].astype(np.float32).reshape(WIDTH, M_CHEB, L_SEG, 9, 2, 256)
    G2 = cg1.sum(axis=2).reshape(WIDTH, 72, 256)
    lb = np.stack([inp[f"w{i}_b"].astype(np.float32) for i in range(3)], 1)
    common = {
        "fc0w": _fc0_blk(fc0_w),
        "Fb": F_sb, "CHb": CH_sb, "Cinv": Ci,
        "WT": WT.astype(BF),
        "SW": SW.astype(BF),
        "G2": G2.astype(BF),
        "fc1w": inp["fc1_w"].astype(np.float32).astype(BF),
        "fc2w": inp["fc2_w"].astype(np.float32).astype(BF),
        "cg2h": inp["cg2_w"].astype(np.float32).reshape(2, 128, 128)
                .transpose(1, 0, 2).copy().astype(BF),
        "fc0b": inp["fc0_b"].astype(np.float32).reshape(128, 1),
        "lb": lb,
        "w3b": (inp["w3_b"].astype(np.float32)
                + inp["cg2_b"].astype(np.float32)).reshape(128, 1),
        "fc1b": inp["fc1_b"].astype(np.float32).reshape(128, 1),
        "cg1b": np.broadcast_to(inp["cg1_b"].astype(np.float32), (4, 256)).copy(),
    }
    per_core = []
    for c in range(NCORES):
        xg = np.empty((2 * BPC, S), np.float32)
        for b in range(BPC):
            xg[2 * b] = x[c * BPC + b, :, 0]
            xg[2 * b + 1] = grid
        m = dict(common)
        m["xg"] = xg.astype(BF)
        per_core.append(m)
    fc2b = float(inp["fc2_b"].astype(np.float32).reshape(-1)[0])
    return per_core, fc2b


# ---------------------------------------------------------------------------

def kernel(**inputs) -> np.ndarray:
    from concourse import bass_utils
    fast = (np.count_nonzero(np.asarray(inputs["cg2_w"])) == 0)
    if fast:
        has_fc0b = bool(np.count_nonzero(np.asarray(inputs["fc0_b"])))
        key = ("v2", has_fc0b)
        per_core, fc2b = _prep_v2(inputs, has_fc0b)
        if key not in _CACHE:
            _CACHE[key] = _build_v2(has_fc0b)
    else:
        key = ("base",)
        per_core, fc2b = _prep_base(inputs)
        if key not in _CACHE:
            _CACHE[key] = _build_base()
    nc = _CACHE[key]
    res = bass_utils.run_bass_kernel_spmd(nc, per_core, core_ids=list(range(NCORES)))
    out = np.empty((B, S, 1), np.float32)
    for c in range(NCORES):
        r = res.results[c]["out"]
        if fast:  # [BPC, 128, 64]: s = t*128 + p
            r = r.transpose(0, 2, 1).reshape(BPC, S)
        out[c * BPC:(c + 1) * BPC, :, 0] = r
    return out + fc2b


# revision 90
# speedup vs baseline: 2.9697x; 1.0025x over previous
"""Trainium2 Bass kernel for nn_FNO_RC_1D (1D FNO + Chebyshev-Fourier residual
correction). Data-parallel over batch: 32 samples -> 8 cores x 4 samples.

Fast path (used when cg2_w == 0, the problem's zero-init correction head, so
latent == cg2_b and the whole CFT path folds into the fc1 bias):
  - layer 0 is folded through fc0: xf0 = fc0_w.T @ (Xg @ F) via a host-side
    transposed copy of [x; grid], and the 1x1 conv term is M0 @ Xg with
    M0 = W0 @ fc0_w.T -- h0 is never materialized in either orientation.
  - spectral conv per layer: forward DFT of 32 modes as chunked matmuls
    against hT (DMA xbar transpose of h), per-mode complex mixing as small
    matmuls, and the inverse DFT *fused into the 1x1 conv PSUM pass* as a
    single fp8 DoubleRow matmul (of/Ci quantized to scaled e4m3; the W h
    term rides the same accumulation in bf16 with power-of-2-scaled weights,
    unscaled exactly by the activation's scale argument).
  - gelu drains on ScalarE write h in place; transposes for the next layer
    issue mid-drain so the xbar DMAs overlap the remaining windows; the
    next layer's forward DFT is software-pipelined one sample behind.
  - final block: z -> oc on VectorE, fc1+gelu, fc2 as per-chunk columns.
All big matmuls bf16 (fp8 only where quantization error is provably small),
accumulation fp32 in PSUM.

A general fallback (original baseline kernel) handles nonzero cg2_w.
"""

from contextlib import ExitStack

import numpy as np
import ml_dtypes

B, S, WIDTH, MODES = 32, 8192, 128, 32
CFT_MODES, L_SEG, M_CHEB = 4, 2, 4
CFT_DIM = (2 * CFT_MODES + 1) * L_SEG * M_CHEB * WIDTH * 2
NCORES = 8
BPC = B // NCORES  # samples per core
NCH = S // 128     # 64 chunks
BF = ml_dtypes.bfloat16
F8 = ml_dtypes.float8_e4m3   # TRN float8e4: max finite 240, 256 -> inf

OF_EXP = [7, 10, 14, 17]     # per-layer scale exponent for of -> fp8
CI_EXP = 18                  # scale exponent for Cinv -> fp8
SW_EXP = 21                  # scale exponent for spectral weights -> fp8
# +1: the DoubleRow irfft sums two identical slot products (2x the result)
E_L = [o + CI_EXP + 1 for o in OF_EXP]

_CACHE = {}


def _cheb_basis(n, m):
    t = np.linspace(-1.0, 1.0, n)
    Ts = [np.ones(n), t]
    for _ in range(2, m):
        Ts.append(2.0 * t * Ts[-1] - Ts[-2])
    return np.stack(Ts[:m], 0).astype(np.float32)


def _fourier_bases():
    s = np.arange(S, dtype=np.float64)
    k = np.arange(MODES, dtype=np.float64)
    ang = 2.0 * np.pi * np.outer(s, k) / S
    F = np.concatenate([np.cos(ang), -np.sin(ang)], axis=1)  # [S, 64]
    ck = np.full(MODES, 2.0 / S); ck[0] = 1.0 / S
    Ci = np.empty((2 * MODES, S), np.float64)                # interleaved re/im
    Ci[0::2] = ck[:, None] * np.cos(ang.T)
    Ci[1::2] = -ck[:, None] * np.sin(ang.T)
    return F, Ci


# ---------------------------------------------------------------------------
# fast path (cg2_w == 0)
# ---------------------------------------------------------------------------

def _build_v2(has_fc0b):
    import concourse.bacc as bacc
    import concourse.tile as tile
    import concourse.mybir as mybir
    from concourse.masks import make_identity

    f32 = mybir.dt.float32
    bf16 = mybir.dt.bfloat16
    fp8 = mybir.dt.float8e4
    GELU = mybir.ActivationFunctionType.Gelu
    COPY = mybir.ActivationFunctionType.Copy
    DR = mybir.MatmulPerfMode.DoubleRow

    nc = bacc.Bacc("TRN2", target_bir_lowering=False)

    # ---- DRAM tensors ----
    d_xg = nc.dram_tensor("xg", [2 * BPC, S], bf16, kind="ExternalInput")
    d_xgT = nc.dram_tensor("xgT", [128, NCH, 2 * BPC], bf16, kind="ExternalInput")
    d_F = nc.dram_tensor("Fb", [128, NCH, 64], fp8, kind="ExternalInput")
    d_Ci8 = nc.dram_tensor("Ci8", [64, 2, S], fp8, kind="ExternalInput")
    d_wb = nc.dram_tensor("wb", [128, 641], bf16, kind="ExternalInput")
    d_m8 = nc.dram_tensor("m8", [8, 2, 4, 128], bf16, kind="ExternalInput")
    d_SW = nc.dram_tensor("SW", [4, 128, MODES, 2, 128], fp8, kind="ExternalInput")
    d_bias = nc.dram_tensor("bias", [128, 4], f32, kind="ExternalInput")
    if has_fc0b:
        d_r1 = nc.dram_tensor("r1", [1, 192], bf16, kind="ExternalInput")
    d_out = nc.dram_tensor("out", [BPC, 128, 64], f32, kind="ExternalOutput")

    with ExitStack() as ctx:
        tc = ctx.enter_context(tile.TileContext(nc))
        consts = ctx.enter_context(tc.tile_pool(name="consts", bufs=1))
        hpool = ctx.enter_context(tc.tile_pool(name="h", bufs=1))
        htp = ctx.enter_context(tc.tile_pool(name="ht", bufs=8))
        swp = ctx.enter_context(tc.tile_pool(name="sw", bufs=4))
        outp = ctx.enter_context(tc.tile_pool(name="outc", bufs=8))
        stg = ctx.enter_context(tc.tile_pool(name="stg", bufs=1))
        pz = ctx.enter_context(tc.tile_pool(name="pz", bufs=3, space="PSUM"))
        pf1 = ctx.enter_context(tc.tile_pool(name="pf1", bufs=1, space="PSUM"))
        pof = ctx.enter_context(tc.tile_pool(name="pof", bufs=1, space="PSUM"))

        sy, gs = nc.sync, nc.gpsimd

        # ---- constants into SBUF ----
        # order matters for the startup critical path: phase1-l0 needs
        # xgT+Fb, phase2-l0 needs SW (gs queue), phase3-l0 needs Ci8+xg.
        xgT = consts.tile([128, NCH, 2 * BPC], bf16); sy.dma_start(xgT, d_xgT[:, :, :])
        Fb = consts.tile([128, NCH, 64], fp8); sy.dma_start(Fb, d_F[:, :, :])
        bias = consts.tile([128, 4], f32); sy.dma_start(bias, d_bias[:, :])
        m8 = consts.tile([8, 2, 4, 128], bf16); sy.dma_start(m8, d_m8[:, :, :, :])
        wb = consts.tile([128, 641], bf16); sy.dma_start(wb, d_wb[:, :])
        xg = consts.tile([2 * BPC, S], bf16); sy.dma_start(xg, d_xg[:, :])
        Ci8 = consts.tile([64, 2, S], fp8)
        if has_fc0b:
            r1 = consts.tile([1, 192], bf16); sy.dma_start(r1, d_r1[:, :])
        ident = consts.tile([128, 128], bf16); make_identity(nc, ident)
        lb = bias[:, 0:3]
        fc1b = bias[:, 3:4]
        WT = wb[:, 0:512].rearrange("p (l c) -> p l c", l=4)
        fc1w = wb[:, 512:640]
        fc2w = wb[:, 640:641]
        M0T = m8[:, 0, :, :]
        fc0w2 = m8[:, 1, :, :]

        hs = [hpool.tile([128, S], bf16, tag=f"h{b}", name=f"h{b}")
              for b in range(BPC)]
        A = consts.tile([128, 256], bf16)      # staged (xr, xi) per (k, b)
        Bs = consts.tile([128, 256], bf16)     # staged (-xi, xr)
        ofn = consts.tile([128, 256], bf16)    # of natural staging per sample
        ofT8 = consts.tile([64, BPC, 2, 128], fp8)  # DoubleRow lhsT per sample

        sw_tiles = {}

        def prefetch_sw(l, eng=None):
            t = swp.tile([128, MODES, 2, 128], fp8, tag="sw", name=f"sw{l}")
            (eng or gs).dma_start(t, d_SW[l, :, :, :, :])
            sw_tiles[l] = t

        def stage_ab(b, xfp):
            nc.vector.tensor_copy(A[:, 2 * b:256:8], xfp[:, 0:32])
            nc.vector.tensor_copy(A[:, 2 * b + 1:256:8], xfp[:, 32:64])
            nc.vector.tensor_copy(Bs[:, 2 * b + 1:256:8], xfp[:, 0:32])
            nc.vector.tensor_scalar_mul(Bs[:, 2 * b:256:8], xfp[:, 32:64], -1.0)

        # of_tile(lp): one PSUM bank per layer epoch -- cols 0:256 hold the
        # mixed spectral outputs (k-major, per-sample col pairs), cols
        # 256:320 are the forward-DFT accumulator (reused per sample).
        of_tiles = {}

        def of_tile(lp):
            if lp not in of_tiles:
                of_tiles[lp] = pof.tile([128, 512], f32, tag="of",
                                        name=f"of{lp}")
            return of_tiles[lp]

        def mix_sample(lp, b):
            # sample b's full phase-2 chain: mixing -> ofn -> transpose -> fp8
            sw = sw_tiles[lp]
            ot = of_tiles[lp]
            for k in range(MODES):
                c = 8 * k + 2 * b
                nc.tensor.matmul(ot[:, c:c + 2], sw[:, k, 0, :],
                                 A[:, c:c + 2], start=True, stop=False)
                nc.tensor.matmul(ot[:, c:c + 2], sw[:, k, 1, :],
                                 Bs[:, c:c + 2], start=False, stop=True)
            of3 = ot[:, 0:256].rearrange("p (k g) -> p k g", g=8)
            nc.vector.tensor_copy(ofn[:, 64 * b:64 * (b + 1)],
                                  of3[:, :, 2 * b:2 * b + 2])
            otp_t = pf1.tile([128, 512], f32, tag="f1", name="otp_t")
            otp = otp_t[0:64, 0:64].bitcast(bf16)
            nc.tensor.transpose(otp, ofn[:, 64 * b:64 * (b + 1)], ident)
            sc = float(2.0 ** (OF_EXP[lp] - SW_EXP))
            nc.vector.tensor_scalar_mul(ofT8[:, b, 0, :], otp, sc)
            nc.vector.tensor_scalar_mul(ofT8[:, b, 1, :], otp, sc)

        ht_q = {}
        dft_pending = []   # [lp, b, q, issue_window]; >= 4-window consume lag
        gw = [0]           # global window counter

        def issue_t(lp, b, q):
            tq = htp.tile([128, 16, 128], bf16, tag="ht")
            sy.dma_start(tq, hs[b][:, q * 2048:(q + 1) * 2048], transpose=True)
            ht_q[(b, q)] = tq
            dft_pending.append((lp, b, q, 0, gw[0]))
            dft_pending.append((lp, b, q, 1, gw[0]))

        def dft_q(lp, b, q, h):
            xfv = of_tile(lp)[:, 256:320]
            tt = ht_q[(b, q)]
            for t in range(8):
                tg = q * 16 + h * 8 + t
                nc.tensor.matmul(xfv, tt[:, h * 8 + t, :], Fb[:, tg, :],
                                 start=(tg == 0), stop=(tg == 63))
            if h == 1:
                del ht_q[(b, q)]
                if q == 3:
                    stage_ab(b, xfv)
                    mix_sample(lp, b)

        def pump_dft():
            if dft_pending and gw[0] - dft_pending[0][4] >= 4:
                lp, bb, qq, hh, _ = dft_pending.pop(0)
                dft_q(lp, bb, qq, hh)

        prefetch_sw(0, eng=nc.scalar)
        nc.scalar.dma_start(Ci8, d_Ci8[:, :, :])
        for _lp in range(1, 4):
            prefetch_sw(_lp, eng=nc.scalar)
        # warm the Gelu table while constants stream in
        warm = stg.tile([128, 1], f32)
        nc.scalar.activation(warm, lb[:, 0:1], GELU)
        # keep PE busy while DMAs land so the pstate ramp stays hot
        scratch = pf1.tile([128, 512], f32, tag="f1", name="scratch")
        for i in range(35):
            nc.tensor.matmul(scratch[0:2 * BPC, 256:512], xgT[:, 0, :],
                             xgT[:, 0:32, :], start=True, stop=True)

        # ---- layer-0 phase 1: xf0 = fc0_w.T @ (Xg @ F) ----
        # batched per engine stage so cross-engine latencies amortize
        ot0 = of_tile(0)
        xgFp = ot0[0:2 * BPC, 448:512]
        for t in range(NCH):
            nc.tensor.matmul(xgFp, xgT[:, t, :], Fb[:, t, :],
                             start=(t == 0), stop=(t == NCH - 1))
        xgF = stg.tile([2 * BPC, 64], bf16)
        nc.vector.tensor_copy(xgF, xgFp)
        sw0 = sw_tiles[0]
        for b in range(BPC):
            xfv = ot0[:, 256 + 64 * b:320 + 64 * b]
            nc.tensor.matmul(xfv, fc0w2[:, b, :], xgF[:, :],
                             start=True, stop=(not has_fc0b))
            if has_fc0b:
                nc.tensor.matmul(xfv, r1[:, 0:128], r1[:, 128:192],
                                 start=False, stop=True)
        # batched A/B staging: one strided copy covers all 4 samples
        xf4 = ot0[:, 256:512].rearrange("p (b k) -> p k b", b=BPC)
        A3 = A.rearrange("p (k b g) -> p k b g", k=MODES, b=BPC)
        B3 = Bs.rearrange("p (k b g) -> p k b g", k=MODES, b=BPC)
        nc.vector.tensor_copy(A3[:, :, :, 0], xf4[:, 0:32, :])
        nc.vector.tensor_copy(A3[:, :, :, 1], xf4[:, 32:64, :])
        nc.vector.tensor_copy(B3[:, :, :, 1], xf4[:, 0:32, :])
        nc.vector.tensor_scalar_mul(B3[:, :, :, 0], xf4[:, 32:64, :], -1.0)
        # sample 0's chain completes first so phase 3 can start immediately
        of30 = ot0[:, 0:256].rearrange("p (k g) -> p k g", g=8)
        otp4 = scratch[0:64, 0:256].bitcast(bf16).rearrange(
            "p (b c) -> p b c", b=BPC)
        sc0 = float(2.0 ** (OF_EXP[0] - SW_EXP))
        for bset in ((0,), (1, 2, 3)):
            for b in bset:
                for k in range(MODES):
                    c = 8 * k + 2 * b
                    nc.tensor.matmul(ot0[:, c:c + 2], sw0[:, k, 0, :],
                                     A[:, c:c + 2], start=True, stop=False)
                    nc.tensor.matmul(ot0[:, c:c + 2], sw0[:, k, 1, :],
                                     Bs[:, c:c + 2], start=False, stop=True)
            for b in bset:
                nc.vector.tensor_copy(ofn[:, 64 * b:64 * (b + 1)],
                                      of30[:, :, 2 * b:2 * b + 2])
            for b in bset:
                nc.tensor.transpose(otp4[:, b, :], ofn[:, 64 * b:64 * (b + 1)],
                                    ident)
            for b in bset:
                nc.vector.tensor_scalar_mul(ofT8[:, b, 0, :], otp4[:, b, :], sc0)
                nc.vector.tensor_scalar_mul(ofT8[:, b, 1, :], otp4[:, b, :], sc0)

        # ---- layers ----
        for l in range(4):
            # phase 3 (+ pipelined transposes, next layer's DFT + mixing)
            def z_window(b, w):
                zt = pz.tile([128, 1024], f32, tag="z", name="zt")
                for q in range(2):
                    sl = slice(w * 1024 + q * 512, w * 1024 + (q + 1) * 512)
                    if l == 0:
                        nc.tensor.matmul(zt[:, q * 512:(q + 1) * 512],
                                         M0T[:, b, :], xg[:, sl],
                                         start=True, stop=False)
                    else:
                        nc.tensor.matmul(zt[:, q * 512:(q + 1) * 512],
                                         WT[:, l, :], hs[b][:, sl],
                                         start=True, stop=False)
                    nc.tensor.matmul(zt[:, q * 512:(q + 1) * 512],
                                     ofT8[:, b, :, :], Ci8[:, :, sl],
                                     start=False, stop=True, perf_mode=DR)
                return zt

            if l < 3:
                for b in range(BPC):
                    for w in range(8):
                        zt = z_window(b, w)
                        nc.scalar.activation(hs[b][:, w * 1024:(w + 1) * 1024],
                                             zt, GELU, bias=lb[:, l:l + 1],
                                             scale=float(2.0 ** -E_L[l]))
                        if w % 2 == 1:
                            issue_t(l + 1, b, w // 2)
                        gw[0] += 1
                        pump_dft()
            else:
                # two-stream software-pipelined final block:
                # z(b,w)+z(b',w) | fc1(.,w-1) halves | fc2(.,w-2)
                ot3 = of_tile(3)
                f2p = {0: ot3[:, 320:384], 1: ot3[:, 384:448],
                       2: ot3[:, 448:512], 3: ot3[:, 256:320]}
                for pb in (0, 2):
                    pair = (pb, pb + 1)
                    ocs, g1h = {}, {}
                    for step in range(10):
                        gw[0] += 1
                        if pb > 0 or step >= 2:
                            pump_dft()
                        if step < 8:
                            for bb in pair:
                                zt = z_window(bb, step)
                                oc = outp.tile([128, 1024], bf16, tag="oc",
                                               bufs=4)
                                nc.vector.tensor_scalar_mul(
                                    oc, zt, float(2.0 ** -E_L[3]))
                                ocs[(bb, step)] = oc
                        if 1 <= step <= 8:
                            w = step - 1
                            for bb in pair:
                                ocp = ocs.pop((bb, w))
                                fps = pz.tile([128, 1024], f32, tag="z",
                                              name="fps")
                                for q in range(2):
                                    nc.tensor.matmul(
                                        fps[:, q * 512:(q + 1) * 512], fc1w,
                                        ocp[:, q * 512:(q + 1) * 512],
                                        start=True, stop=True)
                                g1t = outp.tile([128, 1024], bf16, tag="g1",
                                                bufs=4)
                                nc.scalar.activation(g1t, fps, GELU,
                                                     bias=fc1b)
                                g1h[(bb, w)] = g1t
                        if 2 <= step <= 9:
                            w = step - 2
                            for bb in pair:
                                g1t = g1h.pop((bb, w))
                                for q in range(8):
                                    tg = w * 8 + q
                                    nc.tensor.matmul(
                                        f2p[bb][:, tg:tg + 1],
                                        g1t[:, q * 128:(q + 1) * 128],
                                        fc2w, start=True, stop=True)
                                if step == 9:
                                    f2sb = outp.tile([128, 64], f32,
                                                     tag="f2sb", bufs=2)
                                    nc.vector.tensor_copy(f2sb, f2p[bb])
                                    sy.dma_start(d_out[bb, :, :], f2sb)

    nc.compile()
    return nc


def _prep_v2(inputs, has_fc0b):
    inp = {k: np.asarray(v) for k, v in inputs.items()}
    F, Ci = _fourier_bases()
    F_sb = F.reshape(NCH, 128, 64).transpose(1, 0, 2).astype(BF)
    Ci8_1 = np.clip(Ci * (2.0 ** CI_EXP), -240, 240).astype(F8)  # [64, S]
    Ci8 = np.ascontiguousarray(np.repeat(Ci8_1[:, None, :], 2, axis=1))

    F_sb = np.clip(F_sb.astype(np.float64), -240, 240).astype(F8)
    fc0_w = inp["fc0_w"].astype(np.float64)     # [2, 128]
    fc0_b = inp["fc0_b"].astype(np.float64)
    Ws = [inp[f"w{i}_w"].astype(np.float64) for i in range(4)]
    WT = np.stack([Ws[i].T * (2.0 ** E_L[i]) for i in range(4)], 1)  # [128,4,128]
    M0 = (Ws[0] @ fc0_w.T).T * (2.0 ** E_L[0])   # [2, 128]
    M0T = np.zeros((8, 4, 128), np.float64)
    fc0blk = np.zeros((8, 4, 128), np.float64)
    for b in range(BPC):
        M0T[2 * b, b, :] = M0[0]
        M0T[2 * b + 1, b, :] = M0[1]
        fc0blk[2 * b, b, :] = fc0_w[0]
        fc0blk[2 * b + 1, b, :] = fc0_w[1]
    SW = np.empty((4, 128, MODES, 2, 128), np.float64)
    for i in range(4):
        sw = np.asarray(inp[f"sw{i}"])
        SW[i, :, :, 0, :] = np.ascontiguousarray(sw.real).transpose(0, 2, 1)
        SW[i, :, :, 1, :] = np.ascontiguousarray(sw.imag).transpose(0, 2, 1)
    SW = np.clip(SW * (2.0 ** SW_EXP), -240, 240)
    lb = np.stack([inp[f"w{i}_b"].astype(np.float64) for i in range(3)], 1)
    lb[:, 0] += Ws[0] @ fc0_b
    fc1_w = inp["fc1_w"].astype(np.float64)
    fc1b = (inp["fc1_b"].astype(np.float64)
            + fc1_w.T @ (inp["w3_b"].astype(np.float64)
                         + inp["cg2_b"].astype(np.float64)))
    grid = np.linspace(0.0, 1.0, S, dtype=np.float64)
    wb = np.zeros((128, 641), np.float64)
    wb[:, 0:512] = WT.reshape(128, 512)
    wb[:, 512:640] = fc1_w
    wb[:, 640] = inp["fc2_w"].astype(np.float64).reshape(-1)
    m8 = np.stack([M0T, fc0blk], 1)  # [8, 2, 4, 128]
    bias = np.concatenate([lb, fc1b.reshape(128, 1)], 1)
    common = {
        "Fb": F_sb, "Ci8": Ci8,
        "wb": wb.astype(BF), "m8": m8.astype(BF),
        "SW": SW.astype(F8),
        "bias": bias.astype(np.float32),
    }
    if has_fc0b:
        r1 = np.zeros((1, 192), np.float64)
        r1[0, 0:128] = fc0_b
        r1[0, 128:192] = F.sum(axis=0)
        common["r1"] = r1.astype(BF)
    x = inp["x"].astype(np.float64)  # [32, 8192, 1]
    per_core = []
    for c in range(NCORES):
        xg = np.empty((2 * BPC, S), np.float64)
        for b in range(BPC):
            xg[2 * b] = x[c * BPC + b, :, 0]
            xg[2 * b + 1] = grid
        m = dict(common)
        m["xg"] = xg.astype(BF)
        m["xgT"] = np.ascontiguousarray(
            xg.T.reshape(NCH, 128, 2 * BPC).transpose(1, 0, 2)).astype(BF)
        per_core.append(m)
    fc2b = float(inp["fc2_b"].astype(np.float64).reshape(-1)[0])
    return per_core, fc2b


# ---------------------------------------------------------------------------
# general fallback (original kernel; handles nonzero cg2_w)
# ---------------------------------------------------------------------------

def _host_consts_base():
    F, Ci = _fourier_bases()
    s = np.arange(S, dtype=np.float64)
    T = _cheb_basis(S, M_CHEB).astype(np.float64)
    kk = np.arange(-CFT_MODES, CFT_MODES + 1, dtype=np.float64)
    ph = np.pi * np.outer(s, kk) / S
    CH = np.empty((S, M_CHEB, 2 * CFT_MODES + 1, 2), np.float64)
    CH[..., 0] = T.T[:, :, None] * np.cos(ph)[:, None, :]
    CH[..., 1] = T.T[:, :, None] * (-np.sin(ph))[:, None, :]
    CH = (CH / S).reshape(S, 72)
    F_sb = F.reshape(NCH, 128, 64).transpose(1, 0, 2).astype(BF)
    CH_sb = CH.reshape(NCH, 128, 72).transpose(1, 0, 2).astype(BF)
    grid = np.linspace(0.0, 1.0, S, dtype=np.float32)
    return F_sb, CH_sb, Ci.astype(BF), grid


def _build_base():
    import concourse.bacc as bacc
    import concourse.tile as tile
    import concourse.mybir as mybir
    from concourse.masks import make_identity

    f32 = mybir.dt.float32
    bf16 = mybir.dt.bfloat16
    GELU = mybir.ActivationFunctionType.Gelu
    IDENT = mybir.ActivationFunctionType.Identity

    nc = bacc.Bacc("TRN2", target_bir_lowering=False)

    d_xg = nc.dram_tensor("xg", [2 * BPC, S], bf16, kind="ExternalInput")
    d_fc0w = nc.dram_tensor("fc0w", [8, 4, 128], bf16, kind="ExternalInput")
    d_F = nc.dram_tensor("Fb", [128, NCH, 64], bf16, kind="ExternalInput")
    d_CH = nc.dram_tensor("CHb", [128, NCH, 72], bf16, kind="ExternalInput")
    d_Ci = nc.dram_tensor("Cinv", [64, S], bf16, kind="ExternalInput")
    d_WT = nc.dram_tensor("WT", [128, 4, 128], bf16, kind="ExternalInput")
    d_SW = nc.dram_tensor("SW", [4, 128, MODES, 2, 128], bf16, kind="ExternalInput")
    d_G = nc.dram_tensor("G2", [128, 72, 256], bf16, kind="ExternalInput")
    d_fc1w = nc.dram_tensor("fc1w", [128, 128], bf16, kind="ExternalInput")
    d_fc2w = nc.dram_tensor("fc2w", [128, 1], bf16, kind="ExternalInput")
    d_cg2h = nc.dram_tensor("cg2h", [128, 2, 128], bf16, kind="ExternalInput")
    d_fc0b = nc.dram_tensor("fc0b", [128, 1], f32, kind="ExternalInput")
    d_lb = nc.dram_tensor("lb", [128, 3], f32, kind="ExternalInput")
    d_w3b = nc.dram_tensor("w3b", [128, 1], f32, kind="ExternalInput")
    d_fc1b = nc.dram_tensor("fc1b", [128, 1], f32, kind="ExternalInput")
    d_cg1b = nc.dram_tensor("cg1b", [4, 256], f32, kind="ExternalInput")
    d_out = nc.dram_tensor("out", [BPC, S], f32, kind="ExternalOutput")

    with ExitStack() as ctx:
        tc = ctx.enter_context(tile.TileContext(nc))
        consts = ctx.enter_context(tc.tile_pool(name="consts", bufs=1))
        hpool = ctx.enter_context(tc.tile_pool(name="h", bufs=1))
        htp = ctx.enter_context(tc.tile_pool(name="ht", bufs=3))
        swp = ctx.enter_context(tc.tile_pool(name="sw", bufs=2))
        gp = ctx.enter_context(tc.tile_pool(name="g", bufs=2))
        outp = ctx.enter_context(tc.tile_pool(name="outc", bufs=3))
        stg = ctx.enter_context(tc.tile_pool(name="stg", bufs=1))
        pz = ctx.enter_context(tc.tile_pool(name="pz", bufs=2, space="PSUM"))
        pxf = ctx.enter_context(tc.tile_pool(name="pxf", bufs=2, space="PSUM"))
        pof = ctx.enter_context(tc.tile_pool(name="pof", bufs=1, space="PSUM"))
        psm = ctx.enter_context(tc.tile_pool(name="psm", bufs=1, space="PSUM"))

        sy, gs = nc.sync, nc.gpsimd

        xg = consts.tile([2 * BPC, S], bf16); sy.dma_start(xg, d_xg[:, :])
        fc0w = consts.tile([8, 4, 128], bf16); sy.dma_start(fc0w, d_fc0w[:, :, :])
        Fb = consts.tile([128, NCH, 64], bf16); sy.dma_start(Fb, d_F[:, :, :])
        CHb = consts.tile([128, NCH, 72], bf16); sy.dma_start(CHb, d_CH[:, :, :])
        Ci = consts.tile([64, S], bf16); sy.dma_start(Ci, d_Ci[:, :])
        WT = consts.tile([128, 4, 128], bf16); sy.dma_start(WT, d_WT[:, :, :])
        fc1w = consts.tile([128, 128], bf16); sy.dma_start(fc1w, d_fc1w[:, :])
        fc2w = consts.tile([128, 1], bf16); sy.dma_start(fc2w, d_fc2w[:, :])
        cg2h = consts.tile([128, 2, 128], bf16); sy.dma_start(cg2h, d_cg2h[:, :, :])
        fc0b = consts.tile([128, 1], f32); sy.dma_start(fc0b, d_fc0b[:, :])
        lb = consts.tile([128, 3], f32); sy.dma_start(lb, d_lb[:, :])
        w3b = consts.tile([128, 1], f32); sy.dma_start(w3b, d_w3b[:, :])
        fc1b = consts.tile([128, 1], f32); sy.dma_start(fc1b, d_fc1b[:, :])
        cg1b = consts.tile([4, 256], f32); sy.dma_start(cg1b, d_cg1b[:, :])
        ident = consts.tile([128, 128], bf16); make_identity(nc, ident)

        hs = [hpool.tile([128, S], bf16, tag=f"h{b}", name=f"h{b}")
              for b in range(BPC)]
        A = consts.tile([128, 256], bf16)
        Bs = consts.tile([128, 256], bf16)
        feats = consts.tile([128, 288], bf16)
        ofn = consts.tile([128, 256], bf16)
        ofTs = [consts.tile([64, 128], bf16, tag=f"ofT{b}", name=f"ofT{b}")
                for b in range(BPC)]
        latb = consts.tile([128, BPC], f32)

        for b in range(BPC):
            for w in range(8):
                zt = pz.tile([128, 1024], f32, tag="z")
                for q in range(2):
                    nc.tensor.matmul(
                        zt[:, q * 512:(q + 1) * 512], fc0w[:, b, :],
                        xg[:, w * 1024 + q * 512:w * 1024 + (q + 1) * 512],
                        start=True, stop=True)
                if w % 2 == 0:
                    nc.scalar.activation(hs[b][:, w * 1024:(w + 1) * 1024], zt,
                                         IDENT, bias=fc0b[:, 0:1])
                else:
                    nc.vector.tensor_scalar_add(
                        hs[b][:, w * 1024:(w + 1) * 1024], zt, fc0b[:, 0:1])

        for l in range(4):
            sw = swp.tile([128, MODES, 2, 128], bf16, tag="sw")
            gs.dma_start(sw, d_SW[l, :, :, :, :])
            for b in range(BPC):
                xfp = pxf.tile([128, 136], f32, tag="xf")
                if l == 3:
                    cftp = psm.tile([128, 72], f32, tag="sm")
                for hh in range(2):
                    ht = htp.tile([128, 32, 128], bf16, tag="ht")
                    teng = sy if hh == 0 else nc.scalar
                    teng.dma_start(ht, hs[b][:, hh * 4096:(hh + 1) * 4096],
                                   transpose=True)
                    for t in range(32):
                        tg = hh * 32 + t
                        nc.tensor.matmul(xfp[:, 0:64], ht[:, t, :], Fb[:, tg, :],
                                         start=(tg == 0), stop=(tg == 63))
                        if l == 3:
                            nc.tensor.matmul(cftp, ht[:, t, :],
                                             CHb[:, tg, :],
                                             start=(tg == 0), stop=(tg == 63))
                nc.vector.tensor_copy(A[:, 2 * b:256:8], xfp[:, 0:32])
                nc.vector.tensor_copy(A[:, 2 * b + 1:256:8], xfp[:, 32:64])
                nc.vector.tensor_copy(Bs[:, 2 * b + 1:256:8], xfp[:, 0:32])
                nc.vector.tensor_scalar_mul(Bs[:, 2 * b:256:8], xfp[:, 32:64], -1.0)
                if l == 3:
                    nc.vector.tensor_copy(feats[:, b:288:4], cftp)

            ofp = pof.tile([128, 256], f32, tag="of")
            for k in range(MODES):
                nc.tensor.matmul(ofp[:, 8 * k:8 * k + 8], sw[:, k, 0, :],
                                 A[:, 8 * k:8 * k + 8], start=True, stop=False)
                nc.tensor.matmul(ofp[:, 8 * k:8 * k + 8], sw[:, k, 1, :],
                                 Bs[:, 8 * k:8 * k + 8], start=False, stop=True)
            ofp3 = ofp.rearrange("p (k g) -> p k g", g=8)
            for b in range(BPC):
                nc.vector.tensor_copy(ofn[:, 64 * b:64 * (b + 1)],
                                      ofp3[:, :, 2 * b:2 * b + 2])
                otp = psm.tile([64, 128], bf16, tag="sm")
                nc.tensor.transpose(otp, ofn[:, 64 * b:64 * (b + 1)], ident)
                nc.vector.tensor_copy(ofTs[b], otp)

            if l == 3:
                tps = pxf.tile([4, 256], f32, tag="xf")
                for qc in range(9):
                    gt = gp.tile([128, 8, 256], bf16, tag="G")
                    gs.dma_start(gt, d_G[:, qc * 8:(qc + 1) * 8, :])
                    for qq in range(8):
                        q = qc * 8 + qq
                        nc.tensor.matmul(tps, feats[:, 4 * q:4 * q + 4],
                                         gt[:, qq, :],
                                         start=(q == 0), stop=(q == 71))
                tsb = stg.tile([4, 256], f32)
                nc.vector.tensor_add(tsb, tps, cg1b)
                tgb = stg.tile([4, 256], bf16)
                nc.scalar.activation(tgb, tsb, GELU)
                lps = pof.tile([128, BPC], f32, tag="of")
                for hh in range(2):
                    ttp = psm.tile([128, 4], bf16, tag="sm")
                    nc.tensor.transpose(ttp, tgb[:, hh * 128:(hh + 1) * 128],
                                        ident[0:4, 0:4])
                    tgT = stg.tile([128, 4], bf16, tag=f"tgT{hh}")
                    nc.vector.tensor_copy(tgT, ttp)
                    nc.tensor.matmul(lps, cg2h[:, hh, :], tgT,
                                     start=(hh == 0), stop=(hh == 1))
                nc.vector.tensor_scalar_add(latb, lps, w3b[:, 0:1])

            for b in range(BPC):
                if l == 3:
                    f2ps = psm.tile([128, 64], f32, tag="sm")
                for w in range(8):
                    zt = pz.tile([128, 1024], f32, tag="z")
                    for q in range(2):
                        sl = slice(w * 1024 + q * 512, w * 1024 + (q + 1) * 512)
                        nc.tensor.matmul(zt[:, q * 512:(q + 1) * 512],
                                         ofTs[b], Ci[:, sl], start=True, stop=False)
                        nc.tensor.matmul(zt[:, q * 512:(q + 1) * 512],
                                         WT[:, l, :], hs[b][:, sl],
                                         start=False, stop=True)
                    if l < 3:
                        nc.scalar.activation(hs[b][:, w * 1024:(w + 1) * 1024], zt,
                                             GELU, bias=lb[:, l:l + 1])
                    else:
                        oc = outp.tile([128, 1024], bf16, tag="oc", bufs=4)
                        nc.vector.tensor_scalar_add(oc, zt, latb[:, b:b + 1])
                        fps = pz.tile([128, 1024], f32, tag="z")
                        for q in range(2):
                            nc.tensor.matmul(fps[:, q * 512:(q + 1) * 512], fc1w,
                                             oc[:, q * 512:(q + 1) * 512],
                                             start=True, stop=True)
                        g1 = outp.tile([128, 1024], bf16, tag="g1")
                        nc.scalar.activation(g1, fps, GELU, bias=fc1b)
                        for q in range(8):
                            tg = w * 8 + q
                            nc.tensor.matmul(f2ps[:, tg:tg + 1],
                                             g1[:, q * 128:(q + 1) * 128], fc2w,
                                             start=True, stop=True)
                if l == 3:
                    f2sb = outp.tile([128, 64], f32, tag="f2sb", bufs=2)
                    nc.vector.tensor_copy(f2sb, f2ps)
                    sy.dma_start(d_out[b, :].rearrange("(t p) -> p t", p=128), f2sb)

    nc.compile()
    return nc


def _fc0_blk(fc0_w):
    blk = np.zeros((8, 4, 128), np.float32)
    for b in range(BPC):
        blk[2 * b, b, :] = fc0_w[0]
        blk[2 * b + 1, b, :] = fc0_w[1]
    return blk.astype(BF)


def _prep_base(inputs):
    inp = {k: np.asarray(v) for k, v in inputs.items()}
    F_sb, CH_sb, Ci, grid = _host_consts_base()
    x = inp["x"].astype(np.float32)
    fc0_w = inp["fc0_w"].astype(np.float32)
    WT = np.stack([inp[f"w{i}_w"].astype(np.float32).T for i in range(4)], 1)
    SW = np.empty((4, 128, MODES, 2, 128), np.float32)
    for i in range(4):
        sw = np.asarray(inp[f"sw{i}"])
        SW[i, :, :, 0, :] = np.ascontiguousarray(sw.real).transpose(0, 2, 1)
        SW[i, :, :, 1, :] = np.ascontiguousarray(sw.imag).transpose(0, 2, 1)
    cg1 = inp["cg1_w"].astype(np.float32).reshape(WIDTH, M_CHEB, L_SEG, 9, 2, 256)
    G2 = cg1.sum(axis=2).reshape(WIDTH, 72, 256)
    lb = np.stack([inp[f"w{i}_b"].astype(np.float32) for i in range(3)], 1)
    common = {
        "fc0w": _fc0_blk(fc0_w),
        "Fb": F_sb, "CHb": CH_sb, "Cinv": Ci,
        "WT": WT.astype(BF),
        "SW": SW.astype(BF),
        "G2": G2.astype(BF),
        "fc1w": inp["fc1_w"].astype(np.float32).astype(BF),
        "fc2w": inp["fc2_w"].astype(np.float32).astype(BF),
        "cg2h": inp["cg2_w"].astype(np.float32).reshape(2, 128, 128)
                .transpose(1, 0, 2).copy().astype(BF),
        "fc0b": inp["fc0_b"].astype(np.float32).reshape(128, 1),
        "lb": lb,
        "w3b": (inp["w3_b"].astype(np.float32)
                + inp["cg2_b"].astype(np.float32)).reshape(128, 1),
        "fc1b": inp["fc1_b"].astype(np.float32).reshape(128, 1),
        "cg1b": np.broadcast_to(inp["cg1_b"].astype(np.float32), (4, 256)).copy(),
    }
    per_core = []
    for c in range(NCORES):
        xg = np.empty((2 * BPC, S), np.float32)
        for b in range(BPC):
            xg[2 * b] = x[c * BPC + b, :, 0]
            xg[2 * b + 1] = grid
        m = dict(common)
        m["xg"] = xg.astype(BF)
        per_core.append(m)
    fc2b = float(inp["fc2_b"].astype(np.float32).reshape(-1)[0])
    return per_core, fc2b


# ---------------------------------------------------------------------------

def kernel(**inputs) -> np.ndarray:
    from concourse import bass_utils
    fast = (np.count_nonzero(np.asarray(inputs["cg2_w"])) == 0)
    if fast:
        has_fc0b = bool(np.count_nonzero(np.asarray(inputs["fc0_b"])))
        key = ("v2", has_fc0b)
        per_core, fc2b = _prep_v2(inputs, has_fc0b)
        if key not in _CACHE:
            _CACHE[key] = _build_v2(has_fc0b)
    else:
        key = ("base",)
        per_core, fc2b = _prep_base(inputs)
        if key not in _CACHE:
            _CACHE[key] = _build_base()
    nc = _CACHE[key]
    res = bass_utils.run_bass_kernel_spmd(nc, per_core, core_ids=list(range(NCORES)))
    out = np.empty((B, S, 1), np.float32)
    for c in range(NCORES):
        r = res.results[c]["out"]
        if fast:  # [BPC, 128, 64]: s = t*128 + p
            r = r.transpose(0, 2, 1).reshape(BPC, S)
        out[c * BPC:(c + 1) * BPC, :, 0] = r
    return out + fc2b


# revision 95
# speedup vs baseline: 2.9756x; 1.0020x over previous
"""Trainium2 Bass kernel for nn_FNO_RC_1D (1D FNO + Chebyshev-Fourier residual
correction). Data-parallel over batch: 32 samples -> 8 cores x 4 samples.

Fast path (used when cg2_w == 0, the problem's zero-init correction head, so
latent == cg2_b and the whole CFT path folds into the fc1 bias):
  - layer 0 is folded through fc0: xf0 = fc0_w.T @ (Xg @ F) via a host-side
    transposed copy of [x; grid], and the 1x1 conv term is M0 @ Xg with
    M0 = W0 @ fc0_w.T -- h0 is never materialized in either orientation.
  - spectral conv per layer: forward DFT of 32 modes as chunked matmuls
    against hT (DMA xbar transpose of h), per-mode complex mixing as small
    matmuls, and the inverse DFT *fused into the 1x1 conv PSUM pass* as a
    single fp8 DoubleRow matmul (of/Ci quantized to scaled e4m3; the W h
    term rides the same accumulation in bf16 with power-of-2-scaled weights,
    unscaled exactly by the activation's scale argument).
  - gelu drains on ScalarE write h in place; transposes for the next layer
    issue mid-drain so the xbar DMAs overlap the remaining windows; the
    next layer's forward DFT is software-pipelined one sample behind.
  - final block: z -> oc on VectorE, fc1+gelu, fc2 as per-chunk columns.
All big matmuls bf16 (fp8 only where quantization error is provably small),
accumulation fp32 in PSUM.

A general fallback (original baseline kernel) handles nonzero cg2_w.
"""

from contextlib import ExitStack

import numpy as np
import ml_dtypes

B, S, WIDTH, MODES = 32, 8192, 128, 32
CFT_MODES, L_SEG, M_CHEB = 4, 2, 4
CFT_DIM = (2 * CFT_MODES + 1) * L_SEG * M_CHEB * WIDTH * 2
NCORES = 8
BPC = B // NCORES  # samples per core
NCH = S // 128     # 64 chunks
BF = ml_dtypes.bfloat16
F8 = ml_dtypes.float8_e4m3   # TRN float8e4: max finite 240, 256 -> inf

OF_EXP = [7, 10, 14, 17]     # per-layer scale exponent for of -> fp8
CI_EXP = 18                  # scale exponent for Cinv -> fp8
SW_EXP = 21                  # scale exponent for spectral weights -> fp8
# +1: the DoubleRow irfft sums two identical slot products (2x the result)
E_L = [o + CI_EXP + 1 for o in OF_EXP]

_CACHE = {}


def _cheb_basis(n, m):
    t = np.linspace(-1.0, 1.0, n)
    Ts = [np.ones(n), t]
    for _ in range(2, m):
        Ts.append(2.0 * t * Ts[-1] - Ts[-2])
    return np.stack(Ts[:m], 0).astype(np.float32)


def _fourier_bases():
    s = np.arange(S, dtype=np.float64)
    k = np.arange(MODES, dtype=np.float64)
    ang = 2.0 * np.pi * np.outer(s, k) / S
    F = np.concatenate([np.cos(ang), -np.sin(ang)], axis=1)  # [S, 64]
    ck = np.full(MODES, 2.0 / S); ck[0] = 1.0 / S
    Ci = np.empty((2 * MODES, S), np.float64)                # interleaved re/im
    Ci[0::2] = ck[:, None] * np.cos(ang.T)
    Ci[1::2] = -ck[:, None] * np.sin(ang.T)
    return F, Ci


# ---------------------------------------------------------------------------
# fast path (cg2_w == 0)
# ---------------------------------------------------------------------------

def _build_v2(has_fc0b):
    import concourse.bacc as bacc
    import concourse.tile as tile
    import concourse.mybir as mybir
    from concourse.masks import make_identity

    f32 = mybir.dt.float32
    bf16 = mybir.dt.bfloat16
    fp8 = mybir.dt.float8e4
    GELU = mybir.ActivationFunctionType.Gelu
    COPY = mybir.ActivationFunctionType.Copy
    DR = mybir.MatmulPerfMode.DoubleRow

    nc = bacc.Bacc("TRN2", target_bir_lowering=False)

    # ---- DRAM tensors ----
    d_xg = nc.dram_tensor("xg", [2 * BPC, S], bf16, kind="ExternalInput")
    d_xgT = nc.dram_tensor("xgT", [128, NCH, 2 * BPC], bf16, kind="ExternalInput")
    d_F = nc.dram_tensor("Fb", [128, NCH, 64], fp8, kind="ExternalInput")
    d_Ci8 = nc.dram_tensor("Ci8", [64, 2, S], fp8, kind="ExternalInput")
    d_wb = nc.dram_tensor("wb", [128, 641], bf16, kind="ExternalInput")
    d_m8 = nc.dram_tensor("m8", [8, 2, 4, 128], bf16, kind="ExternalInput")
    d_SW = nc.dram_tensor("SW", [4, 128, MODES, 2, 128], fp8, kind="ExternalInput")
    d_bias = nc.dram_tensor("bias", [128, 4], f32, kind="ExternalInput")
    if has_fc0b:
        d_r1 = nc.dram_tensor("r1", [1, 192], bf16, kind="ExternalInput")
    d_out = nc.dram_tensor("out", [BPC, 128, 64], f32, kind="ExternalOutput")

    with ExitStack() as ctx:
        tc = ctx.enter_context(tile.TileContext(nc))
        consts = ctx.enter_context(tc.tile_pool(name="consts", bufs=1))
        hpool = ctx.enter_context(tc.tile_pool(name="h", bufs=1))
        htp = ctx.enter_context(tc.tile_pool(name="ht", bufs=8))
        swp = ctx.enter_context(tc.tile_pool(name="sw", bufs=4))
        outp = ctx.enter_context(tc.tile_pool(name="outc", bufs=8))
        stg = ctx.enter_context(tc.tile_pool(name="stg", bufs=1))
        pz = ctx.enter_context(tc.tile_pool(name="pz", bufs=3, space="PSUM"))
        pf1 = ctx.enter_context(tc.tile_pool(name="pf1", bufs=1, space="PSUM"))
        pof = ctx.enter_context(tc.tile_pool(name="pof", bufs=1, space="PSUM"))

        sy, gs = nc.sync, nc.gpsimd

        # ---- constants into SBUF ----
        # order matters for the startup critical path: phase1-l0 needs
        # xgT+Fb, phase2-l0 needs SW (gs queue), phase3-l0 needs Ci8+xg.
        xgT = consts.tile([128, NCH, 2 * BPC], bf16); sy.dma_start(xgT, d_xgT[:, :, :])
        Fb = consts.tile([128, NCH, 64], fp8); nc.scalar.dma_start(Fb, d_F[:, :, :])
        bias = consts.tile([128, 4], f32); sy.dma_start(bias, d_bias[:, :])
        m8 = consts.tile([8, 2, 4, 128], bf16); sy.dma_start(m8, d_m8[:, :, :, :])
        wb = consts.tile([128, 641], bf16); sy.dma_start(wb, d_wb[:, :])
        xg = consts.tile([2 * BPC, S], bf16); sy.dma_start(xg, d_xg[:, :])
        Ci8 = consts.tile([64, 2, S], fp8)
        if has_fc0b:
            r1 = consts.tile([1, 192], bf16); sy.dma_start(r1, d_r1[:, :])
        ident = consts.tile([128, 128], bf16); make_identity(nc, ident)
        lb = bias[:, 0:3]
        fc1b = bias[:, 3:4]
        WT = wb[:, 0:512].rearrange("p (l c) -> p l c", l=4)
        fc1w = wb[:, 512:640]
        fc2w = wb[:, 640:641]
        M0T = m8[:, 0, :, :]
        fc0w2 = m8[:, 1, :, :]

        hs = [hpool.tile([128, S], bf16, tag=f"h{b}", name=f"h{b}")
              for b in range(BPC)]
        A = consts.tile([128, 256], bf16)      # staged (xr, xi) per (k, b)
        Bs = consts.tile([128, 256], bf16)     # staged (-xi, xr)
        ofn = consts.tile([128, 256], bf16)    # of natural staging per sample
        ofT8 = consts.tile([64, BPC, 2, 128], fp8)  # DoubleRow lhsT per sample

        sw_tiles = {}

        def prefetch_sw(l, eng=None):
            t = swp.tile([128, MODES, 2, 128], fp8, tag="sw", name=f"sw{l}")
            (eng or gs).dma_start(t, d_SW[l, :, :, :, :])
            sw_tiles[l] = t

        def stage_ab(b, xfp):
            nc.vector.tensor_copy(A[:, 2 * b:256:8], xfp[:, 0:32])
            nc.vector.tensor_copy(A[:, 2 * b + 1:256:8], xfp[:, 32:64])
            nc.vector.tensor_copy(Bs[:, 2 * b + 1:256:8], xfp[:, 0:32])
            nc.vector.tensor_scalar_mul(Bs[:, 2 * b:256:8], xfp[:, 32:64], -1.0)

        # of_tile(lp): one PSUM bank per layer epoch -- cols 0:256 hold the
        # mixed spectral outputs (k-major, per-sample col pairs), cols
        # 256:320 are the forward-DFT accumulator (reused per sample).
        of_tiles = {}

        def of_tile(lp):
            if lp not in of_tiles:
                of_tiles[lp] = pof.tile([128, 512], f32, tag="of",
                                        name=f"of{lp}")
            return of_tiles[lp]

        def mix_sample(lp, b):
            # sample b's full phase-2 chain: mixing -> ofn -> transpose -> fp8
            sw = sw_tiles[lp]
            ot = of_tiles[lp]
            for k in range(MODES):
                c = 8 * k + 2 * b
                nc.tensor.matmul(ot[:, c:c + 2], sw[:, k, 0, :],
                                 A[:, c:c + 2], start=True, stop=False)
                nc.tensor.matmul(ot[:, c:c + 2], sw[:, k, 1, :],
                                 Bs[:, c:c + 2], start=False, stop=True)
            of3 = ot[:, 0:256].rearrange("p (k g) -> p k g", g=8)
            nc.vector.tensor_copy(ofn[:, 64 * b:64 * (b + 1)],
                                  of3[:, :, 2 * b:2 * b + 2])
            otp_t = pf1.tile([128, 512], f32, tag="f1", name="otp_t")
            otp = otp_t[0:64, 0:64].bitcast(bf16)
            nc.tensor.transpose(otp, ofn[:, 64 * b:64 * (b + 1)], ident)
            sc = float(2.0 ** (OF_EXP[lp] - SW_EXP))
            nc.vector.tensor_scalar_mul(ofT8[:, b, 0, :], otp, sc)
            nc.vector.tensor_scalar_mul(ofT8[:, b, 1, :], otp, sc)

        ht_q = {}
        dft_pending = []   # [lp, b, q, issue_window]; >= 4-window consume lag
        gw = [0]           # global window counter

        def issue_t(lp, b, q):
            tq = htp.tile([128, 16, 128], bf16, tag="ht")
            sy.dma_start(tq, hs[b][:, q * 2048:(q + 1) * 2048], transpose=True)
            ht_q[(b, q)] = tq
            dft_pending.append((lp, b, q, 0, gw[0]))
            dft_pending.append((lp, b, q, 1, gw[0]))

        def dft_q(lp, b, q, h):
            xfv = of_tile(lp)[:, 256:320]
            tt = ht_q[(b, q)]
            for t in range(8):
                tg = q * 16 + h * 8 + t
                nc.tensor.matmul(xfv, tt[:, h * 8 + t, :], Fb[:, tg, :],
                                 start=(tg == 0), stop=(tg == 63))
            if h == 1:
                del ht_q[(b, q)]
                if q == 3:
                    stage_ab(b, xfv)
                    mix_sample(lp, b)

        def pump_dft():
            if dft_pending and gw[0] - dft_pending[0][4] >= 5:
                lp, bb, qq, hh, _ = dft_pending.pop(0)
                dft_q(lp, bb, qq, hh)

        prefetch_sw(0, eng=nc.scalar)
        nc.scalar.dma_start(Ci8, d_Ci8[:, :, :])
        prefetch_sw(1, eng=nc.scalar)
        # warm the Gelu table while constants stream in
        warm = stg.tile([128, 1], f32)
        nc.scalar.activation(warm, lb[:, 0:1], GELU)
        # keep PE busy while DMAs land so the pstate ramp stays hot
        scratch = pf1.tile([128, 512], f32, tag="f1", name="scratch")
        for i in range(12):
            nc.tensor.matmul(scratch[0:2 * BPC, 256:512], xgT[:, 0, :],
                             xgT[:, 0:32, :], start=True, stop=True)

        # ---- layer-0 phase 1: xf0 = fc0_w.T @ (Xg @ F) ----
        # batched per engine stage so cross-engine latencies amortize
        ot0 = of_tile(0)
        xgFp = ot0[0:2 * BPC, 448:512]
        for t in range(NCH):
            nc.tensor.matmul(xgFp, xgT[:, t, :], Fb[:, t, :],
                             start=(t == 0), stop=(t == NCH - 1))
        xgF = stg.tile([2 * BPC, 64], bf16)
        nc.vector.tensor_copy(xgF, xgFp)
        sw0 = sw_tiles[0]
        for b in range(BPC):
            xfv = ot0[:, 256 + 64 * b:320 + 64 * b]
            nc.tensor.matmul(xfv, fc0w2[:, b, :], xgF[:, :],
                             start=True, stop=(not has_fc0b))
            if has_fc0b:
                nc.tensor.matmul(xfv, r1[:, 0:128], r1[:, 128:192],
                                 start=False, stop=True)
        # batched A/B staging: one strided copy covers all 4 samples
        xf4 = ot0[:, 256:512].rearrange("p (b k) -> p k b", b=BPC)
        A3 = A.rearrange("p (k b g) -> p k b g", k=MODES, b=BPC)
        B3 = Bs.rearrange("p (k b g) -> p k b g", k=MODES, b=BPC)
        nc.vector.tensor_copy(A3[:, :, :, 0], xf4[:, 0:32, :])
        nc.vector.tensor_copy(A3[:, :, :, 1], xf4[:, 32:64, :])
        nc.vector.tensor_copy(B3[:, :, :, 1], xf4[:, 0:32, :])
        nc.vector.tensor_scalar_mul(B3[:, :, :, 0], xf4[:, 32:64, :], -1.0)
        # sample 0's chain completes first so phase 3 can start immediately
        of30 = ot0[:, 0:256].rearrange("p (k g) -> p k g", g=8)
        otp4 = scratch[0:64, 0:256].bitcast(bf16).rearrange(
            "p (b c) -> p b c", b=BPC)
        sc0 = float(2.0 ** (OF_EXP[0] - SW_EXP))
        for bset in ((0,), (1, 2, 3)):
            for b in bset:
                for k in range(MODES):
                    c = 8 * k + 2 * b
                    nc.tensor.matmul(ot0[:, c:c + 2], sw0[:, k, 0, :],
                                     A[:, c:c + 2], start=True, stop=False)
                    nc.tensor.matmul(ot0[:, c:c + 2], sw0[:, k, 1, :],
                                     Bs[:, c:c + 2], start=False, stop=True)
            for b in bset:
                nc.vector.tensor_copy(ofn[:, 64 * b:64 * (b + 1)],
                                      of30[:, :, 2 * b:2 * b + 2])
            for b in bset:
                nc.tensor.transpose(otp4[:, b, :], ofn[:, 64 * b:64 * (b + 1)],
                                    ident)
            for b in bset:
                nc.vector.tensor_scalar_mul(ofT8[:, b, 0, :], otp4[:, b, :], sc0)
                nc.vector.tensor_scalar_mul(ofT8[:, b, 1, :], otp4[:, b, :], sc0)

        # ---- layers ----
        for l in range(4):
            # phase 3 (+ pipelined transposes, next layer's DFT + mixing)
            def z_window(b, w):
                zt = pz.tile([128, 1024], f32, tag="z", name="zt")
                for q in range(2):
                    sl = slice(w * 1024 + q * 512, w * 1024 + (q + 1) * 512)
                    if l == 0:
                        nc.tensor.matmul(zt[:, q * 512:(q + 1) * 512],
                                         M0T[:, b, :], xg[:, sl],
                                         start=True, stop=False)
                    else:
                        nc.tensor.matmul(zt[:, q * 512:(q + 1) * 512],
                                         WT[:, l, :], hs[b][:, sl],
                                         start=True, stop=False)
                    nc.tensor.matmul(zt[:, q * 512:(q + 1) * 512],
                                     ofT8[:, b, :, :], Ci8[:, :, sl],
                                     start=False, stop=True, perf_mode=DR)
                return zt

            if l < 3:
                for b in range(BPC):
                    for w in range(8):
                        zt = z_window(b, w)
                        nc.scalar.activation(hs[b][:, w * 1024:(w + 1) * 1024],
                                             zt, GELU, bias=lb[:, l:l + 1],
                                             scale=float(2.0 ** -E_L[l]))
                        if w % 2 == 1:
                            issue_t(l + 1, b, w // 2)
                        if l < 2 and b == 2 and w == 0:
                            prefetch_sw(l + 2, eng=sy)
                        gw[0] += 1
                        pump_dft()
            else:
                # two-stream software-pipelined final block:
                # z(b,w)+z(b',w) | fc1(.,w-1) halves | fc2(.,w-2)
                ot3 = of_tile(3)
                f2p = {0: ot3[:, 320:384], 1: ot3[:, 384:448],
                       2: ot3[:, 448:512], 3: ot3[:, 256:320]}
                for pb in (0, 2):
                    pair = (pb, pb + 1)
                    ocs, g1h = {}, {}
                    for step in range(10):
                        gw[0] += 1
                        if pb > 0 or step >= 2:
                            pump_dft()
                        if step < 8:
                            for bb in pair:
                                zt = z_window(bb, step)
                                oc = outp.tile([128, 1024], bf16, tag="oc",
                                               bufs=4)
                                nc.vector.tensor_scalar_mul(
                                    oc, zt, float(2.0 ** -E_L[3]))
                                ocs[(bb, step)] = oc
                        if 1 <= step <= 8:
                            w = step - 1
                            for bb in pair:
                                ocp = ocs.pop((bb, w))
                                fps = pz.tile([128, 1024], f32, tag="z",
                                              name="fps")
                                for q in range(2):
                                    nc.tensor.matmul(
                                        fps[:, q * 512:(q + 1) * 512], fc1w,
                                        ocp[:, q * 512:(q + 1) * 512],
                                        start=True, stop=True)
                                g1t = outp.tile([128, 1024], bf16, tag="g1",
                                                bufs=4)
                                nc.scalar.activation(g1t, fps, GELU,
                                                     bias=fc1b)
                                g1h[(bb, w)] = g1t
                        if 2 <= step <= 9:
                            w = step - 2
                            for bb in pair:
                                g1t = g1h.pop((bb, w))
                                for q in range(8):
                                    tg = w * 8 + q
                                    nc.tensor.matmul(
                                        f2p[bb][:, tg:tg + 1],
                                        g1t[:, q * 128:(q + 1) * 128],
                                        fc2w, start=True, stop=True)
                                if step == 9:
                                    f2sb = outp.tile([128, 64], f32,
                                                     tag="f2sb", bufs=2)
                                    nc.vector.tensor_copy(f2sb, f2p[bb])
                                    sy.dma_start(d_out[bb, :, :], f2sb)

    nc.compile()
    return nc


def _prep_v2(inputs, has_fc0b):
    inp = {k: np.asarray(v) for k, v in inputs.items()}
    F, Ci = _fourier_bases()
    F_sb = F.reshape(NCH, 128, 64).transpose(1, 0, 2).astype(BF)
    Ci8_1 = np.clip(Ci * (2.0 ** CI_EXP), -240, 240).astype(F8)  # [64, S]
    Ci8 = np.ascontiguousarray(np.repeat(Ci8_1[:, None, :], 2, axis=1))

    F_sb = np.clip(F_sb.astype(np.float64), -240, 240).astype(F8)
    fc0_w = inp["fc0_w"].astype(np.float64)     # [2, 128]
    fc0_b = inp["fc0_b"].astype(np.float64)
    Ws = [inp[f"w{i}_w"].astype(np.float64) for i in range(4)]
    WT = np.stack([Ws[i].T * (2.0 ** E_L[i]) for i in range(4)], 1)  # [128,4,128]
    M0 = (Ws[0] @ fc0_w.T).T * (2.0 ** E_L[0])   # [2, 128]
    M0T = np.zeros((8, 4, 128), np.float64)
    fc0blk = np.zeros((8, 4, 128), np.float64)
    for b in range(BPC):
        M0T[2 * b, b, :] = M0[0]
        M0T[2 * b + 1, b, :] = M0[1]
        fc0blk[2 * b, b, :] = fc0_w[0]
        fc0blk[2 * b + 1, b, :] = fc0_w[1]
    SW = np.empty((4, 128, MODES, 2, 128), np.float64)
    for i in range(4):
        sw = np.asarray(inp[f"sw{i}"])
        SW[i, :, :, 0, :] = np.ascontiguousarray(sw.real).transpose(0, 2, 1)
        SW[i, :, :, 1, :] = np.ascontiguousarray(sw.imag).transpose(0, 2, 1)
    SW = np.clip(SW * (2.0 ** SW_EXP), -240, 240)
    lb = np.stack([inp[f"w{i}_b"].astype(np.float64) for i in range(3)], 1)
    lb[:, 0] += Ws[0] @ fc0_b
    fc1_w = inp["fc1_w"].astype(np.float64)
    fc1b = (inp["fc1_b"].astype(np.float64)
            + fc1_w.T @ (inp["w3_b"].astype(np.float64)
                         + inp["cg2_b"].astype(np.float64)))
    grid = np.linspace(0.0, 1.0, S, dtype=np.float64)
    wb = np.zeros((128, 641), np.float64)
    wb[:, 0:512] = WT.reshape(128, 512)
    wb[:, 512:640] = fc1_w
    wb[:, 640] = inp["fc2_w"].astype(np.float64).reshape(-1)
    m8 = np.stack([M0T, fc0blk], 1)  # [8, 2, 4, 128]
    bias = np.concatenate([lb, fc1b.reshape(128, 1)], 1)
    common = {
        "Fb": F_sb, "Ci8": Ci8,
        "wb": wb.astype(BF), "m8": m8.astype(BF),
        "SW": SW.astype(F8),
        "bias": bias.astype(np.float32),
    }
    if has_fc0b:
        r1 = np.zeros((1, 192), np.float64)
        r1[0, 0:128] = fc0_b
        r1[0, 128:192] = F.sum(axis=0)
        common["r1"] = r1.astype(BF)
    x = inp["x"].astype(np.float64)  # [32, 8192, 1]
    per_core = []
    for c in range(NCORES):
        xg = np.empty((2 * BPC, S), np.float64)
        for b in range(BPC):
            xg[2 * b] = x[c * BPC + b, :, 0]
            xg[2 * b + 1] = grid
        m = dict(common)
        m["xg"] = xg.astype(BF)
        m["xgT"] = np.ascontiguousarray(
            xg.T.reshape(NCH, 128, 2 * BPC).transpose(1, 0, 2)).astype(BF)
        per_core.append(m)
    fc2b = float(inp["fc2_b"].astype(np.float64).reshape(-1)[0])
    return per_core, fc2b


# ---------------------------------------------------------------------------
# general fallback (original kernel; handles nonzero cg2_w)
# ---------------------------------------------------------------------------

def _host_consts_base():
    F, Ci = _fourier_bases()
    s = np.arange(S, dtype=np.float64)
    T = _cheb_basis(S, M_CHEB).astype(np.float64)
    kk = np.arange(-CFT_MODES, CFT_MODES + 1, dtype=np.float64)
    ph = np.pi * np.outer(s, kk) / S
    CH = np.empty((S, M_CHEB, 2 * CFT_MODES + 1, 2), np.float64)
    CH[..., 0] = T.T[:, :, None] * np.cos(ph)[:, None, :]
    CH[..., 1] = T.T[:, :, None] * (-np.sin(ph))[:, None, :]
    CH = (CH / S).reshape(S, 72)
    F_sb = F.reshape(NCH, 128, 64).transpose(1, 0, 2).astype(BF)
    CH_sb = CH.reshape(NCH, 128, 72).transpose(1, 0, 2).astype(BF)
    grid = np.linspace(0.0, 1.0, S, dtype=np.float32)
    return F_sb, CH_sb, Ci.astype(BF), grid


def _build_base():
    import concourse.bacc as bacc
    import concourse.tile as tile
    import concourse.mybir as mybir
    from concourse.masks import make_identity

    f32 = mybir.dt.float32
    bf16 = mybir.dt.bfloat16
    GELU = mybir.ActivationFunctionType.Gelu
    IDENT = mybir.ActivationFunctionType.Identity

    nc = bacc.Bacc("TRN2", target_bir_lowering=False)

    d_xg = nc.dram_tensor("xg", [2 * BPC, S], bf16, kind="ExternalInput")
    d_fc0w = nc.dram_tensor("fc0w", [8, 4, 128], bf16, kind="ExternalInput")
    d_F = nc.dram_tensor("Fb", [128, NCH, 64], bf16, kind="ExternalInput")
    d_CH = nc.dram_tensor("CHb", [128, NCH, 72], bf16, kind="ExternalInput")
    d_Ci = nc.dram_tensor("Cinv", [64, S], bf16, kind="ExternalInput")
    d_WT = nc.dram_tensor("WT", [128, 4, 128], bf16, kind="ExternalInput")
    d_SW = nc.dram_tensor("SW", [4, 128, MODES, 2, 128], bf16, kind="ExternalInput")
    d_G = nc.dram_tensor("G2", [128, 72, 256], bf16, kind="ExternalInput")
    d_fc1w = nc.dram_tensor("fc1w", [128, 128], bf16, kind="ExternalInput")
    d_fc2w = nc.dram_tensor("fc2w", [128, 1], bf16, kind="ExternalInput")
    d_cg2h = nc.dram_tensor("cg2h", [128, 2, 128], bf16, kind="ExternalInput")
    d_fc0b = nc.dram_tensor("fc0b", [128, 1], f32, kind="ExternalInput")
    d_lb = nc.dram_tensor("lb", [128, 3], f32, kind="ExternalInput")
    d_w3b = nc.dram_tensor("w3b", [128, 1], f32, kind="ExternalInput")
    d_fc1b = nc.dram_tensor("fc1b", [128, 1], f32, kind="ExternalInput")
    d_cg1b = nc.dram_tensor("cg1b", [4, 256], f32, kind="ExternalInput")
    d_out = nc.dram_tensor("out", [BPC, S], f32, kind="ExternalOutput")

    with ExitStack() as ctx:
        tc = ctx.enter_context(tile.TileContext(nc))
        consts = ctx.enter_context(tc.tile_pool(name="consts", bufs=1))
        hpool = ctx.enter_context(tc.tile_pool(name="h", bufs=1))
        htp = ctx.enter_context(tc.tile_pool(name="ht", bufs=3))
        swp = ctx.enter_context(tc.tile_pool(name="sw", bufs=2))
        gp = ctx.enter_context(tc.tile_pool(name="g", bufs=2))
        outp = ctx.enter_context(tc.tile_pool(name="outc", bufs=3))
        stg = ctx.enter_context(tc.tile_pool(name="stg", bufs=1))
        pz = ctx.enter_context(tc.tile_pool(name="pz", bufs=2, space="PSUM"))
        pxf = ctx.enter_context(tc.tile_pool(name="pxf", bufs=2, space="PSUM"))
        pof = ctx.enter_context(tc.tile_pool(name="pof", bufs=1, space="PSUM"))
        psm = ctx.enter_context(tc.tile_pool(name="psm", bufs=1, space="PSUM"))

        sy, gs = nc.sync, nc.gpsimd

        xg = consts.tile([2 * BPC, S], bf16); sy.dma_start(xg, d_xg[:, :])
        fc0w = consts.tile([8, 4, 128], bf16); sy.dma_start(fc0w, d_fc0w[:, :, :])
        Fb = consts.tile([128, NCH, 64], bf16); sy.dma_start(Fb, d_F[:, :, :])
        CHb = consts.tile([128, NCH, 72], bf16); sy.dma_start(CHb, d_CH[:, :, :])
        Ci = consts.tile([64, S], bf16); sy.dma_start(Ci, d_Ci[:, :])
        WT = consts.tile([128, 4, 128], bf16); sy.dma_start(WT, d_WT[:, :, :])
        fc1w = consts.tile([128, 128], bf16); sy.dma_start(fc1w, d_fc1w[:, :])
        fc2w = consts.tile([128, 1], bf16); sy.dma_start(fc2w, d_fc2w[:, :])
        cg2h = consts.tile([128, 2, 128], bf16); sy.dma_start(cg2h, d_cg2h[:, :, :])
        fc0b = consts.tile([128, 1], f32); sy.dma_start(fc0b, d_fc0b[:, :])
        lb = consts.tile([128, 3], f32); sy.dma_start(lb, d_lb[:, :])
        w3b = consts.tile([128, 1], f32); sy.dma_start(w3b, d_w3b[:, :])
        fc1b = consts.tile([128, 1], f32); sy.dma_start(fc1b, d_fc1b[:, :])
        cg1b = consts.tile([4, 256], f32); sy.dma_start(cg1b, d_cg1b[:, :])
        ident = consts.tile([128, 128], bf16); make_identity(nc, ident)

        hs = [hpool.tile([128, S], bf16, tag=f"h{b}", name=f"h{b}")
              for b in range(BPC)]
        A = consts.tile([128, 256], bf16)
        Bs = consts.tile([128, 256], bf16)
        feats = consts.tile([128, 288], bf16)
        ofn = consts.tile([128, 256], bf16)
        ofTs = [consts.tile([64, 128], bf16, tag=f"ofT{b}", name=f"ofT{b}")
                for b in range(BPC)]
        latb = consts.tile([128, BPC], f32)

        for b in range(BPC):
            for w in range(8):
                zt = pz.tile([128, 1024], f32, tag="z")
                for q in range(2):
                    nc.tensor.matmul(
                        zt[:, q * 512:(q + 1) * 512], fc0w[:, b, :],
                        xg[:, w * 1024 + q * 512:w * 1024 + (q + 1) * 512],
                        start=True, stop=True)
                if w % 2 == 0:
                    nc.scalar.activation(hs[b][:, w * 1024:(w + 1) * 1024], zt,
                                         IDENT, bias=fc0b[:, 0:1])
                else:
                    nc.vector.tensor_scalar_add(
                        hs[b][:, w * 1024:(w + 1) * 1024], zt, fc0b[:, 0:1])

        for l in range(4):
            sw = swp.tile([128, MODES, 2, 128], bf16, tag="sw")
            gs.dma_start(sw, d_SW[l, :, :, :, :])
            for b in range(BPC):
                xfp = pxf.tile([128, 136], f32, tag="xf")
                if l == 3:
                    cftp = psm.tile([128, 72], f32, tag="sm")
                for hh in range(2):
                    ht = htp.tile([128, 32, 128], bf16, tag="ht")
                    teng = sy if hh == 0 else nc.scalar
                    teng.dma_start(ht, hs[b][:, hh * 4096:(hh + 1) * 4096],
                                   transpose=True)
                    for t in range(32):
                        tg = hh * 32 + t
                        nc.tensor.matmul(xfp[:, 0:64], ht[:, t, :], Fb[:, tg, :],
                                         start=(tg == 0), stop=(tg == 63))
                        if l == 3:
                            nc.tensor.matmul(cftp, ht[:, t, :],
                                             CHb[:, tg, :],
                                             start=(tg == 0), stop=(tg == 63))
                nc.vector.tensor_copy(A[:, 2 * b:256:8], xfp[:, 0:32])
                nc.vector.tensor_copy(A[:, 2 * b + 1:256:8], xfp[:, 32:64])
                nc.vector.tensor_copy(Bs[:, 2 * b + 1:256:8], xfp[:, 0:32])
                nc.vector.tensor_scalar_mul(Bs[:, 2 * b:256:8], xfp[:, 32:64], -1.0)
                if l == 3:
                    nc.vector.tensor_copy(feats[:, b:288:4], cftp)

            ofp = pof.tile([128, 256], f32, tag="of")
            for k in range(MODES):
                nc.tensor.matmul(ofp[:, 8 * k:8 * k + 8], sw[:, k, 0, :],
                                 A[:, 8 * k:8 * k + 8], start=True, stop=False)
                nc.tensor.matmul(ofp[:, 8 * k:8 * k + 8], sw[:, k, 1, :],
                                 Bs[:, 8 * k:8 * k + 8], start=False, stop=True)
            ofp3 = ofp.rearrange("p (k g) -> p k g", g=8)
            for b in range(BPC):
                nc.vector.tensor_copy(ofn[:, 64 * b:64 * (b + 1)],
                                      ofp3[:, :, 2 * b:2 * b + 2])
                otp = psm.tile([64, 128], bf16, tag="sm")
                nc.tensor.transpose(otp, ofn[:, 64 * b:64 * (b + 1)], ident)
                nc.vector.tensor_copy(ofTs[b], otp)

            if l == 3:
                tps = pxf.tile([4, 256], f32, tag="xf")
                for qc in range(9):
                    gt = gp.tile([128, 8, 256], bf16, tag="G")
                    gs.dma_start(gt, d_G[:, qc * 8:(qc + 1) * 8, :])
                    for qq in range(8):
                        q = qc * 8 + qq
                        nc.tensor.matmul(tps, feats[:, 4 * q:4 * q + 4],
                                         gt[:, qq, :],
                                         start=(q == 0), stop=(q == 71))
                tsb = stg.tile([4, 256], f32)
                nc.vector.tensor_add(tsb, tps, cg1b)
                tgb = stg.tile([4, 256], bf16)
                nc.scalar.activation(tgb, tsb, GELU)
                lps = pof.tile([128, BPC], f32, tag="of")
                for hh in range(2):
                    ttp = psm.tile([128, 4], bf16, tag="sm")
                    nc.tensor.transpose(ttp, tgb[:, hh * 128:(hh + 1) * 128],
                                        ident[0:4, 0:4])
                    tgT = stg.tile([128, 4], bf16, tag=f"tgT{hh}")
                    nc.vector.tensor_copy(tgT, ttp)
                    nc.tensor.matmul(lps, cg2h[:, hh, :], tgT,
                                     start=(hh == 0), stop=(hh == 1))
                nc.vector.tensor_scalar_add(latb, lps, w3b[:, 0:1])

            for b in range(BPC):
                if l == 3:
                    f2ps = psm.tile([128, 64], f32, tag="sm")
                for w in range(8):
                    zt = pz.tile([128, 1024], f32, tag="z")
                    for q in range(2):
                        sl = slice(w * 1024 + q * 512, w * 1024 + (q + 1) * 512)
                        nc.tensor.matmul(zt[:, q * 512:(q + 1) * 512],
                                         ofTs[b], Ci[:, sl], start=True, stop=False)
                        nc.tensor.matmul(zt[:, q * 512:(q + 1) * 512],
                                         WT[:, l, :], hs[b][:, sl],
                                         start=False, stop=True)
                    if l < 3:
                        nc.scalar.activation(hs[b][:, w * 1024:(w + 1) * 1024], zt,
                                             GELU, bias=lb[:, l:l + 1])
                    else:
                        oc = outp.tile([128, 1024], bf16, tag="oc", bufs=4)
                        nc.vector.tensor_scalar_add(oc, zt, latb[:, b:b + 1])
                        fps = pz.tile([128, 1024], f32, tag="z")
                        for q in range(2):
                            nc.tensor.matmul(fps[:, q * 512:(q + 1) * 512], fc1w,
                                             oc[:, q * 512:(q + 1) * 512],
                                             start=True, stop=True)
                        g1 = outp.tile([128, 1024], bf16, tag="g1")
                        nc.scalar.activation(g1, fps, GELU, bias=fc1b)
                        for q in range(8):
                            tg = w * 8 + q
                            nc.tensor.matmul(f2ps[:, tg:tg + 1],
                                             g1[:, q * 128:(q + 1) * 128], fc2w,
                                             start=True, stop=True)
                if l == 3:
                    f2sb = outp.tile([128, 64], f32, tag="f2sb", bufs=2)
                    nc.vector.tensor_copy(f2sb, f2ps)
                    sy.dma_start(d_out[b, :].rearrange("(t p) -> p t", p=128), f2sb)

    nc.compile()
    return nc


def _fc0_blk(fc0_w):
    blk = np.zeros((8, 4, 128), np.float32)
    for b in range(BPC):
        blk[2 * b, b, :] = fc0_w[0]
        blk[2 * b + 1, b, :] = fc0_w[1]
    return blk.astype(BF)


def _prep_base(inputs):
    inp = {k: np.asarray(v) for k, v in inputs.items()}
    F_sb, CH_sb, Ci, grid = _host_consts_base()
    x = inp["x"].astype(np.float32)
    fc0_w = inp["fc0_w"].astype(np.float32)
    WT = np.stack([inp[f"w{i}_w"].astype(np.float32).T for i in range(4)], 1)
    SW = np.empty((4, 128, MODES, 2, 128), np.float32)
    for i in range(4):
        sw = np.asarray(inp[f"sw{i}"])
        SW[i, :, :, 0, :] = np.ascontiguousarray(sw.real).transpose(0, 2, 1)
        SW[i, :, :, 1, :] = np.ascontiguousarray(sw.imag).transpose(0, 2, 1)
    cg1 = inp["cg1_w"].astype(np.float32).reshape(WIDTH, M_CHEB, L_SEG, 9, 2, 256)
    G2 = cg1.sum(axis=2).reshape(WIDTH, 72, 256)
    lb = np.stack([inp[f"w{i}_b"].astype(np.float32) for i in range(3)], 1)
    common = {
        "fc0w": _fc0_blk(fc0_w),
        "Fb": F_sb, "CHb": CH_sb, "Cinv": Ci,
        "WT": WT.astype(BF),
        "SW": SW.astype(BF),
        "G2": G2.astype(BF),
        "fc1w": inp["fc1_w"].astype(np.float32).astype(BF),
        "fc2w": inp["fc2_w"].astype(np.float32).astype(BF),
        "cg2h": inp["cg2_w"].astype(np.float32).reshape(2, 128, 128)
                .transpose(1, 0, 2).copy().astype(BF),
        "fc0b": inp["fc0_b"].astype(np.float32).reshape(128, 1),
        "lb": lb,
        "w3b": (inp["w3_b"].astype(np.float32)
                + inp["cg2_b"].astype(np.float32)).reshape(128, 1),
        "fc1b": inp["fc1_b"].astype(np.float32).reshape(128, 1),
        "cg1b": np.broadcast_to(inp["cg1_b"].astype(np.float32), (4, 256)).copy(),
    }
    per_core = []
    for c in range(NCORES):
        xg = np.empty((2 * BPC, S), np.float32)
        for b in range(BPC):
            xg[2 * b] = x[c * BPC + b, :, 0]
            xg[2 * b + 1] = grid
        m = dict(common)
        m["xg"] = xg.astype(BF)
        per_core.append(m)
    fc2b = float(inp["fc2_b"].astype(np.float32).reshape(-1)[0])
    return per_core, fc2b


# ---------------------------------------------------------------------------

def kernel(**inputs) -> np.ndarray:
    from concourse import bass_utils
    fast = (np.count_nonzero(np.asarray(inputs["cg2_w"])) == 0)
    if fast:
        has_fc0b = bool(np.count_nonzero(np.asarray(inputs["fc0_b"])))
        key = ("v2", has_fc0b)
        per_core, fc2b = _prep_v2(inputs, has_fc0b)
        if key not in _CACHE:
            _CACHE[key] = _build_v2(has_fc0b)
    else:
        key = ("base",)
        per_core, fc2b = _prep_base(inputs)
        if key not in _CACHE:
            _CACHE[key] = _build_base()
    nc = _CACHE[key]
    res = bass_utils.run_bass_kernel_spmd(nc, per_core, core_ids=list(range(NCORES)))
    out = np.empty((B, S, 1), np.float32)
    for c in range(NCORES):
        r = res.results[c]["out"]
        if fast:  # [BPC, 128, 64]: s = t*128 + p
            r = r.transpose(0, 2, 1).reshape(BPC, S)
        out[c * BPC:(c + 1) * BPC, :, 0] = r
    return out + fc2b
